# revision 10
# baseline (speedup 1.0000x reference)
"""Trainium2 Bass kernel for nn_DecoderWithRelativePositionalAttentionLayer.

Sharding: pure data-parallel over batch (B=8 -> 8 NeuronCores, one batch
element per core, identical SPMD program, no collectives).

Key algorithmic move: the reference materializes rel = enc[ridx] as a
[Lq, Lq, D] tensor and pushes it through a [D, D] dense (38.7 GMAC); since
ridx = clip(i-j, -R, R) takes only 201 values and the causal mask kills
j > i, we project a [384, D] extended/reversed table once and realize the
per-(i, j) band with a DRAM "skew" bounce: rows written at pitch 385 and
read back at pitch 384 / offset 383 land row i's entry t at column
j = i - 383 + t.

Layouts: activations are feature-major [F, T] so weight matmuls need no
transposes (lhsT = native [in, out] weight k-tiles, rhs = activations).
Attention-0 scores run in [i, j] (softmax along the free dim, causal via
gpsimd affine_select, band added pre-exp); the probability tiles are
PE-transposed for the PV matmul. Attention-1 has no positional term and is
computed directly transposed [j, i], with the softmax normalizer taken from
an extra ones-column interleaved into the value matrix.

All matmul operands are float32r (TF32-like, full-rate on the PE array;
measured ~2.5e-4 per-matmul relative error on HW).
"""

import sys

sys.path.insert(0, "/opt/trn_rl_repo")

import contextlib
import numpy as np
import concourse.bass as bass
import concourse.mybir as mybir
import concourse.tile as tile
from concourse.bass_utils import run_bass_kernel_spmd
import bass_rust

F32 = mybir.dt.float32
F32R = mybir.dt.float32r
AF = mybir.ActivationFunctionType
OP = mybir.AluOpType
AX = mybir.AxisListType

B, LQ, LK, D, H, HID = 8, 384, 384, 512, 8, 2048
DIM = D // H
REL = 100
P = 128
NT = LQ // P   # 3 token tiles
ND = D // P    # 4 feature tiles
NH = HID // P  # 16 hidden tiles
SCALE = float(1.0 / np.sqrt(np.float32(DIM)))
G_WPITCH = LQ + 1           # 385: write pitch of the skew scratch
G_STRIDE = G_WPITCH * LQ    # per-head segment


def _split_multiwait_instructions(nc):
    """This toolchain's walrus accepts at most ONE sync wait per
    instruction, but Tile's tail drain aggregates several. Move extras onto
    same-engine nops placed immediately before the offending instruction."""
    counter = [0]

    def fresh_nop(engine, wait):
        counter[0] += 1
        nop = bass_rust.InstNoOp(name=f"WSPLIT-{counter[0]}", ins=[], outs=[])
        nop.engine = engine
        nop.sync_info = bass_rust.SyncInfo(on_wait=[wait], on_update=[])
        return nop

    for fn in nc.m.functions:
        for bb in fn.blocks:
            out = []
            changed = False
            for inst in bb.instructions:
                si = inst.sync_info
                if si is not None and len(si.on_wait) > 1:
                    waits = list(si.on_wait)
                    for w in waits[:-1]:
                        out.append(fresh_nop(inst.engine, w))
                    inst.sync_info = bass_rust.SyncInfo(
                        on_wait=[waits[-1]], on_update=list(si.on_update)
                    )
                    changed = True
                out.append(inst)
            if changed:
                bb.instructions = out


def _sinusoid_ext_rev():
    """encER[t] = enc[min(383 - t, REL) + REL]  -> [384, 512]."""
    pos = np.arange(-REL, REL + 1, dtype=np.float32)[:, None]
    i = np.arange(D // 2, dtype=np.float32)[None, :]
    ang = pos / np.power(np.float32(10000.0), 2.0 * i / np.float32(D))
    enc = np.concatenate([np.sin(ang), np.cos(ang)], axis=-1).astype(np.float32)
    o = (LQ - 1) - np.arange(LQ)
    return enc[np.minimum(o, REL) + REL]


def build_nc():
    nc = bass.Bass()

    def pin(name, shape, dt=F32R):
        return nc.declare_dram_parameter(name, list(shape), dt, isOutput=False)

    q_in = pin("q_in", [LQ, D])
    v_in = pin("v_in", [LK, D])
    qmadd = pin("qmadd", [1, LQ], F32)   # (qm-1)*8e9, pre-scale additive
    vmadd = pin("vmadd", [P, NT], F32)   # (vm-1)*1e9, post-scale additive

    w = {}
    for nm, shp in [
        ("b0_W", [D, HID]), ("q0_W", [HID, D]), ("ke0_W", [HID, D]),
        ("kv0_W", [HID, D]), ("kr0_W", [D, D]), ("ab0_W", [D, H]),
        ("ab1_W", [D, H]), ("b1_W1", [D, HID]), ("b1_W2", [HID, D]),
        ("b2_W1", [D, HID]), ("b2_W2k", [HID, D]), ("b2_W2v", [HID, D]),
        ("b3_W1", [D, HID]), ("b3_W2", [HID, D]),
    ]:
        w[nm] = pin(nm, shp)
    for nm, nt in [
        ("b0_b", NH), ("q0_b", ND), ("ke0_b", ND), ("kr0_b", ND),
        ("b1_b1", NH), ("b1_b2", ND), ("b2_b1", NH), ("b2_b2k", ND),
        ("b3_b1", NH), ("b3_b2", ND),
        ("ln0_g", ND), ("ln0_b", ND), ("ln1_g", ND), ("ln1_b", ND),
        ("ln2_g", ND), ("ln2_b", ND), ("ln3_g", ND), ("ln3_b", ND),
    ]:
        w[nm] = pin(nm, [P, nt], F32)
    w["ab0_b"] = pin("ab0_b", [H, 1], F32)
    w["ab1_b"] = pin("ab1_b", [H, 1], F32)
    w["kv0_b_row"] = pin("kv0_b_row", [1, D])
    w["b2_b2v_row"] = pin("b2_b2v_row", [1, D])

    encR = pin("encR", [D, LQ])
    idn = pin("idn", [P, P])
    ones_r = pin("ones_r", [1, P])
    ones_c = pin("ones_c", [P, 1])
    ones8 = pin("ones8", [P, H])
    epsc = pin("epsc", [P, 1], F32)

    out = nc.declare_dram_parameter("out", [LQ, D], F32, isOutput=True)
    g_scr = nc.dram_tensor("g_scratch", [H * G_STRIDE + 1024], F32)

    with tile.TileContext(nc) as tc, nc.allow_low_precision(
        reason="float32r dataflow is intentional (TF32-like matmul operands)"
    ):
        _emit(
            nc, tc, q_in, v_in, qmadd, vmadd, w, encR, idn, ones_r, ones_c,
            ones8, epsc, out, g_scr,
        )
    _split_multiwait_instructions(nc)
    return nc


def _emit(nc, tc, q_in, v_in, qmadd, vmadd, w, encR, idn, ones_r, ones_c,
          ones8, epsc, out, g_scr):
    ctx = contextlib.ExitStack()

    def pool(name, bufs, **kw):
        return ctx.enter_context(tc.tile_pool(name=name, bufs=bufs, **kw))

    const = pool("const", 1)
    wbig = pool("wbig", 4)       # [128, 2048] k-tiles, full block resident
    wsm = pool("wsm", 4)         # [128, <=512] k-tiles, streaming (k-outer)
    act = pool("act", 2)         # token-major staging
    fm_ln = pool("fm_ln", 4)     # LN output streams (ln0->ln1->ln2->ln3)
    fm_raw = pool("fm_raw", 4)   # raw queries, feature-major
    attA = pool("attA", 8)       # qp, kep -> y, k1
    kvA = pool("kvA", 3)         # kv_tok -> v1i
    resid = pool("resid", 8)     # ep -> q1 -> q2 -> q3
    hid = pool("hid", 18)        # hidden tiles + LN scratch
    soft = pool("soft", 2)
    pt_pool = pool("pt", 2)
    sm = pool("sm", 1)
    ps = pool("ps", 4, space="PSUM")
    ps_t = pool("ps_t", 2, space="PSUM")
    ps_small = pool("ps_small", 2, space="PSUM")

    # ---- constants
    identity = const.tile([P, P], F32R)
    nc.sync.dma_start(identity[:], idn[:])
    ones1 = const.tile([1, P], F32R)
    nc.sync.dma_start(ones1[:], ones_r[:])
    onescol = const.tile([P, 1], F32R)
    nc.sync.dma_start(onescol[:], ones_c[:])
    ones_col8 = const.tile([P, H], F32R)
    nc.sync.dma_start(ones_col8[:], ones8[:])
    eps_t = const.tile([P, 1], F32)
    nc.sync.dma_start(eps_t[:], epsc[:])
    encR_sb = const.tile([P, ND, LQ], F32R)
    nc.sync.dma_start(encR_sb[:], encR[:].rearrange("(k p) t -> p k t", p=P))
    qmadd_sb = const.tile([1, LQ], F32)
    nc.sync.dma_start(qmadd_sb[:], qmadd[:])
    vmadd_sb = const.tile([P, NT], F32)
    nc.sync.dma_start(vmadd_sb[:], vmadd[:])

    def load_vec(name):
        t = const.tile(list(w[name].shape), F32, name=f"v_{name}")
        nc.sync.dma_start(t[:], w[name][:])
        return t

    vecs = {
        nm: load_vec(nm)
        for nm in [
            "b0_b", "q0_b", "ke0_b", "kr0_b", "b1_b1", "b1_b2", "b2_b1",
            "b2_b2k", "b3_b1", "b3_b2", "ln0_g", "ln0_b", "ln1_g", "ln1_b",
            "ln2_g", "ln2_b", "ln3_g", "ln3_b", "ab0_b", "ab1_b",
        ]
    }
    kv0_b_row = const.tile([1, D], F32R)
    nc.sync.dma_start(kv0_b_row[:], w["kv0_b_row"][:])
    b2_b2v_row = const.tile([1, D], F32R)
    nc.sync.dma_start(b2_b2v_row[:], w["b2_b2v_row"][:])

    def w_ktile(name, k, ncols, p, tag):
        t = p.tile([P, ncols], F32R, tag=tag, name=f"{name}k{k}")
        nc.sync.dma_start(t[:], w[name][k * P : (k + 1) * P, :])
        return t

    # ---- dense helpers -------------------------------------------------
    def dense_wide(x_tiles, wname, bias_vec, out_tag):
        """[D -> HID] with relu. m-outer over 16 output tiles; the 4 wide
        k-tiles stay resident (wbig bufs=4)."""
        wk = [w_ktile(wname, k, HID, wbig, "w2048") for k in range(ND)]
        outs = []
        for m in range(NH):
            pso = ps.tile([P, LQ], F32, tag="mm")
            for k in range(ND):
                nc.tensor.matmul(
                    pso[:], wk[k][:, m * P : (m + 1) * P], x_tiles[k][:],
                    start=(k == 0), stop=(k == ND - 1),
                )
            o = hid.tile([P, LQ], F32R, tag="hidden", name=f"hw{m}")
            nc.scalar.activation(o[:], pso[:], AF.Relu, bias=bias_vec[:, m : m + 1])
            outs.append(o)
        return outs

    def dense_narrow(x_tiles, wname, n_in, bias_vec, out_pool, out_tag,
                     relu=False, evict=None):
        """[n_in -> 512] feature-major. k-outer so weight k-tiles stream
        with bufs=4; the 4 output psums accumulate concurrently."""
        nk = n_in // P
        psos = [ps.tile([P, LQ], F32, tag="mm", name=f"dnps{m}") for m in range(ND)]
        for k in range(nk):
            wk = w_ktile(wname, k, D, wsm, "w512")
            for m in range(ND):
                nc.tensor.matmul(
                    psos[m][:], wk[:, m * P : (m + 1) * P], x_tiles[k][:],
                    start=(k == 0), stop=(k == nk - 1),
                )
        outs = []
        for m in range(ND):
            o = out_pool.tile([P, LQ], F32R, tag=out_tag, name=f"dn{m}")
            if evict is not None:
                evict(o, psos[m], m)
            elif relu:
                nc.scalar.activation(
                    o[:], psos[m][:], AF.Relu, bias=bias_vec[:, m : m + 1]
                )
            else:
                nc.vector.tensor_scalar(
                    o[:], psos[m][:], bias_vec[:, m : m + 1], None, OP.add
                )
            outs.append(o)
        return outs

    def dense_tok(x_tiles, wname, bias_row, post):
        """[HID -> 512] token-major out: for each token tile jt a [128, 512]
        psum accumulates x[k][:, jt] @ W[k]; bias added via K=1 ones matmul.
        post(jt, psum) consumes the result."""
        psos = [ps.tile([P, D], F32, tag="mm", name=f"dtps{j}") for j in range(NT)]
        for k in range(NH):
            wk = w_ktile(wname, k, D, wsm, "w512")
            for jt in range(NT):
                nc.tensor.matmul(
                    psos[jt][:], x_tiles[k][:, jt * P : (jt + 1) * P], wk[:],
                    start=(k == 0), stop=False,
                )
        for jt in range(NT):
            nc.tensor.matmul(psos[jt][:], ones1[:], bias_row[:], start=False, stop=True)
            post(jt, psos[jt])

    # ---- layernorm helpers ---------------------------------------------
    def ln_tok_to_fm(src_dram, g_vec, b_vec, want_raw=False):
        fm_tiles = [fm_ln.tile([P, LQ], F32R, tag="lnstream", name=f"lnfm{c}") for c in range(ND)]
        raw_tiles = (
            [fm_raw.tile([P, LQ], F32R, tag="qraw", name=f"qraw{c}") for c in range(ND)]
            if want_raw else None
        )
        for it in range(NT):
            xt = act.tile([P, D], F32R, tag="xt_in")
            nc.sync.dma_start(xt[:], src_dram[it * P : (it + 1) * P, :])
            stats = sm.tile([P, nc.vector.BN_STATS_DIM], F32, tag="bnst", bufs=2)
            nc.vector.bn_stats(stats[:], xt[:].bitcast(F32))
            mv = sm.tile([P, nc.vector.BN_AGGR_DIM], F32, tag="bnmv", bufs=2)
            nc.vector.bn_aggr(mv[:], stats[:])
            sd = sm.tile([P, 1], F32, tag="bnsd", bufs=2)
            nc.scalar.activation(sd[:], mv[:, 1:2], AF.Sqrt, bias=eps_t[:])
            nc.vector.reciprocal(sd[:], sd[:])
            xn = act.tile([P, D], F32R, tag="xt_n")
            nc.vector.tensor_scalar(
                xn[:], xt[:], mv[:, 0:1], sd[:], OP.subtract, OP.mult
            )
            for c in range(ND):
                tp = ps_t.tile([P, P], F32R, tag="tps")
                nc.tensor.transpose(tp[:], xn[:, c * P : (c + 1) * P], identity[:])
                nc.vector.tensor_scalar(
                    fm_tiles[c][:, it * P : (it + 1) * P], tp[:],
                    g_vec[:, c : c + 1], b_vec[:, c : c + 1], OP.mult, OP.add,
                )
                if raw_tiles is not None:
                    tpr = ps_t.tile([P, P], F32R, tag="tps")
                    nc.tensor.transpose(tpr[:], xt[:, c * P : (c + 1) * P], identity[:])
                    nc.vector.tensor_copy(raw_tiles[c][:, it * P : (it + 1) * P], tpr[:])
        return fm_tiles, raw_tiles

    def ln_fm(x_tiles, g_vec, b_vec):
        """LayerNorm over the partition (feature) direction of feature-major
        tiles, via ones-matmul sums and a PE broadcast."""
        s_ps = ps_small.tile([1, LQ], F32, tag="small")
        s2_ps = ps_small.tile([1, LQ], F32, tag="small")
        for c in range(ND):
            nc.tensor.matmul(
                s_ps[:], onescol[:], x_tiles[c][:],
                start=(c == 0), stop=(c == ND - 1),
            )
        sqs = []
        for c in range(ND):
            sq = hid.tile([P, LQ], F32R, tag="hidden", name=f"sq{c}")
            nc.scalar.activation(sq[:], x_tiles[c][:], AF.Square)
            sqs.append(sq)
        for c in range(ND):
            nc.tensor.matmul(
                s2_ps[:], onescol[:], sqs[c][:],
                start=(c == 0), stop=(c == ND - 1),
            )
        mu = sm.tile([1, LQ], F32R, tag="lnmu")
        nc.vector.tensor_scalar(mu[:], s_ps[:], 1.0 / D, None, OP.mult)
        var = sm.tile([1, LQ], F32, tag="lnvar")
        nc.vector.tensor_scalar(var[:], s2_ps[:], 1.0 / D, None, OP.mult)
        m2 = sm.tile([1, LQ], F32, tag="lnm2")
        nc.vector.tensor_tensor(m2[:], mu[:].bitcast(F32), mu[:].bitcast(F32), OP.mult)
        nc.vector.tensor_tensor(var[:], var[:], m2[:], OP.subtract)
        sd = sm.tile([1, LQ], F32R, tag="lnsd")
        nc.scalar.activation(sd[:], var[:], AF.Sqrt, bias=eps_t[0:1, 0:1])
        nc.vector.reciprocal(sd[:], sd[:])
        mub = ps_small.tile([P, LQ], F32, tag="small")
        nc.tensor.matmul(mub[:], ones1[:], mu[:], start=True, stop=True)
        sdb = ps_small.tile([P, LQ], F32, tag="small")
        nc.tensor.matmul(sdb[:], ones1[:], sd[:], start=True, stop=True)
        outs = []
        for c in range(ND):
            t1 = hid.tile([P, LQ], F32, tag="hidden", name=f"lt{c}")
            nc.vector.tensor_tensor(t1[:], x_tiles[c][:].bitcast(F32), mub[:], OP.subtract)
            nc.vector.tensor_tensor(t1[:], t1[:], sdb[:], OP.mult)
            o = fm_ln.tile([P, LQ], F32R, tag="lnstream", name=f"lno{c}")
            nc.vector.tensor_scalar(
                o[:], t1[:], g_vec[:, c : c + 1], b_vec[:, c : c + 1],
                OP.mult, OP.add,
            )
            outs.append(o)
        return outs

    def head_slice(tiles, h):
        return tiles[h // 2][64 * (h % 2) : 64 * (h % 2) + 64, :]

    # ================= block0 =================
    lnq, q_fm = ln_tok_to_fm(q_in, vecs["ln0_g"], vecs["ln0_b"], want_raw=True)
    x_tiles = dense_wide(lnq, "b0_W", vecs["b0_b"], "x")

    qp = dense_narrow(x_tiles, "q0_W", HID, vecs["q0_b"], attA, "attA")
    kep = dense_narrow(x_tiles, "ke0_W", HID, vecs["ke0_b"], attA, "attA")

    kv_tok = []

    def kv_post(jt, psv):
        t = kvA.tile([P, H * 65], F32R, tag="kvA")
        nc.vector.tensor_copy(t[:, 0:D], psv[:])
        kv_tok.append(t)

    dense_tok(x_tiles, "kv0_W", kv0_b_row, kv_post)

    # ---- rel tables
    ep_fm = []
    krk = [w_ktile("kr0_W", k, D, wsm, "w512") for k in range(ND)]
    for m in range(ND):
        pse = ps.tile([P, LQ], F32, tag="mm")
        for k in range(ND):
            nc.tensor.matmul(
                pse[:], krk[k][:, m * P : (m + 1) * P], encR_sb[:, k, :],
                start=(k == 0), stop=(k == ND - 1),
            )
        o = resid.tile([P, LQ], F32R, tag="resid", name=f"ep{m}")
        nc.vector.tensor_scalar(o[:], pse[:], vecs["kr0_b"][:, m : m + 1], None, OP.add)
        ep_fm.append(o)

    ab1k = [w_ktile("ab1_W", k, H, wsm, "w8") for k in range(ND)]
    gam_ps = ps_small.tile([H, LQ], F32, tag="small")
    for k in range(ND):
        nc.tensor.matmul(
            gam_ps[:], ab1k[k][:], ep_fm[k][:], start=(k == 0), stop=(k == ND - 1)
        )
    gam = sm.tile([H, LQ], F32R, tag="gam")
    nc.vector.tensor_scalar(gam[:], gam_ps[:], vecs["ab1_b"][:], None, OP.add)

    ab0k = [w_ktile("ab0_W", k, H, wsm, "w8") for k in range(ND)]
    bke_ps = ps_small.tile([H, LQ], F32, tag="small")
    for k in range(ND):
        nc.tensor.matmul(
            bke_ps[:], ab0k[k][:], kep[k][:], start=(k == 0), stop=(k == ND - 1)
        )
    bke = sm.tile([H, LQ], F32R, tag="bke")
    nc.vector.tensor_scalar(bke[:], bke_ps[:], vecs["ab0_b"][:], None, OP.add)

    gam_all = sm.tile([1, H * LQ], F32R, tag="gamall")
    bkem_all = sm.tile([1, H * LQ], F32R, tag="bkemall")
    for h in range(H):
        nc.sync.dma_start(gam_all[:, h * LQ : (h + 1) * LQ], gam[h : h + 1, :])
        nc.sync.dma_start(bkem_all[:, h * LQ : (h + 1) * LQ], bke[h : h + 1, :])
        nc.vector.tensor_tensor(
            bkem_all[:, h * LQ : (h + 1) * LQ],
            bkem_all[:, h * LQ : (h + 1) * LQ], qmadd_sb[:], OP.add,
        )
    gam_rows = [gam_all[:, h * LQ : (h + 1) * LQ] for h in range(H)]
    bkem_rows = [bkem_all[:, h * LQ : (h + 1) * LQ] for h in range(H)]

    # ================= attention 0 =================
    att_q1 = [resid.tile([P, LQ], F32R, tag="resid", name=f"q1_{c}") for c in range(ND)]
    for h in range(H):
        qh = head_slice(qp, h)
        keh = head_slice(kep, h)
        eph = head_slice(ep_fm, h)
        pn_tiles = []
        for it in range(NT):
            psb = ps.tile([P, LQ], F32, tag="mm")
            nc.tensor.matmul(
                psb[:], qh[:, it * P : (it + 1) * P], eph[:],
                start=True, stop=False,
            )
            nc.tensor.matmul(psb[:], ones1[:], gam_rows[h], start=False, stop=True)
            band_w = soft.tile([P, G_WPITCH], F32, tag="bandw", name="band_w")
            nc.vector.tensor_copy(band_w[:, 0:LQ], psb[:, 0:LQ])
            nc.vector.memset(band_w[:, LQ : LQ + 1], 0.0)
            gw = bass.AP(
                tensor=g_scr,
                offset=h * G_STRIDE + it * P * G_WPITCH,
                ap=[[G_WPITCH, P], [1, G_WPITCH]],
            )
            nc.sync.dma_start(gw, band_w[:])
            band = soft.tile([P, LQ], F32, tag="band")
            gr_ap = bass.AP(
                tensor=g_scr,
                offset=h * G_STRIDE + (LQ - 1) + it * P * LQ,
                ap=[[LQ, P], [1, LQ]],
            )
            nc.sync.dma_start(band[:], gr_ap)

            pss = ps.tile([P, LQ], F32, tag="mm")
            nc.tensor.matmul(
                pss[:], qh[:, it * P : (it + 1) * P], keh[:], start=True, stop=False
            )
            nc.tensor.matmul(pss[:], ones1[:], bkem_rows[h], start=False, stop=True)
            nc.vector.tensor_tensor(band[:], pss[:], band[:], OP.add)
            p_t = soft.tile([P, LQ], F32R, tag=f"p{it}", name="p_t")
            nc.scalar.activation(p_t[:], band[:], AF.Exp, scale=SCALE)
            nc.gpsimd.affine_select(
                out=p_t[:], in_=p_t[:], compare_op=OP.is_ge, fill=0.0,
                base=it * P, pattern=[[-1, LQ]], channel_multiplier=1,
            )
            z = sm.tile([P, 1], F32, tag="z0", bufs=2)
            nc.vector.reduce_sum(z[:], p_t[:].bitcast(F32), axis=AX.X)
            nc.vector.reciprocal(z[:], z[:])
            nc.vector.tensor_scalar(p_t[:], p_t[:], z[:], None, OP.mult)
            pn_tiles.append(p_t)

        pt_tiles = [pt_pool.tile([P, LQ], F32R, tag=f"pt{jt}", name=f"pt{jt}") for jt in range(NT)]
        for it in range(NT):
            for jt in range(it + 1):
                tp = ps_t.tile([P, P], F32R, tag="tps")
                nc.tensor.transpose(
                    tp[:], pn_tiles[it][:, jt * P : (jt + 1) * P], identity[:]
                )
                nc.vector.tensor_copy(pt_tiles[jt][:, it * P : (it + 1) * P], tp[:])
        for it in range(NT):
            pso = ps_small.tile([64, P], F32, tag="small")
            for jt in range(it + 1):
                nc.tensor.matmul(
                    pso[:], kv_tok[jt][:, 64 * h : 64 * h + 64],
                    pt_tiles[jt][:, it * P : (it + 1) * P],
                    start=(jt == 0), stop=(jt == it),
                )
            dst = att_q1[h // 2][
                64 * (h % 2) : 64 * (h % 2) + 64, it * P : (it + 1) * P
            ]
            src_q = q_fm[h // 2][
                64 * (h % 2) : 64 * (h % 2) + 64, it * P : (it + 1) * P
            ]
            nc.vector.tensor_tensor(dst, pso[:], src_q, OP.add)

    # ================= block1 -> y =================
    lnq1 = ln_fm(att_q1, vecs["ln1_g"], vecs["ln1_b"])
    h1 = dense_wide(lnq1, "b1_W1", vecs["b1_b1"], "h1")
    y_fm = dense_narrow(h1, "b1_W2", HID, vecs["b1_b2"], attA, "attA")

    # ================= block2 -> k1 (fm), v1 (tok + ones col) ============
    lnv, _ = ln_tok_to_fm(v_in, vecs["ln2_g"], vecs["ln2_b"])
    h2 = dense_wide(lnv, "b2_W1", vecs["b2_b1"], "h2")
    k1_fm = dense_narrow(h2, "b2_W2k", HID, vecs["b2_b2k"], attA, "attA")

    v1i = []

    def v1_post(jt, psv):
        t = kvA.tile([P, H * 65], F32R, tag="kvA")
        tv = t[:].rearrange("p (h x) -> p h x", h=H)
        nc.vector.tensor_copy(
            tv[:, :, 0:64], psv[:].rearrange("p (h d) -> p h d", h=H)
        )
        nc.vector.tensor_copy(
            tv[:, :, 64:65], ones_col8[:].rearrange("p (h x) -> p h x", x=1)
        )
        v1i.append(t)

    dense_tok(h2, "b2_W2v", b2_b2v_row, v1_post)

    # ================= attention 1 (transposed) =================
    att_q2 = [resid.tile([P, LQ], F32R, tag="resid", name=f"q2_{c}") for c in range(ND)]
    for h in range(H):
        yh = head_slice(y_fm, h)
        k1h = head_slice(k1_fm, h)
        p1t_tiles = []
        for jt in range(NT):
            pss = ps.tile([P, LQ], F32, tag="mm")
            nc.tensor.matmul(
                pss[:], k1h[:, jt * P : (jt + 1) * P], yh[:], start=True, stop=True
            )
            p1 = soft.tile([P, LQ], F32R, tag=f"p{jt}", name="p1")
            nc.scalar.activation(
                p1[:], pss[:], AF.Exp, scale=SCALE, bias=vmadd_sb[:, jt : jt + 1]
            )
            p1t_tiles.append(p1)
        pso = ps.tile([65, LQ], F32, tag="mm")
        for jt in range(NT):
            nc.tensor.matmul(
                pso[:], v1i[jt][:, 65 * h : 65 * h + 65], p1t_tiles[jt][:],
                start=(jt == 0), stop=(jt == NT - 1),
            )
        rz = sm.tile([1, LQ], F32R, tag="rz1", bufs=2)
        nc.vector.reciprocal(rz[:], pso[64:65, :])
        psb = ps_small.tile([64, LQ], F32, tag="small")
        nc.tensor.matmul(psb[:], ones1[:, 0:64], rz[:], start=True, stop=True)
        o1 = soft.tile([P, LQ], F32, tag="o1")
        o1s = o1[64 * (h % 2) : 64 * (h % 2) + 64, :]
        nc.scalar.activation(o1s, pso[0:64, :], AF.Copy)
        nc.vector.tensor_tensor(o1s, o1s, psb[:], OP.mult)
        dst = att_q2[h // 2][64 * (h % 2) : 64 * (h % 2) + 64, :]
        src_q = att_q1[h // 2][64 * (h % 2) : 64 * (h % 2) + 64, :]
        nc.vector.tensor_tensor(dst, o1s, src_q, OP.add)

    # ================= block3 residual FFN =================
    lnq3 = ln_fm(att_q2, vecs["ln3_g"], vecs["ln3_b"])
    h3 = dense_wide(lnq3, "b3_W1", vecs["b3_b1"], "h3")

    def b3_evict(o, pso, m):
        nc.vector.scalar_tensor_tensor(
            o[:], pso[:], vecs["b3_b2"][:, m : m + 1], att_q2[m][:],
            OP.add, OP.add,
        )

    q3 = dense_narrow(h3, "b3_W2", HID, None, resid, "resid", evict=b3_evict)

    # ---- back to token-major, store
    for it in range(NT):
        ot = act.tile([P, D], F32, tag="out_tok")
        for c in range(ND):
            tp = ps_t.tile([P, P], F32R, tag="tps")
            nc.tensor.transpose(tp[:], q3[c][:, it * P : (it + 1) * P], identity[:])
            nc.vector.tensor_copy(ot[:, c * P : (c + 1) * P], tp[:].bitcast(F32))
        nc.sync.dma_start(out[it * P : (it + 1) * P, :], ot[:])

    ctx.close()


_NC = None


def _get_nc():
    global _NC
    if _NC is None:
        _NC = build_nc()
    return _NC


def _build_in_maps(queries, values, queries_mask, values_mask, params):
    p = params
    f32 = np.float32

    def fmvec(v, n_tiles):
        return np.ascontiguousarray(np.asarray(v, f32).reshape(n_tiles, P).T)

    w2 = np.asarray(p["b2_W2"], f32).reshape(HID, H, 2, DIM)
    b2b = np.asarray(p["b2_b2"], f32).reshape(H, 2, DIM)

    shared = {
        "b0_W": np.asarray(p["b0_W"], f32), "q0_W": np.asarray(p["q0_W"], f32),
        "ke0_W": np.asarray(p["ke0_W"], f32), "kv0_W": np.asarray(p["kv0_W"], f32),
        "kr0_W": np.asarray(p["kr0_W"], f32), "ab0_W": np.asarray(p["ab0_W"], f32),
        "ab1_W": np.asarray(p["ab1_W"], f32),
        "b1_W1": np.asarray(p["b1_W1"], f32), "b1_W2": np.asarray(p["b1_W2"], f32),
        "b2_W1": np.asarray(p["b2_W1"], f32),
        "b2_W2k": np.ascontiguousarray(w2[:, :, 0, :].reshape(HID, D)),
        "b2_W2v": np.ascontiguousarray(w2[:, :, 1, :].reshape(HID, D)),
        "b3_W1": np.asarray(p["b3_W1"], f32), "b3_W2": np.asarray(p["b3_W2"], f32),
        "b0_b": fmvec(p["b0_b"], NH), "q0_b": fmvec(p["q0_b"], ND),
        "ke0_b": fmvec(p["ke0_b"], ND), "kr0_b": fmvec(p["kr0_b"], ND),
        "b1_b1": fmvec(p["b1_b1"], NH), "b1_b2": fmvec(p["b1_b2"], ND),
        "b2_b1": fmvec(p["b2_b1"], NH),
        "b2_b2k": fmvec(b2b[:, 0, :].reshape(D), ND),
        "b3_b1": fmvec(p["b3_b1"], NH), "b3_b2": fmvec(p["b3_b2"], ND),
        "ln0_g": fmvec(p["ln0_g"], ND), "ln0_b": fmvec(p["ln0_b"], ND),
        "ln1_g": fmvec(p["ln1_g"], ND), "ln1_b": fmvec(p["ln1_b"], ND),
        "ln2_g": fmvec(p["ln2_g"], ND), "ln2_b": fmvec(p["ln2_b"], ND),
        "ln3_g": fmvec(p["ln3_g"], ND), "ln3_b": fmvec(p["ln3_b"], ND),
        "ab0_b": np.asarray(p["ab0_b"], f32).reshape(H, 1),
        "ab1_b": np.asarray(p["ab1_b"], f32).reshape(H, 1),
        "kv0_b_row": np.asarray(p["kv0_b"], f32).reshape(1, D),
        "b2_b2v_row": np.ascontiguousarray(b2b[:, 1, :].reshape(1, D)),
        "encR": np.ascontiguousarray(_sinusoid_ext_rev().T),
        "idn": np.eye(P, dtype=f32),
        "ones_r": np.ones((1, P), f32),
        "ones_c": np.ones((P, 1), f32),
        "ones8": np.ones((P, H), f32),
        "epsc": np.full((P, 1), 1e-3, f32),
    }

    qm = np.asarray(queries_mask, f32)
    vm = np.asarray(values_mask, f32)
    in_maps = []
    for c in range(B):
        m = dict(shared)
        m["q_in"] = np.ascontiguousarray(np.asarray(queries[c], f32))
        m["v_in"] = np.ascontiguousarray(np.asarray(values[c], f32))
        m["qmadd"] = ((qm[c] - 1.0) * 8e9).reshape(1, LQ).astype(f32)
        m["vmadd"] = np.ascontiguousarray(
            ((vm[c] - 1.0) * 1e9).reshape(NT, P).T
        ).astype(f32)
        in_maps.append(m)
    return in_maps


def kernel(queries, values, queries_mask, values_mask, params):
    in_maps = _build_in_maps(queries, values, queries_mask, values_mask, params)
    nc = _get_nc()
    res = run_bass_kernel_spmd(nc, in_maps, core_ids=list(range(B)))
    return np.stack([res.results[c]["out"] for c in range(B)], axis=0).astype(
        np.float32
    )


def kernel_profiled(queries, values, queries_mask, values_mask, params,
                    tmpdir=None):
    """Same as kernel() but with NTFF tracing; returns (output, results)."""
    import kernel as _self  # works both as module and as __main__ helper

    in_maps = _build_in_maps(queries, values, queries_mask, values_mask, params)
    nc = _get_nc()
    res = run_bass_kernel_spmd(
        nc, in_maps, core_ids=list(range(B)), trace=True, tmpdir=tmpdir
    )
    out = np.stack([res.results[c]["out"] for c in range(B)], axis=0).astype(
        np.float32
    )
    return out, res


# revision 12
# speedup vs baseline: 1.1129x; 1.1129x over previous
"""Trainium2 Bass kernel for nn_DecoderWithRelativePositionalAttentionLayer.

Sharding: pure data-parallel over batch (B=8 -> 8 NeuronCores, one batch
element per core, identical SPMD program, no collectives).

Key algorithmic move: the reference materializes rel = enc[ridx] as a
[Lq, Lq, D] tensor and pushes it through a [D, D] dense (38.7 GMAC); since
ridx = clip(i-j, -R, R) takes only 201 values and the causal mask kills
j > i, we project a [384, D] extended/reversed table once and realize the
per-(i, j) band with a DRAM "skew" bounce: rows written at pitch 385 and
read back at pitch 384 / offset 383 land row i's entry t at column
j = i - 383 + t.

Layouts: activations are feature-major [F, T] so weight matmuls need no
transposes (lhsT = native [in, out] weight k-tiles, rhs = activations).
Attention-0 scores run in [i, j] (softmax along the free dim, causal via
gpsimd affine_select, band added pre-exp); the probability tiles are
PE-transposed for the PV matmul. Attention-1 has no positional term and is
computed directly transposed [j, i], with the softmax normalizer taken from
an extra ones-column interleaved into the value matrix.

All matmul operands are float32r (TF32-like, full-rate on the PE array;
measured ~2.5e-4 per-matmul relative error on HW).
"""

import sys

sys.path.insert(0, "/opt/trn_rl_repo")

import contextlib
import numpy as np
import concourse.bass as bass
import concourse.mybir as mybir
import concourse.tile as tile
from concourse.bass_utils import run_bass_kernel_spmd
import bass_rust

F32 = mybir.dt.float32
F32R = mybir.dt.float32r
AF = mybir.ActivationFunctionType
OP = mybir.AluOpType
AX = mybir.AxisListType

B, LQ, LK, D, H, HID = 8, 384, 384, 512, 8, 2048
DIM = D // H
REL = 100
P = 128
NT = LQ // P   # 3 token tiles
ND = D // P    # 4 feature tiles
NH = HID // P  # 16 hidden tiles
SCALE = float(1.0 / np.sqrt(np.float32(DIM)))
G_WPITCH = LQ + 1           # 385: write pitch of the skew scratch
G_STRIDE = G_WPITCH * LQ    # per-head segment


def _split_multiwait_instructions(nc):
    """This toolchain's walrus accepts at most ONE sync wait per
    instruction, but Tile's tail drain aggregates several. Move extras onto
    same-engine nops placed immediately before the offending instruction."""
    counter = [0]

    def fresh_nop(engine, wait):
        counter[0] += 1
        nop = bass_rust.InstNoOp(name=f"WSPLIT-{counter[0]}", ins=[], outs=[])
        nop.engine = engine
        nop.sync_info = bass_rust.SyncInfo(on_wait=[wait], on_update=[])
        return nop

    for fn in nc.m.functions:
        for bb in fn.blocks:
            out = []
            changed = False
            for inst in bb.instructions:
                si = inst.sync_info
                if si is not None and len(si.on_wait) > 1:
                    waits = list(si.on_wait)
                    for w in waits[:-1]:
                        out.append(fresh_nop(inst.engine, w))
                    inst.sync_info = bass_rust.SyncInfo(
                        on_wait=[waits[-1]], on_update=list(si.on_update)
                    )
                    changed = True
                out.append(inst)
            if changed:
                bb.instructions = out


def _sinusoid_ext_rev():
    """encER[t] = enc[min(383 - t, REL) + REL]  -> [384, 512]."""
    pos = np.arange(-REL, REL + 1, dtype=np.float32)[:, None]
    i = np.arange(D // 2, dtype=np.float32)[None, :]
    ang = pos / np.power(np.float32(10000.0), 2.0 * i / np.float32(D))
    enc = np.concatenate([np.sin(ang), np.cos(ang)], axis=-1).astype(np.float32)
    o = (LQ - 1) - np.arange(LQ)
    return enc[np.minimum(o, REL) + REL]


def build_nc():
    nc = bass.Bass()

    def pin(name, shape, dt=F32R):
        return nc.declare_dram_parameter(name, list(shape), dt, isOutput=False)

    q_in = pin("q_in", [LQ, D])
    v_in = pin("v_in", [LK, D])
    qmadd = pin("qmadd", [1, LQ], F32)   # (qm-1)*8e9, pre-scale additive
    vmadd = pin("vmadd", [P, NT], F32)   # (vm-1)*1e9, post-scale additive

    w = {}
    for nm, shp in [
        ("b0_W", [D, HID]), ("q0_W", [HID, D]), ("ke0_W", [HID, D]),
        ("kv0_W", [HID, D]), ("kr0_W", [D, D]), ("ab0_W", [D, H]),
        ("ab1_W", [D, H]), ("b1_W1", [D, HID]), ("b1_W2", [HID, D]),
        ("b2_W1", [D, HID]), ("b2_W2k", [HID, D]), ("b2_W2v", [HID, D]),
        ("b3_W1", [D, HID]), ("b3_W2", [HID, D]),
    ]:
        w[nm] = pin(nm, shp)
    for nm, nt in [
        ("b0_b", NH), ("q0_b", ND), ("ke0_b", ND), ("kr0_b", ND),
        ("b1_b1", NH), ("b1_b2", ND), ("b2_b1", NH), ("b2_b2k", ND),
        ("b3_b1", NH), ("b3_b2", ND),
        ("ln0_g", ND), ("ln0_b", ND), ("ln1_g", ND), ("ln1_b", ND),
        ("ln2_g", ND), ("ln2_b", ND), ("ln3_g", ND), ("ln3_b", ND),
    ]:
        w[nm] = pin(nm, [P, nt], F32)
    w["ab0_b"] = pin("ab0_b", [H, 1], F32)
    w["ab1_b"] = pin("ab1_b", [H, 1], F32)
    w["kv0_b_full"] = pin("kv0_b_full", [P, D], F32)
    w["b2_b2v_full"] = pin("b2_b2v_full", [P, D], F32)

    encR = pin("encR", [D, LQ])
    idn = pin("idn", [P, P])
    ones_r = pin("ones_r", [1, P])
    ones_c = pin("ones_c", [P, 1])
    ones8 = pin("ones8", [P, H])
    epsc = pin("epsc", [P, 1], F32)

    out = nc.declare_dram_parameter("out", [LQ, D], F32, isOutput=True)
    g_scr = [
        nc.dram_tensor(f"g_scratch{h}", [G_STRIDE + 1024], mybir.dt.bfloat16)
        for h in range(H)
    ]

    with tile.TileContext(nc) as tc, nc.allow_low_precision(
        reason="float32r dataflow is intentional (TF32-like matmul operands)"
    ):
        _emit(
            nc, tc, q_in, v_in, qmadd, vmadd, w, encR, idn, ones_r, ones_c,
            ones8, epsc, out, g_scr,
        )
    _split_multiwait_instructions(nc)
    return nc


def _emit(nc, tc, q_in, v_in, qmadd, vmadd, w, encR, idn, ones_r, ones_c,
          ones8, epsc, out, g_scr):
    ctx = contextlib.ExitStack()

    def pool(name, bufs, **kw):
        return ctx.enter_context(tc.tile_pool(name=name, bufs=bufs, **kw))

    const = pool("const", 1)
    wbig = pool("wbig", 8)       # [128, 1024] half k-tiles
    wsm = pool("wsm", 6)         # [128, <=512] k-tiles, streaming (k-outer)
    act = pool("act", 2)         # token-major staging
    fm_ln = pool("fm_ln", 4)     # LN output streams (ln0->ln1->ln2->ln3)
    fm_raw = pool("fm_raw", 4)   # raw queries, feature-major
    attA = pool("attA", 8)       # qp, kep -> y, k1
    kvA = pool("kvA", 3)         # kv_tok -> v1i
    resid = pool("resid", 8)     # ep -> q1 -> q2 -> q3
    hid = pool("hid", 17)        # hidden tiles + LN scratch
    soft = pool("soft", 2)
    pt_pool = pool("pt", 2)
    sm = pool("sm", 1)
    ps = pool("ps", 4, space="PSUM")
    ps_t = pool("ps_t", 2, space="PSUM")
    ps_small = pool("ps_small", 2, space="PSUM")

    # ---- constants
    identity = const.tile([P, P], F32R)
    nc.sync.dma_start(identity[:], idn[:])
    ones1 = const.tile([1, P], F32R)
    nc.sync.dma_start(ones1[:], ones_r[:])
    onescol = const.tile([P, 1], F32R)
    nc.sync.dma_start(onescol[:], ones_c[:])
    ones_col8 = const.tile([P, H], F32R)
    nc.sync.dma_start(ones_col8[:], ones8[:])
    eps_t = const.tile([P, 1], F32)
    nc.sync.dma_start(eps_t[:], epsc[:])
    encR_sb = const.tile([P, ND, LQ], F32R)
    nc.sync.dma_start(encR_sb[:], encR[:].rearrange("(k p) t -> p k t", p=P))
    qmadd_sb = const.tile([1, LQ], F32)
    nc.sync.dma_start(qmadd_sb[:], qmadd[:])
    vmadd_sb = const.tile([P, NT], F32)
    nc.sync.dma_start(vmadd_sb[:], vmadd[:])

    def load_vec(name):
        t = const.tile(list(w[name].shape), F32, name=f"v_{name}")
        nc.sync.dma_start(t[:], w[name][:])
        return t

    vecs = {
        nm: load_vec(nm)
        for nm in [
            "b0_b", "q0_b", "ke0_b", "kr0_b", "b1_b1", "b1_b2", "b2_b1",
            "b2_b2k", "b3_b1", "b3_b2", "ln0_g", "ln0_b", "ln1_g", "ln1_b",
            "ln2_g", "ln2_b", "ln3_g", "ln3_b", "ab0_b", "ab1_b",
        ]
    }
    kv0_b_full = const.tile([P, D], F32)
    nc.sync.dma_start(kv0_b_full[:], w["kv0_b_full"][:])
    b2_b2v_full = const.tile([P, D], F32)
    nc.sync.dma_start(b2_b2v_full[:], w["b2_b2v_full"][:])

    def w_ktile(name, k, ncols, p, tag):
        t = p.tile([P, ncols], F32R, tag=tag, name=f"{name}k{k}")
        nc.sync.dma_start(t[:], w[name][k * P : (k + 1) * P, :])
        return t

    # ---- dense helpers -------------------------------------------------
    def dense_wide(x_tiles, wname, bias_vec, out_tag):
        """[D -> HID] with relu. m-outer over 16 output tiles. Weights load
        as [128, 1024] half-m k-tiles so the second half (and the next
        block's first half) can prefetch while the first computes."""
        halves = [
            [None] * ND,
            [None] * ND,
        ]
        for half in range(2):
            for k in range(ND):
                t = wbig.tile([P, HID // 2], F32R, tag="w1024", name=f"{wname}h{half}k{k}")
                nc.sync.dma_start(
                    t[:],
                    w[wname][k * P : (k + 1) * P, half * (HID // 2) : (half + 1) * (HID // 2)],
                )
                halves[half][k] = t
        outs = []
        for m in range(NH):
            half, mloc = divmod(m, NH // 2)
            pso = ps.tile([P, LQ], F32, tag="mm")
            for k in range(ND):
                nc.tensor.matmul(
                    pso[:], halves[half][k][:, mloc * P : (mloc + 1) * P], x_tiles[k][:],
                    start=(k == 0), stop=(k == ND - 1),
                )
            o = hid.tile([P, LQ], F32R, tag="hidden", name=f"hw{m}")
            nc.scalar.activation(o[:], pso[:], AF.Relu, bias=bias_vec[:, m : m + 1])
            outs.append(o)
        return outs

    def dense_narrow(x_tiles, wname, n_in, bias_vec, out_pool, out_tag,
                     relu=False, evict=None):
        """[n_in -> 512] feature-major. k-outer so weight k-tiles stream
        with bufs=4; the 4 output psums accumulate concurrently."""
        nk = n_in // P
        psos = [ps.tile([P, LQ], F32, tag="mm", name=f"dnps{m}") for m in range(ND)]
        for k in range(nk):
            wk = w_ktile(wname, k, D, wsm, "w512")
            for m in range(ND):
                nc.tensor.matmul(
                    psos[m][:], wk[:, m * P : (m + 1) * P], x_tiles[k][:],
                    start=(k == 0), stop=(k == nk - 1),
                )
        outs = []
        for m in range(ND):
            o = out_pool.tile([P, LQ], F32R, tag=out_tag, name=f"dn{m}")
            if evict is not None:
                evict(o, psos[m], m)
            elif relu:
                nc.scalar.activation(
                    o[:], psos[m][:], AF.Relu, bias=bias_vec[:, m : m + 1]
                )
            else:
                nc.vector.tensor_scalar(
                    o[:], psos[m][:], bias_vec[:, m : m + 1], None, OP.add
                )
            outs.append(o)
        return outs

    def dense_tok(x_tiles, wname, bias_full, post):
        """[HID -> 512] token-major out: for each token tile jt a [128, 512]
        psum accumulates x[k][:, jt] @ W[k]; bias (a host-replicated full
        tile) is folded in by the consumer. post(jt, psum, bias) consumes."""
        psos = [ps.tile([P, D], F32, tag="mm", name=f"dtps{j}") for j in range(NT)]
        for k in range(NH):
            wk = w_ktile(wname, k, D, wsm, "w512")
            for jt in range(NT):
                nc.tensor.matmul(
                    psos[jt][:], x_tiles[k][:, jt * P : (jt + 1) * P], wk[:],
                    start=(k == 0), stop=(k == NH - 1),
                )
        for jt in range(NT):
            post(jt, psos[jt], bias_full)

    # ---- layernorm helpers ---------------------------------------------
    def ln_tok_to_fm(src_dram, g_vec, b_vec, want_raw=False):
        fm_tiles = [fm_ln.tile([P, LQ], F32R, tag="lnstream", name=f"lnfm{c}") for c in range(ND)]
        raw_tiles = (
            [fm_raw.tile([P, LQ], F32R, tag="qraw", name=f"qraw{c}") for c in range(ND)]
            if want_raw else None
        )
        for it in range(NT):
            xt = act.tile([P, D], F32R, tag="xt_in")
            nc.sync.dma_start(xt[:], src_dram[it * P : (it + 1) * P, :])
            stats = sm.tile([P, nc.vector.BN_STATS_DIM], F32, tag="bnst", bufs=2)
            nc.vector.bn_stats(stats[:], xt[:].bitcast(F32))
            mv = sm.tile([P, nc.vector.BN_AGGR_DIM], F32, tag="bnmv", bufs=2)
            nc.vector.bn_aggr(mv[:], stats[:])
            sd = sm.tile([P, 1], F32, tag="bnsd", bufs=2)
            nc.scalar.activation(sd[:], mv[:, 1:2], AF.Sqrt, bias=eps_t[:])
            nc.vector.reciprocal(sd[:], sd[:])
            xn = act.tile([P, D], F32R, tag="xt_n")
            nc.vector.tensor_scalar(
                xn[:], xt[:], mv[:, 0:1], sd[:], OP.subtract, OP.mult
            )
            for c in range(ND):
                tp = ps_t.tile([P, P], F32R, tag="tps")
                nc.tensor.transpose(tp[:], xn[:, c * P : (c + 1) * P], identity[:])
                nc.vector.tensor_scalar(
                    fm_tiles[c][:, it * P : (it + 1) * P], tp[:],
                    g_vec[:, c : c + 1], b_vec[:, c : c + 1], OP.mult, OP.add,
                )
                if raw_tiles is not None:
                    tpr = ps_t.tile([P, P], F32R, tag="tps")
                    nc.tensor.transpose(tpr[:], xt[:, c * P : (c + 1) * P], identity[:])
                    nc.vector.tensor_copy(raw_tiles[c][:, it * P : (it + 1) * P], tpr[:])
        return fm_tiles, raw_tiles

    def ln_fm(x_tiles, g_vec, b_vec):
        """LayerNorm over the partition (feature) direction of feature-major
        tiles, via ones-matmul sums and a PE broadcast."""
        s_ps = ps_small.tile([1, LQ], F32, tag="small")
        s2_ps = ps_small.tile([1, LQ], F32, tag="small")
        for c in range(ND):
            nc.tensor.matmul(
                s_ps[:], onescol[:], x_tiles[c][:],
                start=(c == 0), stop=(c == ND - 1),
            )
        sqs = []
        for c in range(ND):
            sq = hid.tile([P, LQ], F32R, tag="hidden", name=f"sq{c}")
            nc.scalar.activation(sq[:], x_tiles[c][:], AF.Square)
            sqs.append(sq)
        for c in range(ND):
            nc.tensor.matmul(
                s2_ps[:], onescol[:], sqs[c][:],
                start=(c == 0), stop=(c == ND - 1),
            )
        mu = sm.tile([1, LQ], F32R, tag="lnmu")
        nc.vector.tensor_scalar(mu[:], s_ps[:], 1.0 / D, None, OP.mult)
        var = sm.tile([1, LQ], F32, tag="lnvar")
        nc.vector.tensor_scalar(var[:], s2_ps[:], 1.0 / D, None, OP.mult)
        m2 = sm.tile([1, LQ], F32, tag="lnm2")
        nc.vector.tensor_tensor(m2[:], mu[:].bitcast(F32), mu[:].bitcast(F32), OP.mult)
        nc.vector.tensor_tensor(var[:], var[:], m2[:], OP.subtract)
        sd = sm.tile([1, LQ], F32R, tag="lnsd")
        nc.scalar.activation(sd[:], var[:], AF.Sqrt, bias=eps_t[0:1, 0:1])
        nc.vector.reciprocal(sd[:], sd[:])
        mub = ps_small.tile([P, LQ], F32, tag="small")
        nc.tensor.matmul(mub[:], ones1[:], mu[:], start=True, stop=True)
        sdb = ps_small.tile([P, LQ], F32, tag="small")
        nc.tensor.matmul(sdb[:], ones1[:], sd[:], start=True, stop=True)
        outs = []
        for c in range(ND):
            t1 = hid.tile([P, LQ], F32, tag="hidden", name=f"lt{c}")
            nc.vector.tensor_tensor(t1[:], x_tiles[c][:].bitcast(F32), mub[:], OP.subtract)
            nc.vector.tensor_tensor(t1[:], t1[:], sdb[:], OP.mult)
            o = fm_ln.tile([P, LQ], F32R, tag="lnstream", name=f"lno{c}")
            nc.vector.tensor_scalar(
                o[:], t1[:], g_vec[:, c : c + 1], b_vec[:, c : c + 1],
                OP.mult, OP.add,
            )
            outs.append(o)
        return outs

    def head_slice(tiles, h):
        return tiles[h // 2][64 * (h % 2) : 64 * (h % 2) + 64, :]

    # ================= block0 =================
    lnq, q_fm = ln_tok_to_fm(q_in, vecs["ln0_g"], vecs["ln0_b"], want_raw=True)
    x_tiles = dense_wide(lnq, "b0_W", vecs["b0_b"], "x")

    qp = dense_narrow(x_tiles, "q0_W", HID, vecs["q0_b"], attA, "attA")
    kep = dense_narrow(x_tiles, "ke0_W", HID, vecs["ke0_b"], attA, "attA")

    kv_tok = []

    def kv_post(jt, psv, bias_full):
        t = kvA.tile([P, H * 65], F32R, tag="kvA")
        nc.vector.tensor_tensor(t[:, 0:D], psv[:], bias_full[:], OP.add)
        kv_tok.append(t)

    dense_tok(x_tiles, "kv0_W", kv0_b_full, kv_post)

    # ---- rel tables
    ep_fm = []
    krk = [w_ktile("kr0_W", k, D, wsm, "w512") for k in range(ND)]
    for m in range(ND):
        pse = ps.tile([P, LQ], F32, tag="mm")
        for k in range(ND):
            nc.tensor.matmul(
                pse[:], krk[k][:, m * P : (m + 1) * P], encR_sb[:, k, :],
                start=(k == 0), stop=(k == ND - 1),
            )
        o = resid.tile([P, LQ], F32R, tag="resid", name=f"ep{m}")
        nc.vector.tensor_scalar(o[:], pse[:], vecs["kr0_b"][:, m : m + 1], None, OP.add)
        ep_fm.append(o)

    ab1k = [w_ktile("ab1_W", k, H, wsm, "w8") for k in range(ND)]
    gam_ps = ps_small.tile([H, LQ], F32, tag="small")
    for k in range(ND):
        nc.tensor.matmul(
            gam_ps[:], ab1k[k][:], ep_fm[k][:], start=(k == 0), stop=(k == ND - 1)
        )
    gam = sm.tile([H, LQ], F32R, tag="gam")
    nc.vector.tensor_scalar(gam[:], gam_ps[:], vecs["ab1_b"][:], None, OP.add)

    ab0k = [w_ktile("ab0_W", k, H, wsm, "w8") for k in range(ND)]
    bke_ps = ps_small.tile([H, LQ], F32, tag="small")
    for k in range(ND):
        nc.tensor.matmul(
            bke_ps[:], ab0k[k][:], kep[k][:], start=(k == 0), stop=(k == ND - 1)
        )
    bke = sm.tile([H, LQ], F32R, tag="bke")
    nc.vector.tensor_scalar(bke[:], bke_ps[:], vecs["ab0_b"][:], None, OP.add)

    gam_all = sm.tile([1, H * LQ], F32R, tag="gamall")
    bkem_all = sm.tile([1, H * LQ], F32R, tag="bkemall")
    for h in range(H):
        nc.sync.dma_start(gam_all[:, h * LQ : (h + 1) * LQ], gam[h : h + 1, :])
        nc.sync.dma_start(bkem_all[:, h * LQ : (h + 1) * LQ], bke[h : h + 1, :])
        nc.vector.tensor_tensor(
            bkem_all[:, h * LQ : (h + 1) * LQ],
            bkem_all[:, h * LQ : (h + 1) * LQ], qmadd_sb[:], OP.add,
        )
    gam_rows = [gam_all[:, h * LQ : (h + 1) * LQ] for h in range(H)]
    bkem_rows = [bkem_all[:, h * LQ : (h + 1) * LQ] for h in range(H)]

    # ================= attention 0 =================
    att_q1 = [resid.tile([P, LQ], F32R, tag="resid", name=f"q1_{c}") for c in range(ND)]
    for h in range(H):
        qh = head_slice(qp, h)
        keh = head_slice(kep, h)
        eph = head_slice(ep_fm, h)
        # per-head broadcasts of gamma and (bias_ke + qmask) rows
        gb_ps = ps.tile([P, LQ], F32, tag="mm", name="gb_ps")
        nc.tensor.matmul(gb_ps[:], ones1[:], gam_rows[h], start=True, stop=True)
        gamb = soft.tile([P, LQ], F32, tag="gamb", name="gamb")
        nc.scalar.activation(gamb[:], gb_ps[:], AF.Copy)
        bk_ps = ps.tile([P, LQ], F32, tag="mm", name="bk_ps")
        nc.tensor.matmul(bk_ps[:], ones1[:], bkem_rows[h], start=True, stop=True)
        bkeb = soft.tile([P, LQ], F32, tag="bkeb", name="bkeb")
        nc.scalar.activation(bkeb[:], bk_ps[:], AF.Copy)

        pn_tiles = []
        for it in range(NT):
            psb = ps.tile([P, LQ], F32, tag="mm")
            nc.tensor.matmul(
                psb[:], qh[:, it * P : (it + 1) * P], eph[:],
                start=True, stop=True,
            )
            band_w = soft.tile([P, G_WPITCH], mybir.dt.bfloat16, tag="bandw", name="band_w")
            nc.vector.tensor_tensor(band_w[:, 0:LQ], psb[:], gamb[:], OP.add)
            nc.vector.memset(band_w[:, LQ : LQ + 1], 0.0)
            gw = bass.AP(
                tensor=g_scr[h],
                offset=it * P * G_WPITCH,
                ap=[[G_WPITCH, P], [1, G_WPITCH]],
            )
            nc.sync.dma_start(gw, band_w[:])
            band = soft.tile([P, LQ], mybir.dt.bfloat16, tag="band")
            gr_ap = bass.AP(
                tensor=g_scr[h],
                offset=(LQ - 1) + it * P * LQ,
                ap=[[LQ, P], [1, LQ]],
            )
            nc.sync.dma_start(band[:], gr_ap)

            pss = ps.tile([P, LQ], F32, tag="mm")
            nc.tensor.matmul(
                pss[:], qh[:, it * P : (it + 1) * P], keh[:], start=True, stop=True
            )
            s2 = soft.tile([P, LQ], F32, tag="s2", name="s2")
            nc.vector.tensor_tensor(s2[:], pss[:], band[:], OP.add)
            nc.vector.tensor_tensor(s2[:], s2[:], bkeb[:], OP.add)
            nc.gpsimd.affine_select(
                out=s2[:], in_=s2[:], compare_op=OP.is_ge, fill=-1e30,
                base=it * P, pattern=[[-1, LQ]], channel_multiplier=1,
            )
            z = sm.tile([P, 1], F32, tag="z0", bufs=2)
            p_t = soft.tile([P, LQ], F32R, tag=f"p{it}", name="p_t")
            nc.scalar.activation(p_t[:], s2[:], AF.Exp, scale=SCALE, accum_out=z[:])
            nc.vector.reciprocal(z[:], z[:])
            nc.vector.tensor_scalar(p_t[:], p_t[:], z[:], None, OP.mult)
            pn_tiles.append(p_t)

        pt_tiles = [pt_pool.tile([P, LQ], F32R, tag=f"pt{jt}", name=f"pt{jt}") for jt in range(NT)]
        for it in range(NT):
            for jt in range(it + 1):
                tp = ps_t.tile([P, P], F32R, tag="tps")
                nc.tensor.transpose(
                    tp[:], pn_tiles[it][:, jt * P : (jt + 1) * P], identity[:]
                )
                nc.vector.tensor_copy(pt_tiles[jt][:, it * P : (it + 1) * P], tp[:])
        for it in range(NT):
            pso = ps_small.tile([64, P], F32, tag="small")
            for jt in range(it + 1):
                nc.tensor.matmul(
                    pso[:], kv_tok[jt][:, 64 * h : 64 * h + 64],
                    pt_tiles[jt][:, it * P : (it + 1) * P],
                    start=(jt == 0), stop=(jt == it),
                )
            dst = att_q1[h // 2][
                64 * (h % 2) : 64 * (h % 2) + 64, it * P : (it + 1) * P
            ]
            src_q = q_fm[h // 2][
                64 * (h % 2) : 64 * (h % 2) + 64, it * P : (it + 1) * P
            ]
            nc.vector.tensor_tensor(dst, pso[:], src_q, OP.add)

    # ================= block1 -> y =================
    lnq1 = ln_fm(att_q1, vecs["ln1_g"], vecs["ln1_b"])
    h1 = dense_wide(lnq1, "b1_W1", vecs["b1_b1"], "h1")
    y_fm = dense_narrow(h1, "b1_W2", HID, vecs["b1_b2"], attA, "attA")

    # ================= block2 -> k1 (fm), v1 (tok + ones col) ============
    lnv, _ = ln_tok_to_fm(v_in, vecs["ln2_g"], vecs["ln2_b"])
    h2 = dense_wide(lnv, "b2_W1", vecs["b2_b1"], "h2")
    k1_fm = dense_narrow(h2, "b2_W2k", HID, vecs["b2_b2k"], attA, "attA")

    v1i = []

    def v1_post(jt, psv, bias_full):
        t = kvA.tile([P, H * 65], F32R, tag="kvA")
        tv = t[:].rearrange("p (h x) -> p h x", h=H)
        nc.vector.tensor_tensor(
            tv[:, :, 0:64], psv[:].rearrange("p (h d) -> p h d", h=H),
            bias_full[:].rearrange("p (h d) -> p h d", h=H), OP.add,
        )
        nc.vector.tensor_copy(
            tv[:, :, 64:65], ones_col8[:].rearrange("p (h x) -> p h x", x=1)
        )
        v1i.append(t)

    dense_tok(h2, "b2_W2v", b2_b2v_full, v1_post)

    # ================= attention 1 (transposed) =================
    att_q2 = [resid.tile([P, LQ], F32R, tag="resid", name=f"q2_{c}") for c in range(ND)]
    for h in range(H):
        yh = head_slice(y_fm, h)
        k1h = head_slice(k1_fm, h)
        p1t_tiles = []
        for jt in range(NT):
            pss = ps.tile([P, LQ], F32, tag="mm")
            nc.tensor.matmul(
                pss[:], k1h[:, jt * P : (jt + 1) * P], yh[:], start=True, stop=True
            )
            p1 = soft.tile([P, LQ], F32R, tag=f"p{jt}", name="p1")
            nc.scalar.activation(
                p1[:], pss[:], AF.Exp, scale=SCALE, bias=vmadd_sb[:, jt : jt + 1]
            )
            p1t_tiles.append(p1)
        pso = ps.tile([65, LQ], F32, tag="mm")
        for jt in range(NT):
            nc.tensor.matmul(
                pso[:], v1i[jt][:, 65 * h : 65 * h + 65], p1t_tiles[jt][:],
                start=(jt == 0), stop=(jt == NT - 1),
            )
        rz = sm.tile([1, LQ], F32R, tag="rz1", bufs=2)
        nc.vector.reciprocal(rz[:], pso[64:65, :])
        psb = ps_small.tile([64, LQ], F32, tag="small")
        nc.tensor.matmul(psb[:], ones1[:, 0:64], rz[:], start=True, stop=True)
        o1 = soft.tile([P, LQ], F32, tag="o1")
        o1s = o1[64 * (h % 2) : 64 * (h % 2) + 64, :]
        nc.scalar.activation(o1s, pso[0:64, :], AF.Copy)
        nc.vector.tensor_tensor(o1s, o1s, psb[:], OP.mult)
        dst = att_q2[h // 2][64 * (h % 2) : 64 * (h % 2) + 64, :]
        src_q = att_q1[h // 2][64 * (h % 2) : 64 * (h % 2) + 64, :]
        nc.vector.tensor_tensor(dst, o1s, src_q, OP.add)

    # ================= block3 residual FFN =================
    lnq3 = ln_fm(att_q2, vecs["ln3_g"], vecs["ln3_b"])
    h3 = dense_wide(lnq3, "b3_W1", vecs["b3_b1"], "h3")

    def b3_evict(o, pso, m):
        nc.vector.scalar_tensor_tensor(
            o[:], pso[:], vecs["b3_b2"][:, m : m + 1], att_q2[m][:],
            OP.add, OP.add,
        )

    q3 = dense_narrow(h3, "b3_W2", HID, None, resid, "resid", evict=b3_evict)

    # ---- back to token-major, store
    for it in range(NT):
        ot = act.tile([P, D], F32, tag="out_tok")
        for c in range(ND):
            tp = ps_t.tile([P, P], F32R, tag="tps")
            nc.tensor.transpose(tp[:], q3[c][:, it * P : (it + 1) * P], identity[:])
            nc.vector.tensor_copy(ot[:, c * P : (c + 1) * P], tp[:].bitcast(F32))
        nc.sync.dma_start(out[it * P : (it + 1) * P, :], ot[:])

    ctx.close()


_NC = None


def _get_nc():
    global _NC
    if _NC is None:
        _NC = build_nc()
    return _NC


def _build_in_maps(queries, values, queries_mask, values_mask, params):
    p = params
    f32 = np.float32

    def fmvec(v, n_tiles):
        return np.ascontiguousarray(np.asarray(v, f32).reshape(n_tiles, P).T)

    w2 = np.asarray(p["b2_W2"], f32).reshape(HID, H, 2, DIM)
    b2b = np.asarray(p["b2_b2"], f32).reshape(H, 2, DIM)

    shared = {
        "b0_W": np.asarray(p["b0_W"], f32), "q0_W": np.asarray(p["q0_W"], f32),
        "ke0_W": np.asarray(p["ke0_W"], f32), "kv0_W": np.asarray(p["kv0_W"], f32),
        "kr0_W": np.asarray(p["kr0_W"], f32), "ab0_W": np.asarray(p["ab0_W"], f32),
        "ab1_W": np.asarray(p["ab1_W"], f32),
        "b1_W1": np.asarray(p["b1_W1"], f32), "b1_W2": np.asarray(p["b1_W2"], f32),
        "b2_W1": np.asarray(p["b2_W1"], f32),
        "b2_W2k": np.ascontiguousarray(w2[:, :, 0, :].reshape(HID, D)),
        "b2_W2v": np.ascontiguousarray(w2[:, :, 1, :].reshape(HID, D)),
        "b3_W1": np.asarray(p["b3_W1"], f32), "b3_W2": np.asarray(p["b3_W2"], f32),
        "b0_b": fmvec(p["b0_b"], NH), "q0_b": fmvec(p["q0_b"], ND),
        "ke0_b": fmvec(p["ke0_b"], ND), "kr0_b": fmvec(p["kr0_b"], ND),
        "b1_b1": fmvec(p["b1_b1"], NH), "b1_b2": fmvec(p["b1_b2"], ND),
        "b2_b1": fmvec(p["b2_b1"], NH),
        "b2_b2k": fmvec(b2b[:, 0, :].reshape(D), ND),
        "b3_b1": fmvec(p["b3_b1"], NH), "b3_b2": fmvec(p["b3_b2"], ND),
        "ln0_g": fmvec(p["ln0_g"], ND), "ln0_b": fmvec(p["ln0_b"], ND),
        "ln1_g": fmvec(p["ln1_g"], ND), "ln1_b": fmvec(p["ln1_b"], ND),
        "ln2_g": fmvec(p["ln2_g"], ND), "ln2_b": fmvec(p["ln2_b"], ND),
        "ln3_g": fmvec(p["ln3_g"], ND), "ln3_b": fmvec(p["ln3_b"], ND),
        "ab0_b": np.asarray(p["ab0_b"], f32).reshape(H, 1),
        "ab1_b": np.asarray(p["ab1_b"], f32).reshape(H, 1),
        "kv0_b_full": np.tile(np.asarray(p["kv0_b"], f32).reshape(1, D), (P, 1)),
        "b2_b2v_full": np.tile(b2b[:, 1, :].reshape(1, D), (P, 1)),
        "encR": np.ascontiguousarray(_sinusoid_ext_rev().T),
        "idn": np.eye(P, dtype=f32),
        "ones_r": np.ones((1, P), f32),
        "ones_c": np.ones((P, 1), f32),
        "ones8": np.ones((P, H), f32),
        "epsc": np.full((P, 1), 1e-3, f32),
    }

    qm = np.asarray(queries_mask, f32)
    vm = np.asarray(values_mask, f32)
    in_maps = []
    for c in range(B):
        m = dict(shared)
        m["q_in"] = np.ascontiguousarray(np.asarray(queries[c], f32))
        m["v_in"] = np.ascontiguousarray(np.asarray(values[c], f32))
        m["qmadd"] = ((qm[c] - 1.0) * 8e9).reshape(1, LQ).astype(f32)
        m["vmadd"] = np.ascontiguousarray(
            ((vm[c] - 1.0) * 1e9).reshape(NT, P).T
        ).astype(f32)
        in_maps.append(m)
    return in_maps


def kernel(queries, values, queries_mask, values_mask, params):
    in_maps = _build_in_maps(queries, values, queries_mask, values_mask, params)
    nc = _get_nc()
    res = run_bass_kernel_spmd(nc, in_maps, core_ids=list(range(B)))
    return np.stack([res.results[c]["out"] for c in range(B)], axis=0).astype(
        np.float32
    )


def kernel_profiled(queries, values, queries_mask, values_mask, params,
                    tmpdir=None):
    """Same as kernel() but with NTFF tracing; returns (output, results)."""
    import kernel as _self  # works both as module and as __main__ helper

    in_maps = _build_in_maps(queries, values, queries_mask, values_mask, params)
    nc = _get_nc()
    res = run_bass_kernel_spmd(
        nc, in_maps, core_ids=list(range(B)), trace=True, tmpdir=tmpdir
    )
    out = np.stack([res.results[c]["out"] for c in range(B)], axis=0).astype(
        np.float32
    )
    return out, res


# revision 15
# speedup vs baseline: 1.1617x; 1.0439x over previous
"""Trainium2 Bass kernel for nn_DecoderWithRelativePositionalAttentionLayer.

Sharding: pure data-parallel over batch (B=8 -> 8 NeuronCores, one batch
element per core, identical SPMD program, no collectives).

Key algorithmic move: the reference materializes rel = enc[ridx] as a
[Lq, Lq, D] tensor and pushes it through a [D, D] dense (38.7 GMAC); since
ridx = clip(i-j, -R, R) takes only 201 values and the causal mask kills
j > i, we project a [384, D] extended/reversed table once and realize the
per-(i, j) band with a DRAM "skew" bounce: rows written at pitch 385 and
read back at pitch 384 / offset 383 land row i's entry t at column
j = i - 383 + t.

Layouts: activations are feature-major [F, T] so weight matmuls need no
transposes (lhsT = native [in, out] weight k-tiles, rhs = activations).
Attention-0 scores run in [i, j] (softmax along the free dim, causal via
gpsimd affine_select, band added pre-exp); the probability tiles are
PE-transposed for the PV matmul. Attention-1 has no positional term and is
computed directly transposed [j, i], with the softmax normalizer taken from
an extra ones-column interleaved into the value matrix.

All matmul operands are float32r (TF32-like, full-rate on the PE array;
measured ~2.5e-4 per-matmul relative error on HW).
"""

import sys

sys.path.insert(0, "/opt/trn_rl_repo")

import contextlib
import numpy as np
import concourse.bass as bass
import concourse.mybir as mybir
import concourse.tile as tile
from concourse.bass_utils import run_bass_kernel_spmd
import bass_rust

F32 = mybir.dt.float32
F32R = mybir.dt.float32r
AF = mybir.ActivationFunctionType
OP = mybir.AluOpType
AX = mybir.AxisListType

B, LQ, LK, D, H, HID = 8, 384, 384, 512, 8, 2048
DIM = D // H
REL = 100
P = 128
NT = LQ // P   # 3 token tiles
ND = D // P    # 4 feature tiles
NH = HID // P  # 16 hidden tiles
SCALE = float(1.0 / np.sqrt(np.float32(DIM)))
G_WPITCH = LQ + 1           # 385: write pitch of the skew scratch
G_STRIDE = G_WPITCH * LQ    # per-head segment


def _split_multiwait_instructions(nc):
    """This toolchain's walrus accepts at most ONE sync wait per
    instruction, but Tile's tail drain aggregates several. Move extras onto
    same-engine nops placed immediately before the offending instruction."""
    counter = [0]

    def fresh_nop(engine, wait):
        counter[0] += 1
        nop = bass_rust.InstNoOp(name=f"WSPLIT-{counter[0]}", ins=[], outs=[])
        nop.engine = engine
        nop.sync_info = bass_rust.SyncInfo(on_wait=[wait], on_update=[])
        return nop

    for fn in nc.m.functions:
        for bb in fn.blocks:
            out = []
            changed = False
            for inst in bb.instructions:
                si = inst.sync_info
                if si is not None and len(si.on_wait) > 1:
                    waits = list(si.on_wait)
                    for w in waits[:-1]:
                        out.append(fresh_nop(inst.engine, w))
                    inst.sync_info = bass_rust.SyncInfo(
                        on_wait=[waits[-1]], on_update=list(si.on_update)
                    )
                    changed = True
                out.append(inst)
            if changed:
                bb.instructions = out


def _sinusoid_ext_rev():
    """encER[t] = enc[min(383 - t, REL) + REL]  -> [384, 512]."""
    pos = np.arange(-REL, REL + 1, dtype=np.float32)[:, None]
    i = np.arange(D // 2, dtype=np.float32)[None, :]
    ang = pos / np.power(np.float32(10000.0), 2.0 * i / np.float32(D))
    enc = np.concatenate([np.sin(ang), np.cos(ang)], axis=-1).astype(np.float32)
    o = (LQ - 1) - np.arange(LQ)
    return enc[np.minimum(o, REL) + REL]


def build_nc():
    nc = bass.Bass()

    def pin(name, shape, dt=F32R):
        return nc.declare_dram_parameter(name, list(shape), dt, isOutput=False)

    q_in = pin("q_in", [LQ, D])
    v_in = pin("v_in", [LK, D])
    qmadd = pin("qmadd", [1, LQ], F32)   # (qm-1)*8e9, pre-scale additive
    vmadd = pin("vmadd", [P, NT], F32)   # (vm-1)*1e9, post-scale additive

    w = {}
    for nm, shp in [
        ("b0_W", [D, HID]), ("q0_W", [HID, D]), ("ke0_W", [HID, D]),
        ("kv0_W", [HID, D]), ("kr0_W", [D, D]), ("ab0_W", [D, H]),
        ("ab1_W", [D, H]), ("b1_W1", [D, HID]), ("b1_W2", [HID, D]),
        ("b2_W1", [D, HID]), ("b2_W2k", [HID, D]), ("b2_W2v", [HID, D]),
        ("b3_W1", [D, HID]), ("b3_W2", [HID, D]),
    ]:
        w[nm] = pin(nm, shp)
    for nm, nt in [
        ("b0_b", NH), ("q0_b", ND), ("ke0_b", ND), ("kr0_b", ND),
        ("b1_b1", NH), ("b1_b2", ND), ("b2_b1", NH), ("b2_b2k", ND),
        ("b3_b1", NH), ("b3_b2", ND),
        ("ln0_g", ND), ("ln0_b", ND), ("ln1_g", ND), ("ln1_b", ND),
        ("ln2_g", ND), ("ln2_b", ND), ("ln3_g", ND), ("ln3_b", ND),
    ]:
        w[nm] = pin(nm, [P, nt], F32)
    w["ab0_b"] = pin("ab0_b", [H, 1], F32)
    w["ab1_b"] = pin("ab1_b", [H, 1], F32)
    w["kv0_b_full"] = pin("kv0_b_full", [P, D], F32)
    w["b2_b2v_full"] = pin("b2_b2v_full", [P, D], F32)

    encR = pin("encR", [D, LQ])
    idn = pin("idn", [P, P])
    ones_r = pin("ones_r", [1, P])
    ones_c = pin("ones_c", [P, 1])
    ones8 = pin("ones8", [P, H])
    epsc = pin("epsc", [P, 1], F32)

    out = nc.declare_dram_parameter("out", [LQ, D], F32, isOutput=True)
    g_scr = [
        [
            nc.dram_tensor(f"g_scratch{h}_{it}", [50048], mybir.dt.bfloat16)
            for it in range(NT)
        ]
        for h in range(H)
    ]

    with tile.TileContext(nc) as tc, nc.allow_low_precision(
        reason="float32r dataflow is intentional (TF32-like matmul operands)"
    ):
        _emit(
            nc, tc, q_in, v_in, qmadd, vmadd, w, encR, idn, ones_r, ones_c,
            ones8, epsc, out, g_scr,
        )
    _split_multiwait_instructions(nc)
    return nc


def _emit(nc, tc, q_in, v_in, qmadd, vmadd, w, encR, idn, ones_r, ones_c,
          ones8, epsc, out, g_scr):
    ctx = contextlib.ExitStack()

    def pool(name, bufs, **kw):
        return ctx.enter_context(tc.tile_pool(name=name, bufs=bufs, **kw))

    const = pool("const", 1)
    wbig = pool("wbig", 8)       # [128, 1024] half k-tiles
    wsm = pool("wsm", 6)         # [128, <=512] k-tiles, streaming (k-outer)
    act = pool("act", 2)         # token-major staging
    fm_ln = pool("fm_ln", 4)     # LN output streams (ln0->ln1->ln2->ln3)
    fm_raw = pool("fm_raw", 4)   # raw queries, feature-major
    attA = pool("attA", 8)       # qp, kep -> y, k1
    kvA = pool("kvA", 3)         # kv_tok -> v1i
    resid = pool("resid", 8)     # ep -> q1 -> q2 -> q3
    hid = pool("hid", 16)        # hidden tiles + LN scratch
    soft = pool("soft", 2)
    pt_pool = pool("pt", 1)
    sm = pool("sm", 1)
    ps = pool("ps", 4, space="PSUM")
    ps_t = pool("ps_t", 2, space="PSUM")
    ps_small = pool("ps_small", 2, space="PSUM")

    # ---- constants
    identity = const.tile([P, P], F32R)
    nc.sync.dma_start(identity[:], idn[:])
    ones1 = const.tile([1, P], F32R)
    nc.sync.dma_start(ones1[:], ones_r[:])
    onescol = const.tile([P, 1], F32R)
    nc.sync.dma_start(onescol[:], ones_c[:])
    ones_col8 = const.tile([P, H], F32R)
    nc.sync.dma_start(ones_col8[:], ones8[:])
    eps_t = const.tile([P, 1], F32)
    nc.sync.dma_start(eps_t[:], epsc[:])
    encR_sb = const.tile([P, ND, LQ], F32R)
    nc.sync.dma_start(encR_sb[:], encR[:].rearrange("(k p) t -> p k t", p=P))
    qmadd_sb = const.tile([1, LQ], F32)
    nc.sync.dma_start(qmadd_sb[:], qmadd[:])
    vmadd_sb = const.tile([P, NT], F32)
    nc.sync.dma_start(vmadd_sb[:], vmadd[:])

    def load_vec(name):
        t = const.tile(list(w[name].shape), F32, name=f"v_{name}")
        nc.sync.dma_start(t[:], w[name][:])
        return t

    vecs = {
        nm: load_vec(nm)
        for nm in [
            "b0_b", "q0_b", "ke0_b", "kr0_b", "b1_b1", "b1_b2", "b2_b1",
            "b2_b2k", "b3_b1", "b3_b2", "ln0_g", "ln0_b", "ln1_g", "ln1_b",
            "ln2_g", "ln2_b", "ln3_g", "ln3_b", "ab0_b", "ab1_b",
        ]
    }
    kv0_b_full = const.tile([P, D], F32)
    nc.sync.dma_start(kv0_b_full[:], w["kv0_b_full"][:])
    b2_b2v_full = const.tile([P, D], F32)
    nc.sync.dma_start(b2_b2v_full[:], w["b2_b2v_full"][:])

    def w_ktile(name, k, ncols, p, tag):
        t = p.tile([P, ncols], F32R, tag=tag, name=f"{name}k{k}")
        nc.sync.dma_start(t[:], w[name][k * P : (k + 1) * P, :])
        return t

    # ---- dense helpers -------------------------------------------------
    def dense_wide(x_tiles, wname, bias_vec, out_tag):
        """[D -> HID] with relu. m-outer over 16 output tiles. Weights load
        as [128, 1024] half-m k-tiles so the second half (and the next
        block's first half) can prefetch while the first computes."""
        halves = [
            [None] * ND,
            [None] * ND,
        ]
        for half in range(2):
            for k in range(ND):
                t = wbig.tile([P, HID // 2], F32R, tag="w1024", name=f"{wname}h{half}k{k}")
                nc.sync.dma_start(
                    t[:],
                    w[wname][k * P : (k + 1) * P, half * (HID // 2) : (half + 1) * (HID // 2)],
                )
                halves[half][k] = t
        outs = []
        for m in range(NH):
            half, mloc = divmod(m, NH // 2)
            pso = ps.tile([P, LQ], F32, tag="mm")
            for k in range(ND):
                nc.tensor.matmul(
                    pso[:], halves[half][k][:, mloc * P : (mloc + 1) * P], x_tiles[k][:],
                    start=(k == 0), stop=(k == ND - 1),
                )
            o = hid.tile([P, LQ], F32R, tag="hidden", name=f"hw{m}")
            nc.scalar.activation(o[:], pso[:], AF.Relu, bias=bias_vec[:, m : m + 1])
            outs.append(o)
        return outs

    def dense_narrow(x_tiles, wname, n_in, bias_vec, out_pool, out_tag,
                     relu=False, evict=None):
        """[n_in -> 512] feature-major. k-outer so weight k-tiles stream
        with bufs=4; the 4 output psums accumulate concurrently."""
        nk = n_in // P
        psos = [ps.tile([P, LQ], F32, tag="mm", name=f"dnps{m}") for m in range(ND)]
        for k in range(nk):
            wk = w_ktile(wname, k, D, wsm, "w512")
            for m in range(ND):
                nc.tensor.matmul(
                    psos[m][:], wk[:, m * P : (m + 1) * P], x_tiles[k][:],
                    start=(k == 0), stop=(k == nk - 1),
                )
        outs = []
        for m in range(ND):
            o = out_pool.tile([P, LQ], F32R, tag=out_tag, name=f"dn{m}")
            if evict is not None:
                evict(o, psos[m], m)
            elif relu:
                nc.scalar.activation(
                    o[:], psos[m][:], AF.Relu, bias=bias_vec[:, m : m + 1]
                )
            else:
                nc.vector.tensor_scalar(
                    o[:], psos[m][:], bias_vec[:, m : m + 1], None, OP.add
                )
            outs.append(o)
        return outs

    def dense_tok(x_tiles, wname, bias_full, post):
        """[HID -> 512] token-major out: for each token tile jt a [128, 512]
        psum accumulates x[k][:, jt] @ W[k]; bias (a host-replicated full
        tile) is folded in by the consumer. post(jt, psum, bias) consumes."""
        psos = [ps.tile([P, D], F32, tag="mm", name=f"dtps{j}") for j in range(NT)]
        for k in range(NH):
            wk = w_ktile(wname, k, D, wsm, "w512")
            for jt in range(NT):
                nc.tensor.matmul(
                    psos[jt][:], x_tiles[k][:, jt * P : (jt + 1) * P], wk[:],
                    start=(k == 0), stop=(k == NH - 1),
                )
        for jt in range(NT):
            post(jt, psos[jt], bias_full)

    # ---- layernorm helpers ---------------------------------------------
    def ln_tok_to_fm(src_dram, g_vec, b_vec, want_raw=False):
        fm_tiles = [fm_ln.tile([P, LQ], F32R, tag="lnstream", name=f"lnfm{c}") for c in range(ND)]
        raw_tiles = (
            [fm_raw.tile([P, LQ], F32R, tag="qraw", name=f"qraw{c}") for c in range(ND)]
            if want_raw else None
        )
        for it in range(NT):
            xt = act.tile([P, D], F32R, tag="xt_in")
            nc.sync.dma_start(xt[:], src_dram[it * P : (it + 1) * P, :])
            stats = sm.tile([P, nc.vector.BN_STATS_DIM], F32, tag="bnst", bufs=2)
            nc.vector.bn_stats(stats[:], xt[:].bitcast(F32))
            mv = sm.tile([P, nc.vector.BN_AGGR_DIM], F32, tag="bnmv", bufs=2)
            nc.vector.bn_aggr(mv[:], stats[:])
            sd = sm.tile([P, 1], F32, tag="bnsd", bufs=2)
            nc.scalar.activation(sd[:], mv[:, 1:2], AF.Sqrt, bias=eps_t[:])
            nc.vector.reciprocal(sd[:], sd[:])
            xn = act.tile([P, D], F32R, tag="xt_n")
            nc.vector.tensor_scalar(
                xn[:], xt[:], mv[:, 0:1], sd[:], OP.subtract, OP.mult
            )
            for c in range(ND):
                tp = ps_t.tile([P, P], F32R, tag="tps")
                nc.tensor.transpose(tp[:], xn[:, c * P : (c + 1) * P], identity[:])
                nc.vector.tensor_scalar(
                    fm_tiles[c][:, it * P : (it + 1) * P], tp[:],
                    g_vec[:, c : c + 1], b_vec[:, c : c + 1], OP.mult, OP.add,
                )
                if raw_tiles is not None:
                    tpr = ps_t.tile([P, P], F32R, tag="tps")
                    nc.tensor.transpose(tpr[:], xt[:, c * P : (c + 1) * P], identity[:])
                    nc.vector.tensor_copy(raw_tiles[c][:, it * P : (it + 1) * P], tpr[:])
        return fm_tiles, raw_tiles

    def ln_fm(x_tiles, g_vec, b_vec):
        """LayerNorm over the partition (feature) direction of feature-major
        tiles, via ones-matmul sums and a PE broadcast."""
        s_ps = ps_small.tile([1, LQ], F32, tag="small")
        s2_ps = ps_small.tile([1, LQ], F32, tag="small")
        for c in range(ND):
            nc.tensor.matmul(
                s_ps[:], onescol[:], x_tiles[c][:],
                start=(c == 0), stop=(c == ND - 1),
            )
        sqs = []
        for c in range(ND):
            sq = hid.tile([P, LQ], F32R, tag="hidden", name=f"sq{c}")
            nc.scalar.activation(sq[:], x_tiles[c][:], AF.Square)
            sqs.append(sq)
        for c in range(ND):
            nc.tensor.matmul(
                s2_ps[:], onescol[:], sqs[c][:],
                start=(c == 0), stop=(c == ND - 1),
            )
        mu = sm.tile([1, LQ], F32R, tag="lnmu")
        nc.vector.tensor_scalar(mu[:], s_ps[:], 1.0 / D, None, OP.mult)
        var = sm.tile([1, LQ], F32, tag="lnvar")
        nc.vector.tensor_scalar(var[:], s2_ps[:], 1.0 / D, None, OP.mult)
        m2 = sm.tile([1, LQ], F32, tag="lnm2")
        nc.vector.tensor_tensor(m2[:], mu[:].bitcast(F32), mu[:].bitcast(F32), OP.mult)
        nc.vector.tensor_tensor(var[:], var[:], m2[:], OP.subtract)
        sd = sm.tile([1, LQ], F32R, tag="lnsd")
        nc.scalar.activation(sd[:], var[:], AF.Sqrt, bias=eps_t[0:1, 0:1])
        nc.vector.reciprocal(sd[:], sd[:])
        mub = ps_small.tile([P, LQ], F32, tag="small")
        nc.tensor.matmul(mub[:], ones1[:], mu[:], start=True, stop=True)
        sdb = ps_small.tile([P, LQ], F32, tag="small")
        nc.tensor.matmul(sdb[:], ones1[:], sd[:], start=True, stop=True)
        outs = []
        for c in range(ND):
            t1 = hid.tile([P, LQ], F32, tag="hidden", name=f"lt{c}")
            nc.vector.tensor_tensor(t1[:], x_tiles[c][:].bitcast(F32), mub[:], OP.subtract)
            nc.vector.tensor_tensor(t1[:], t1[:], sdb[:], OP.mult)
            o = fm_ln.tile([P, LQ], F32R, tag="lnstream", name=f"lno{c}")
            nc.vector.tensor_scalar(
                o[:], t1[:], g_vec[:, c : c + 1], b_vec[:, c : c + 1],
                OP.mult, OP.add,
            )
            outs.append(o)
        return outs

    def head_slice(tiles, h):
        return tiles[h // 2][64 * (h % 2) : 64 * (h % 2) + 64, :]

    # ================= block0 =================
    lnq, q_fm = ln_tok_to_fm(q_in, vecs["ln0_g"], vecs["ln0_b"], want_raw=True)
    x_tiles = dense_wide(lnq, "b0_W", vecs["b0_b"], "x")

    qp = dense_narrow(x_tiles, "q0_W", HID, vecs["q0_b"], attA, "attA")
    kep = dense_narrow(x_tiles, "ke0_W", HID, vecs["ke0_b"], attA, "attA")

    kv_tok = []

    def kv_post(jt, psv, bias_full):
        t = kvA.tile([P, H * 65], F32R, tag="kvA")
        nc.vector.tensor_tensor(t[:, 0:D], psv[:], bias_full[:], OP.add)
        kv_tok.append(t)

    dense_tok(x_tiles, "kv0_W", kv0_b_full, kv_post)

    # ---- rel tables
    ep_fm = []
    krk = [w_ktile("kr0_W", k, D, wsm, "w512") for k in range(ND)]
    for m in range(ND):
        pse = ps.tile([P, LQ], F32, tag="mm")
        for k in range(ND):
            nc.tensor.matmul(
                pse[:], krk[k][:, m * P : (m + 1) * P], encR_sb[:, k, :],
                start=(k == 0), stop=(k == ND - 1),
            )
        o = resid.tile([P, LQ], F32R, tag="resid", name=f"ep{m}")
        nc.vector.tensor_scalar(o[:], pse[:], vecs["kr0_b"][:, m : m + 1], None, OP.add)
        ep_fm.append(o)

    ab1k = [w_ktile("ab1_W", k, H, wsm, "w8") for k in range(ND)]
    gam_ps = ps_small.tile([H, LQ], F32, tag="small")
    for k in range(ND):
        nc.tensor.matmul(
            gam_ps[:], ab1k[k][:], ep_fm[k][:], start=(k == 0), stop=(k == ND - 1)
        )
    gam = sm.tile([H, LQ], F32R, tag="gam")
    nc.vector.tensor_scalar(gam[:], gam_ps[:], vecs["ab1_b"][:], None, OP.add)

    ab0k = [w_ktile("ab0_W", k, H, wsm, "w8") for k in range(ND)]
    bke_ps = ps_small.tile([H, LQ], F32, tag="small")
    for k in range(ND):
        nc.tensor.matmul(
            bke_ps[:], ab0k[k][:], kep[k][:], start=(k == 0), stop=(k == ND - 1)
        )
    bke = sm.tile([H, LQ], F32R, tag="bke")
    nc.vector.tensor_scalar(bke[:], bke_ps[:], vecs["ab0_b"][:], None, OP.add)

    gam_all = sm.tile([1, H * LQ], F32R, tag="gamall")
    bkem_all = sm.tile([1, H * LQ], F32R, tag="bkemall")
    for h in range(H):
        nc.sync.dma_start(gam_all[:, h * LQ : (h + 1) * LQ], gam[h : h + 1, :])
        nc.sync.dma_start(bkem_all[:, h * LQ : (h + 1) * LQ], bke[h : h + 1, :])
        nc.vector.tensor_tensor(
            bkem_all[:, h * LQ : (h + 1) * LQ],
            bkem_all[:, h * LQ : (h + 1) * LQ], qmadd_sb[:], OP.add,
        )
    gam_rows = [gam_all[:, h * LQ : (h + 1) * LQ] for h in range(H)]
    bkem_rows = [bkem_all[:, h * LQ : (h + 1) * LQ] for h in range(H)]

    # ================= attention 0 =================
    # Phase A: all rel-position band tiles -> DRAM (skew write). Reads in
    # phase B then never stall on the HBM round trip.
    for h in range(H):
        qh = head_slice(qp, h)
        eph = head_slice(ep_fm, h)
        gb_ps = ps.tile([P, LQ], F32, tag="mm", name="gb_ps")
        nc.tensor.matmul(gb_ps[:], ones1[:], gam_rows[h], start=True, stop=True)
        gamb = soft.tile([P, LQ], F32, tag="gamb", name="gamb")
        nc.scalar.activation(gamb[:], gb_ps[:], AF.Copy)
        for it in range(NT):
            psb = ps.tile([P, LQ], F32, tag="mm")
            nc.tensor.matmul(
                psb[:], qh[:, it * P : (it + 1) * P], eph[:],
                start=True, stop=True,
            )
            band_w = soft.tile(
                [P, G_WPITCH], mybir.dt.bfloat16, tag="bandw", name="band_w"
            )
            nc.vector.tensor_tensor(band_w[:, 0:LQ], psb[:], gamb[:], OP.add)
            nc.vector.memset(band_w[:, LQ : LQ + 1], 0.0)
            gw = bass.AP(
                tensor=g_scr[h][it], offset=0, ap=[[G_WPITCH, P], [1, G_WPITCH]]
            )
            nc.sync.dma_start(gw, band_w[:])

    # Phase B: scores + softmax + PV, head pairs interleaved so the K=64
    # matmuls land on disjoint PE row groups and run concurrently.
    att_q1 = [resid.tile([P, LQ], F32R, tag="resid", name=f"q1_{c}") for c in range(ND)]
    for hp in range(H // 2):
        pair = (2 * hp, 2 * hp + 1)
        bkebs = {}
        for h in pair:
            bk_ps = ps.tile([P, LQ], F32, tag="mm", name="bk_ps")
            nc.tensor.matmul(bk_ps[:], ones1[:], bkem_rows[h], start=True, stop=True)
            bkeb = soft.tile([P, LQ], F32, tag=f"bkeb{h % 2}", name="bkeb")
            nc.scalar.activation(bkeb[:], bk_ps[:], AF.Copy)
            bkebs[h] = bkeb
        pn = {h: [None] * NT for h in pair}
        for it in range(NT):
            for h in pair:
                qh = head_slice(qp, h)
                keh = head_slice(kep, h)
                band = soft.tile(
                    [P, LQ], mybir.dt.bfloat16, tag=f"band{h % 2}", name="band",
                    bufs=2,
                )
                gr_ap = bass.AP(
                    tensor=g_scr[h][it], offset=LQ - 1, ap=[[LQ, P], [1, LQ]]
                )
                nc.sync.dma_start(band[:], gr_ap)
                pss = ps.tile([P, LQ], F32, tag="mm")
                nc.tensor.matmul(
                    pss[:], qh[:, it * P : (it + 1) * P], keh[:],
                    start=True, stop=True,
                )
                s2 = soft.tile([P, LQ], F32, tag="s2", name="s2", bufs=3)
                nc.vector.tensor_tensor(s2[:], pss[:], band[:], OP.add)
                nc.vector.tensor_tensor(s2[:], s2[:], bkebs[h][:], OP.add)
                nc.gpsimd.affine_select(
                    out=s2[:], in_=s2[:], compare_op=OP.is_ge, fill=-1e30,
                    base=it * P, pattern=[[-1, LQ]], channel_multiplier=1,
                )
                z = sm.tile([P, 1], F32, tag="z0", bufs=3)
                p_t = soft.tile(
                    [P, LQ], F32R, tag=f"p{it}{h % 2}", name="p_t", bufs=1
                )
                nc.scalar.activation(
                    p_t[:], s2[:], AF.Exp, scale=SCALE, accum_out=z[:]
                )
                nc.vector.reciprocal(z[:], z[:])
                nc.vector.tensor_scalar(p_t[:], p_t[:], z[:], None, OP.mult)
                pn[h][it] = p_t
        for h in pair:
            pt_tiles = [
                pt_pool.tile([P, LQ], F32R, tag=f"pt{jt}", name=f"pt{jt}")
                for jt in range(NT)
            ]
            for it in range(NT):
                for jt in range(it + 1):
                    tp = ps_t.tile([P, P], F32R, tag="tps")
                    nc.tensor.transpose(
                        tp[:], pn[h][it][:, jt * P : (jt + 1) * P], identity[:]
                    )
                    nc.vector.tensor_copy(
                        pt_tiles[jt][:, it * P : (it + 1) * P], tp[:]
                    )
            for it in range(NT):
                pso = ps_small.tile([64, P], F32, tag="small")
                for jt in range(it + 1):
                    nc.tensor.matmul(
                        pso[:], kv_tok[jt][:, 64 * h : 64 * h + 64],
                        pt_tiles[jt][:, it * P : (it + 1) * P],
                        start=(jt == 0), stop=(jt == it),
                    )
                dst = att_q1[h // 2][
                    64 * (h % 2) : 64 * (h % 2) + 64, it * P : (it + 1) * P
                ]
                src_q = q_fm[h // 2][
                    64 * (h % 2) : 64 * (h % 2) + 64, it * P : (it + 1) * P
                ]
                nc.vector.tensor_tensor(dst, pso[:], src_q, OP.add)

    # ================= block1 -> y =================
    lnq1 = ln_fm(att_q1, vecs["ln1_g"], vecs["ln1_b"])
    h1 = dense_wide(lnq1, "b1_W1", vecs["b1_b1"], "h1")
    y_fm = dense_narrow(h1, "b1_W2", HID, vecs["b1_b2"], attA, "attA")

    # ================= block2 -> k1 (fm), v1 (tok + ones col) ============
    lnv, _ = ln_tok_to_fm(v_in, vecs["ln2_g"], vecs["ln2_b"])
    h2 = dense_wide(lnv, "b2_W1", vecs["b2_b1"], "h2")
    k1_fm = dense_narrow(h2, "b2_W2k", HID, vecs["b2_b2k"], attA, "attA")

    v1i = []

    def v1_post(jt, psv, bias_full):
        t = kvA.tile([P, H * 65], F32R, tag="kvA")
        tv = t[:].rearrange("p (h x) -> p h x", h=H)
        nc.vector.tensor_tensor(
            tv[:, :, 0:64], psv[:].rearrange("p (h d) -> p h d", h=H),
            bias_full[:].rearrange("p (h d) -> p h d", h=H), OP.add,
        )
        nc.vector.tensor_copy(
            tv[:, :, 64:65], ones_col8[:].rearrange("p (h x) -> p h x", x=1)
        )
        v1i.append(t)

    dense_tok(h2, "b2_W2v", b2_b2v_full, v1_post)

    # ================= attention 1 (transposed) =================
    att_q2 = [resid.tile([P, LQ], F32R, tag="resid", name=f"q2_{c}") for c in range(ND)]
    for h in range(H):
        yh = head_slice(y_fm, h)
        k1h = head_slice(k1_fm, h)
        p1t_tiles = []
        for jt in range(NT):
            pss = ps.tile([P, LQ], F32, tag="mm")
            nc.tensor.matmul(
                pss[:], k1h[:, jt * P : (jt + 1) * P], yh[:], start=True, stop=True
            )
            p1 = soft.tile([P, LQ], F32R, tag=f"p{jt}0", name="p1", bufs=1)
            nc.scalar.activation(
                p1[:], pss[:], AF.Exp, scale=SCALE, bias=vmadd_sb[:, jt : jt + 1]
            )
            p1t_tiles.append(p1)
        pso = ps.tile([65, LQ], F32, tag="mm")
        for jt in range(NT):
            nc.tensor.matmul(
                pso[:], v1i[jt][:, 65 * h : 65 * h + 65], p1t_tiles[jt][:],
                start=(jt == 0), stop=(jt == NT - 1),
            )
        rz = sm.tile([1, LQ], F32R, tag="rz1", bufs=2)
        nc.vector.reciprocal(rz[:], pso[64:65, :])
        psb = ps_small.tile([64, LQ], F32, tag="small")
        nc.tensor.matmul(psb[:], ones1[:, 0:64], rz[:], start=True, stop=True)
        o1 = soft.tile([P, LQ], F32, tag="o1")
        o1s = o1[64 * (h % 2) : 64 * (h % 2) + 64, :]
        nc.scalar.activation(o1s, pso[0:64, :], AF.Copy)
        nc.vector.tensor_tensor(o1s, o1s, psb[:], OP.mult)
        dst = att_q2[h // 2][64 * (h % 2) : 64 * (h % 2) + 64, :]
        src_q = att_q1[h // 2][64 * (h % 2) : 64 * (h % 2) + 64, :]
        nc.vector.tensor_tensor(dst, o1s, src_q, OP.add)

    # ================= block3 residual FFN =================
    lnq3 = ln_fm(att_q2, vecs["ln3_g"], vecs["ln3_b"])
    h3 = dense_wide(lnq3, "b3_W1", vecs["b3_b1"], "h3")

    def b3_evict(o, pso, m):
        nc.vector.scalar_tensor_tensor(
            o[:], pso[:], vecs["b3_b2"][:, m : m + 1], att_q2[m][:],
            OP.add, OP.add,
        )

    q3 = dense_narrow(h3, "b3_W2", HID, None, resid, "resid", evict=b3_evict)

    # ---- back to token-major, store
    for it in range(NT):
        ot = act.tile([P, D], F32, tag="out_tok")
        for c in range(ND):
            tp = ps_t.tile([P, P], F32R, tag="tps")
            nc.tensor.transpose(tp[:], q3[c][:, it * P : (it + 1) * P], identity[:])
            nc.vector.tensor_copy(ot[:, c * P : (c + 1) * P], tp[:].bitcast(F32))
        nc.sync.dma_start(out[it * P : (it + 1) * P, :], ot[:])

    ctx.close()


_NC = None


def _get_nc():
    global _NC
    if _NC is None:
        _NC = build_nc()
    return _NC


def _build_in_maps(queries, values, queries_mask, values_mask, params):
    p = params
    f32 = np.float32

    def fmvec(v, n_tiles):
        return np.ascontiguousarray(np.asarray(v, f32).reshape(n_tiles, P).T)

    w2 = np.asarray(p["b2_W2"], f32).reshape(HID, H, 2, DIM)
    b2b = np.asarray(p["b2_b2"], f32).reshape(H, 2, DIM)

    shared = {
        "b0_W": np.asarray(p["b0_W"], f32), "q0_W": np.asarray(p["q0_W"], f32),
        "ke0_W": np.asarray(p["ke0_W"], f32), "kv0_W": np.asarray(p["kv0_W"], f32),
        "kr0_W": np.asarray(p["kr0_W"], f32), "ab0_W": np.asarray(p["ab0_W"], f32),
        "ab1_W": np.asarray(p["ab1_W"], f32),
        "b1_W1": np.asarray(p["b1_W1"], f32), "b1_W2": np.asarray(p["b1_W2"], f32),
        "b2_W1": np.asarray(p["b2_W1"], f32),
        "b2_W2k": np.ascontiguousarray(w2[:, :, 0, :].reshape(HID, D)),
        "b2_W2v": np.ascontiguousarray(w2[:, :, 1, :].reshape(HID, D)),
        "b3_W1": np.asarray(p["b3_W1"], f32), "b3_W2": np.asarray(p["b3_W2"], f32),
        "b0_b": fmvec(p["b0_b"], NH), "q0_b": fmvec(p["q0_b"], ND),
        "ke0_b": fmvec(p["ke0_b"], ND), "kr0_b": fmvec(p["kr0_b"], ND),
        "b1_b1": fmvec(p["b1_b1"], NH), "b1_b2": fmvec(p["b1_b2"], ND),
        "b2_b1": fmvec(p["b2_b1"], NH),
        "b2_b2k": fmvec(b2b[:, 0, :].reshape(D), ND),
        "b3_b1": fmvec(p["b3_b1"], NH), "b3_b2": fmvec(p["b3_b2"], ND),
        "ln0_g": fmvec(p["ln0_g"], ND), "ln0_b": fmvec(p["ln0_b"], ND),
        "ln1_g": fmvec(p["ln1_g"], ND), "ln1_b": fmvec(p["ln1_b"], ND),
        "ln2_g": fmvec(p["ln2_g"], ND), "ln2_b": fmvec(p["ln2_b"], ND),
        "ln3_g": fmvec(p["ln3_g"], ND), "ln3_b": fmvec(p["ln3_b"], ND),
        "ab0_b": np.asarray(p["ab0_b"], f32).reshape(H, 1),
        "ab1_b": np.asarray(p["ab1_b"], f32).reshape(H, 1),
        "kv0_b_full": np.tile(np.asarray(p["kv0_b"], f32).reshape(1, D), (P, 1)),
        "b2_b2v_full": np.tile(b2b[:, 1, :].reshape(1, D), (P, 1)),
        "encR": np.ascontiguousarray(_sinusoid_ext_rev().T),
        "idn": np.eye(P, dtype=f32),
        "ones_r": np.ones((1, P), f32),
        "ones_c": np.ones((P, 1), f32),
        "ones8": np.ones((P, H), f32),
        "epsc": np.full((P, 1), 1e-3, f32),
    }

    qm = np.asarray(queries_mask, f32)
    vm = np.asarray(values_mask, f32)
    in_maps = []
    for c in range(B):
        m = dict(shared)
        m["q_in"] = np.ascontiguousarray(np.asarray(queries[c], f32))
        m["v_in"] = np.ascontiguousarray(np.asarray(values[c], f32))
        m["qmadd"] = ((qm[c] - 1.0) * 8e9).reshape(1, LQ).astype(f32)
        m["vmadd"] = np.ascontiguousarray(
            ((vm[c] - 1.0) * 1e9).reshape(NT, P).T
        ).astype(f32)
        in_maps.append(m)
    return in_maps


def kernel(queries, values, queries_mask, values_mask, params):
    in_maps = _build_in_maps(queries, values, queries_mask, values_mask, params)
    nc = _get_nc()
    res = run_bass_kernel_spmd(nc, in_maps, core_ids=list(range(B)))
    return np.stack([res.results[c]["out"] for c in range(B)], axis=0).astype(
        np.float32
    )


def kernel_profiled(queries, values, queries_mask, values_mask, params,
                    tmpdir=None):
    """Same as kernel() but with NTFF tracing; returns (output, results)."""
    import kernel as _self  # works both as module and as __main__ helper

    in_maps = _build_in_maps(queries, values, queries_mask, values_mask, params)
    nc = _get_nc()
    res = run_bass_kernel_spmd(
        nc, in_maps, core_ids=list(range(B)), trace=True, tmpdir=tmpdir
    )
    out = np.stack([res.results[c]["out"] for c in range(B)], axis=0).astype(
        np.float32
    )
    return out, res


# revision 16
# speedup vs baseline: 1.1827x; 1.0181x over previous
"""Trainium2 Bass kernel for nn_DecoderWithRelativePositionalAttentionLayer.

Sharding: pure data-parallel over batch (B=8 -> 8 NeuronCores, one batch
element per core, identical SPMD program, no collectives).

Key algorithmic move: the reference materializes rel = enc[ridx] as a
[Lq, Lq, D] tensor and pushes it through a [D, D] dense (38.7 GMAC); since
ridx = clip(i-j, -R, R) takes only 201 values and the causal mask kills
j > i, we project a [384, D] extended/reversed table once and realize the
per-(i, j) band with a DRAM "skew" bounce: rows written at pitch 385 and
read back at pitch 384 / offset 383 land row i's entry t at column
j = i - 383 + t.

Layouts: activations are feature-major [F, T] so weight matmuls need no
transposes (lhsT = native [in, out] weight k-tiles, rhs = activations).
Attention-0 scores run in [i, j] (softmax along the free dim, causal via
gpsimd affine_select, band added pre-exp); the probability tiles are
PE-transposed for the PV matmul. Attention-1 has no positional term and is
computed directly transposed [j, i], with the softmax normalizer taken from
an extra ones-column interleaved into the value matrix.

All matmul operands are float32r (TF32-like, full-rate on the PE array;
measured ~2.5e-4 per-matmul relative error on HW).
"""

import sys

sys.path.insert(0, "/opt/trn_rl_repo")

import contextlib
import numpy as np
import concourse.bass as bass
import concourse.mybir as mybir
import concourse.tile as tile
from concourse.bass_utils import run_bass_kernel_spmd
import bass_rust

F32 = mybir.dt.float32
F32R = mybir.dt.float32r
AF = mybir.ActivationFunctionType
OP = mybir.AluOpType
AX = mybir.AxisListType

B, LQ, LK, D, H, HID = 8, 384, 384, 512, 8, 2048
DIM = D // H
REL = 100
P = 128
NT = LQ // P   # 3 token tiles
ND = D // P    # 4 feature tiles
NH = HID // P  # 16 hidden tiles
SCALE = float(1.0 / np.sqrt(np.float32(DIM)))
G_WPITCH = LQ + 1           # 385: write pitch of the skew scratch
G_STRIDE = G_WPITCH * LQ    # per-head segment


def _split_multiwait_instructions(nc):
    """This toolchain's walrus accepts at most ONE sync wait per
    instruction, but Tile's tail drain aggregates several. Move extras onto
    same-engine nops placed immediately before the offending instruction."""
    counter = [0]

    def fresh_nop(engine, wait):
        counter[0] += 1
        nop = bass_rust.InstNoOp(name=f"WSPLIT-{counter[0]}", ins=[], outs=[])
        nop.engine = engine
        nop.sync_info = bass_rust.SyncInfo(on_wait=[wait], on_update=[])
        return nop

    for fn in nc.m.functions:
        for bb in fn.blocks:
            out = []
            changed = False
            for inst in bb.instructions:
                si = inst.sync_info
                if si is not None and len(si.on_wait) > 1:
                    waits = list(si.on_wait)
                    for w in waits[:-1]:
                        out.append(fresh_nop(inst.engine, w))
                    inst.sync_info = bass_rust.SyncInfo(
                        on_wait=[waits[-1]], on_update=list(si.on_update)
                    )
                    changed = True
                out.append(inst)
            if changed:
                bb.instructions = out


def _sinusoid_ext_rev():
    """encER[t] = enc[min(383 - t, REL) + REL]  -> [384, 512]."""
    pos = np.arange(-REL, REL + 1, dtype=np.float32)[:, None]
    i = np.arange(D // 2, dtype=np.float32)[None, :]
    ang = pos / np.power(np.float32(10000.0), 2.0 * i / np.float32(D))
    enc = np.concatenate([np.sin(ang), np.cos(ang)], axis=-1).astype(np.float32)
    o = (LQ - 1) - np.arange(LQ)
    return enc[np.minimum(o, REL) + REL]


def build_nc():
    nc = bass.Bass()

    def pin(name, shape, dt=F32R):
        return nc.declare_dram_parameter(name, list(shape), dt, isOutput=False)

    q_in = pin("q_in", [LQ, D])
    v_in = pin("v_in", [LK, D])
    qmadd = pin("qmadd", [1, LQ], F32)   # (qm-1)*8e9, pre-scale additive
    vmadd = pin("vmadd", [P, NT], F32)   # (vm-1)*1e9, post-scale additive

    w = {}
    for nm, shp in [
        ("b0_W", [D, HID]), ("q0_W", [HID, D]), ("ke0_W", [HID, D]),
        ("kv0_W", [HID, D]), ("kr0_W", [D, D]), ("ab0_W", [D, H]),
        ("ab1_W", [D, H]), ("b1_W1", [D, HID]), ("b1_W2", [HID, D]),
        ("b2_W1", [D, HID]), ("b2_W2k", [HID, D]), ("b2_W2v", [HID, D]),
        ("b3_W1", [D, HID]), ("b3_W2", [HID, D]),
    ]:
        w[nm] = pin(nm, shp)
    for nm, nt in [
        ("b0_b", NH), ("q0_b", ND), ("ke0_b", ND), ("kr0_b", ND),
        ("b1_b1", NH), ("b1_b2", ND), ("b2_b1", NH), ("b2_b2k", ND),
        ("b3_b1", NH), ("b3_b2", ND),
        ("ln0_g", ND), ("ln0_b", ND), ("ln1_g", ND), ("ln1_b", ND),
        ("ln2_g", ND), ("ln2_b", ND), ("ln3_g", ND), ("ln3_b", ND),
    ]:
        w[nm] = pin(nm, [P, nt], F32)
    w["ab0_b"] = pin("ab0_b", [H, 1], F32)
    w["ab1_b"] = pin("ab1_b", [H, 1], F32)
    w["kv0_b_full"] = pin("kv0_b_full", [P, D], F32)
    w["b2_b2v_full"] = pin("b2_b2v_full", [P, D], F32)

    encR = pin("encR", [D, LQ])
    idn = pin("idn", [P, P])
    ones_r = pin("ones_r", [1, P])
    ones_c = pin("ones_c", [P, 1])
    ones8 = pin("ones8", [P, H])
    epsc = pin("epsc", [P, 1], F32)

    out = nc.declare_dram_parameter("out", [LQ, D], F32, isOutput=True)
    g_scr = [
        [
            nc.dram_tensor(f"g_scratch{h}_{it}", [50048], mybir.dt.bfloat16)
            for it in range(NT)
        ]
        for h in range(H)
    ]

    with tile.TileContext(nc) as tc, nc.allow_low_precision(
        reason="float32r dataflow is intentional (TF32-like matmul operands)"
    ):
        _emit(
            nc, tc, q_in, v_in, qmadd, vmadd, w, encR, idn, ones_r, ones_c,
            ones8, epsc, out, g_scr,
        )
    _split_multiwait_instructions(nc)
    return nc


def _emit(nc, tc, q_in, v_in, qmadd, vmadd, w, encR, idn, ones_r, ones_c,
          ones8, epsc, out, g_scr):
    ctx = contextlib.ExitStack()

    def pool(name, bufs, **kw):
        return ctx.enter_context(tc.tile_pool(name=name, bufs=bufs, **kw))

    const = pool("const", 1)
    wbig = pool("wbig", 8)       # [128, 1024] half k-tiles
    wsm = pool("wsm", 6)         # [128, <=512] k-tiles, streaming (k-outer)
    act = pool("act", 2)         # token-major staging
    fm_ln = pool("fm_ln", 4)     # LN output streams (ln0->ln1->ln2->ln3)
    fm_raw = pool("fm_raw", 4)   # raw queries, feature-major
    attA = pool("attA", 8)       # qp, kep -> y, k1
    kvA = pool("kvA", 3)         # kv_tok -> v1i
    resid = pool("resid", 8)     # ep -> q1 -> q2 -> q3
    hid = pool("hid", 16)        # hidden tiles + LN scratch
    soft = pool("soft", 2)
    pt_pool = pool("pt", 1)
    sm = pool("sm", 1)
    ps = pool("ps", 4, space="PSUM")
    ps_t = pool("ps_t", 2, space="PSUM")
    ps_small = pool("ps_small", 2, space="PSUM")

    # ---- constants
    identity = const.tile([P, P], F32R)
    nc.sync.dma_start(identity[:], idn[:])
    ones1 = const.tile([1, P], F32R)
    nc.sync.dma_start(ones1[:], ones_r[:])
    onescol = const.tile([P, 1], F32R)
    nc.sync.dma_start(onescol[:], ones_c[:])
    ones_col8 = const.tile([P, H], F32R)
    nc.sync.dma_start(ones_col8[:], ones8[:])
    eps_t = const.tile([P, 1], F32)
    nc.sync.dma_start(eps_t[:], epsc[:])
    encR_sb = const.tile([P, ND, LQ], F32R)
    nc.sync.dma_start(encR_sb[:], encR[:].rearrange("(k p) t -> p k t", p=P))
    qmadd_sb = const.tile([1, LQ], F32)
    nc.sync.dma_start(qmadd_sb[:], qmadd[:])
    vmadd_sb = const.tile([P, NT], F32)
    nc.sync.dma_start(vmadd_sb[:], vmadd[:])

    def load_vec(name):
        t = const.tile(list(w[name].shape), F32, name=f"v_{name}")
        nc.sync.dma_start(t[:], w[name][:])
        return t

    vecs = {
        nm: load_vec(nm)
        for nm in [
            "b0_b", "q0_b", "ke0_b", "kr0_b", "b1_b1", "b1_b2", "b2_b1",
            "b2_b2k", "b3_b1", "b3_b2", "ln0_g", "ln0_b", "ln1_g", "ln1_b",
            "ln2_g", "ln2_b", "ln3_g", "ln3_b", "ab0_b", "ab1_b",
        ]
    }
    kv0_b_full = const.tile([P, D], F32)
    nc.sync.dma_start(kv0_b_full[:], w["kv0_b_full"][:])
    b2_b2v_full = const.tile([P, D], F32)
    nc.sync.dma_start(b2_b2v_full[:], w["b2_b2v_full"][:])

    def w_ktile(name, k, ncols, p, tag):
        t = p.tile([P, ncols], F32R, tag=tag, name=f"{name}k{k}")
        nc.sync.dma_start(t[:], w[name][k * P : (k + 1) * P, :])
        return t

    # ---- dense helpers -------------------------------------------------
    def dense_wide(x_tiles, wname, bias_vec, out_tag):
        """[D -> HID] with relu. m-outer over 16 output tiles. Weights load
        as [128, 1024] half-m k-tiles so the second half (and the next
        block's first half) can prefetch while the first computes."""
        halves = [
            [None] * ND,
            [None] * ND,
        ]
        for half in range(2):
            for k in range(ND):
                t = wbig.tile([P, HID // 2], F32R, tag="w1024", name=f"{wname}h{half}k{k}")
                nc.sync.dma_start(
                    t[:],
                    w[wname][k * P : (k + 1) * P, half * (HID // 2) : (half + 1) * (HID // 2)],
                )
                halves[half][k] = t
        outs = []
        for m in range(NH):
            half, mloc = divmod(m, NH // 2)
            pso = ps.tile([P, LQ], F32, tag="mm")
            for k in range(ND):
                nc.tensor.matmul(
                    pso[:], halves[half][k][:, mloc * P : (mloc + 1) * P], x_tiles[k][:],
                    start=(k == 0), stop=(k == ND - 1),
                )
            o = hid.tile([P, LQ], F32R, tag="hidden", name=f"hw{m}")
            nc.scalar.activation(o[:], pso[:], AF.Relu, bias=bias_vec[:, m : m + 1])
            outs.append(o)
        return outs

    def dense_narrow(x_tiles, wname, n_in, bias_vec, out_pool, out_tag,
                     relu=False, evict=None):
        """[n_in -> 512] feature-major. k-outer so weight k-tiles stream
        with bufs=4; the 4 output psums accumulate concurrently."""
        nk = n_in // P
        psos = [ps.tile([P, LQ], F32, tag="mm", name=f"dnps{m}") for m in range(ND)]
        for k in range(nk):
            wk = w_ktile(wname, k, D, wsm, "w512")
            for m in range(ND):
                nc.tensor.matmul(
                    psos[m][:], wk[:, m * P : (m + 1) * P], x_tiles[k][:],
                    start=(k == 0), stop=(k == nk - 1),
                )
        outs = []
        for m in range(ND):
            o = out_pool.tile([P, LQ], F32R, tag=out_tag, name=f"dn{m}")
            if evict is not None:
                evict(o, psos[m], m)
            elif relu:
                nc.scalar.activation(
                    o[:], psos[m][:], AF.Relu, bias=bias_vec[:, m : m + 1]
                )
            else:
                nc.vector.tensor_scalar(
                    o[:], psos[m][:], bias_vec[:, m : m + 1], None, OP.add
                )
            outs.append(o)
        return outs

    def dense_tok(x_tiles, wname, bias_full, post):
        """[HID -> 512] token-major out: for each token tile jt a [128, 512]
        psum accumulates x[k][:, jt] @ W[k]; bias (a host-replicated full
        tile) is folded in by the consumer. post(jt, psum, bias) consumes."""
        psos = [ps.tile([P, D], F32, tag="mm", name=f"dtps{j}") for j in range(NT)]
        for k in range(NH):
            wk = w_ktile(wname, k, D, wsm, "w512")
            for jt in range(NT):
                nc.tensor.matmul(
                    psos[jt][:], x_tiles[k][:, jt * P : (jt + 1) * P], wk[:],
                    start=(k == 0), stop=(k == NH - 1),
                )
        for jt in range(NT):
            post(jt, psos[jt], bias_full)

    # ---- layernorm helpers ---------------------------------------------
    def ln_tok_to_fm(src_dram, g_vec, b_vec, want_raw=False):
        fm_tiles = [fm_ln.tile([P, LQ], F32R, tag="lnstream", name=f"lnfm{c}") for c in range(ND)]
        raw_tiles = (
            [fm_raw.tile([P, LQ], F32R, tag="qraw", name=f"qraw{c}") for c in range(ND)]
            if want_raw else None
        )
        for it in range(NT):
            xt = act.tile([P, D], F32R, tag="xt_in")
            nc.sync.dma_start(xt[:], src_dram[it * P : (it + 1) * P, :])
            stats = sm.tile([P, nc.vector.BN_STATS_DIM], F32, tag="bnst", bufs=2)
            nc.vector.bn_stats(stats[:], xt[:].bitcast(F32))
            mv = sm.tile([P, nc.vector.BN_AGGR_DIM], F32, tag="bnmv", bufs=2)
            nc.vector.bn_aggr(mv[:], stats[:])
            sd = sm.tile([P, 1], F32, tag="bnsd", bufs=2)
            nc.scalar.activation(sd[:], mv[:, 1:2], AF.Sqrt, bias=eps_t[:])
            nc.vector.reciprocal(sd[:], sd[:])
            xn = act.tile([P, D], F32R, tag="xt_n")
            nc.vector.tensor_scalar(
                xn[:], xt[:], mv[:, 0:1], sd[:], OP.subtract, OP.mult
            )
            for c in range(ND):
                tp = ps_t.tile([P, P], F32R, tag="tps")
                nc.tensor.transpose(tp[:], xn[:, c * P : (c + 1) * P], identity[:])
                nc.vector.tensor_scalar(
                    fm_tiles[c][:, it * P : (it + 1) * P], tp[:],
                    g_vec[:, c : c + 1], b_vec[:, c : c + 1], OP.mult, OP.add,
                )
                if raw_tiles is not None:
                    tpr = ps_t.tile([P, P], F32R, tag="tps")
                    nc.tensor.transpose(tpr[:], xt[:, c * P : (c + 1) * P], identity[:])
                    nc.vector.tensor_copy(raw_tiles[c][:, it * P : (it + 1) * P], tpr[:])
        return fm_tiles, raw_tiles

    def ln_fm(x_tiles, g_vec, b_vec):
        """LayerNorm over the partition (feature) direction of feature-major
        tiles, via ones-matmul sums and a PE broadcast."""
        s_ps = ps_small.tile([1, LQ], F32, tag="small")
        s2_ps = ps_small.tile([1, LQ], F32, tag="small")
        for c in range(ND):
            nc.tensor.matmul(
                s_ps[:], onescol[:], x_tiles[c][:],
                start=(c == 0), stop=(c == ND - 1),
            )
        sqs = []
        for c in range(ND):
            sq = hid.tile([P, LQ], F32R, tag="hidden", name=f"sq{c}")
            nc.scalar.activation(sq[:], x_tiles[c][:], AF.Square)
            sqs.append(sq)
        for c in range(ND):
            nc.tensor.matmul(
                s2_ps[:], onescol[:], sqs[c][:],
                start=(c == 0), stop=(c == ND - 1),
            )
        mu = sm.tile([1, LQ], F32R, tag="lnmu")
        nc.vector.tensor_scalar(mu[:], s_ps[:], 1.0 / D, None, OP.mult)
        var = sm.tile([1, LQ], F32, tag="lnvar")
        nc.vector.tensor_scalar(var[:], s2_ps[:], 1.0 / D, None, OP.mult)
        m2 = sm.tile([1, LQ], F32, tag="lnm2")
        nc.vector.tensor_tensor(m2[:], mu[:].bitcast(F32), mu[:].bitcast(F32), OP.mult)
        nc.vector.tensor_tensor(var[:], var[:], m2[:], OP.subtract)
        sd = sm.tile([1, LQ], F32R, tag="lnsd")
        nc.scalar.activation(sd[:], var[:], AF.Sqrt, bias=eps_t[0:1, 0:1])
        nc.vector.reciprocal(sd[:], sd[:])
        mub = ps_small.tile([P, LQ], F32, tag="small")
        nc.tensor.matmul(mub[:], ones1[:], mu[:], start=True, stop=True)
        sdb = ps_small.tile([P, LQ], F32, tag="small")
        nc.tensor.matmul(sdb[:], ones1[:], sd[:], start=True, stop=True)
        outs = []
        for c in range(ND):
            t1 = hid.tile([P, LQ], F32, tag="hidden", name=f"lt{c}")
            nc.vector.tensor_tensor(t1[:], x_tiles[c][:].bitcast(F32), mub[:], OP.subtract)
            nc.vector.tensor_tensor(t1[:], t1[:], sdb[:], OP.mult)
            o = fm_ln.tile([P, LQ], F32R, tag="lnstream", name=f"lno{c}")
            nc.vector.tensor_scalar(
                o[:], t1[:], g_vec[:, c : c + 1], b_vec[:, c : c + 1],
                OP.mult, OP.add,
            )
            outs.append(o)
        return outs

    def head_slice(tiles, h):
        return tiles[h // 2][64 * (h % 2) : 64 * (h % 2) + 64, :]

    # ================= block0 =================
    lnq, q_fm = ln_tok_to_fm(q_in, vecs["ln0_g"], vecs["ln0_b"], want_raw=True)
    x_tiles = dense_wide(lnq, "b0_W", vecs["b0_b"], "x")

    qp = dense_narrow(x_tiles, "q0_W", HID, vecs["q0_b"], attA, "attA")
    kep = dense_narrow(x_tiles, "ke0_W", HID, vecs["ke0_b"], attA, "attA")

    kv_tok = []

    def kv_post(jt, psv, bias_full):
        t = kvA.tile([P, H * 65], F32R, tag="kvA")
        nc.vector.tensor_tensor(t[:, 0:D], psv[:], bias_full[:], OP.add)
        kv_tok.append(t)

    dense_tok(x_tiles, "kv0_W", kv0_b_full, kv_post)

    # ---- rel tables
    ep_fm = []
    krk = [w_ktile("kr0_W", k, D, wsm, "w512") for k in range(ND)]
    for m in range(ND):
        pse = ps.tile([P, LQ], F32, tag="mm")
        for k in range(ND):
            nc.tensor.matmul(
                pse[:], krk[k][:, m * P : (m + 1) * P], encR_sb[:, k, :],
                start=(k == 0), stop=(k == ND - 1),
            )
        o = resid.tile([P, LQ], F32R, tag="resid", name=f"ep{m}")
        nc.vector.tensor_scalar(o[:], pse[:], vecs["kr0_b"][:, m : m + 1], None, OP.add)
        ep_fm.append(o)

    ab1k = [w_ktile("ab1_W", k, H, wsm, "w8") for k in range(ND)]
    gam_ps = ps_small.tile([H, LQ], F32, tag="small")
    for k in range(ND):
        nc.tensor.matmul(
            gam_ps[:], ab1k[k][:], ep_fm[k][:], start=(k == 0), stop=(k == ND - 1)
        )
    gam = sm.tile([H, LQ], F32R, tag="gam")
    nc.vector.tensor_scalar(gam[:], gam_ps[:], vecs["ab1_b"][:], None, OP.add)

    ab0k = [w_ktile("ab0_W", k, H, wsm, "w8") for k in range(ND)]
    bke_ps = ps_small.tile([H, LQ], F32, tag="small")
    for k in range(ND):
        nc.tensor.matmul(
            bke_ps[:], ab0k[k][:], kep[k][:], start=(k == 0), stop=(k == ND - 1)
        )
    bke = sm.tile([H, LQ], F32R, tag="bke")
    nc.vector.tensor_scalar(bke[:], bke_ps[:], vecs["ab0_b"][:], None, OP.add)

    gam_all = sm.tile([1, H * LQ], F32R, tag="gamall")
    bkem_all = sm.tile([1, H * LQ], F32R, tag="bkemall")
    for h in range(H):
        nc.sync.dma_start(gam_all[:, h * LQ : (h + 1) * LQ], gam[h : h + 1, :])
        nc.sync.dma_start(bkem_all[:, h * LQ : (h + 1) * LQ], bke[h : h + 1, :])
        nc.vector.tensor_tensor(
            bkem_all[:, h * LQ : (h + 1) * LQ],
            bkem_all[:, h * LQ : (h + 1) * LQ], qmadd_sb[:], OP.add,
        )
    gam_rows = [gam_all[:, h * LQ : (h + 1) * LQ] for h in range(H)]
    bkem_rows = [bkem_all[:, h * LQ : (h + 1) * LQ] for h in range(H)]

    # ================= attention 0 =================
    # Phase A: all rel-position band tiles -> DRAM (skew write). Reads in
    # phase B then never stall on the HBM round trip.
    for h in range(H):
        qh = head_slice(qp, h)
        eph = head_slice(ep_fm, h)
        gb_ps = ps.tile([P, LQ], F32, tag="mm", name="gb_ps")
        nc.tensor.matmul(gb_ps[:], ones1[:], gam_rows[h], start=True, stop=True)
        gamb = soft.tile([P, LQ], F32, tag="gamb", name="gamb")
        nc.scalar.activation(gamb[:], gb_ps[:], AF.Copy)
        for it in range(NT):
            psb = ps.tile([P, LQ], F32, tag="mm")
            nc.tensor.matmul(
                psb[:], qh[:, it * P : (it + 1) * P], eph[:],
                start=True, stop=True,
            )
            band_w = soft.tile(
                [P, G_WPITCH], mybir.dt.bfloat16, tag="bandw", name="band_w"
            )
            nc.vector.tensor_tensor(band_w[:, 0:LQ], psb[:], gamb[:], OP.add)
            nc.vector.memset(band_w[:, LQ : LQ + 1], 0.0)
            gw = bass.AP(
                tensor=g_scr[h][it], offset=0, ap=[[G_WPITCH, P], [1, G_WPITCH]]
            )
            nc.sync.dma_start(gw, band_w[:])

    # Phase B: scores + softmax + PV, head pairs interleaved so the K=64
    # matmuls land on disjoint PE row groups and run concurrently.
    att_q1 = [resid.tile([P, LQ], F32R, tag="resid", name=f"q1_{c}") for c in range(ND)]
    for hp in range(H // 2):
        pair = (2 * hp, 2 * hp + 1)
        bkebs = {}
        for h in pair:
            bk_ps = ps.tile([P, LQ], F32, tag="mm", name="bk_ps")
            nc.tensor.matmul(bk_ps[:], ones1[:], bkem_rows[h], start=True, stop=True)
            bkeb = soft.tile([P, LQ], F32, tag=f"bkeb{h % 2}", name="bkeb")
            nc.scalar.activation(bkeb[:], bk_ps[:], AF.Copy)
            bkebs[h] = bkeb
        pn = {h: [None] * NT for h in pair}
        for it in range(NT):
            for h in pair:
                qh = head_slice(qp, h)
                keh = head_slice(kep, h)
                band = soft.tile(
                    [P, LQ], mybir.dt.bfloat16, tag=f"band{h % 2}", name="band",
                    bufs=2,
                )
                gr_ap = bass.AP(
                    tensor=g_scr[h][it], offset=(LQ - 1) - it * P,
                    ap=[[LQ, P], [1, LQ]],
                )
                nc.sync.dma_start(band[:], gr_ap)
                pss = ps.tile([P, LQ], F32, tag="mm")
                nc.tensor.matmul(
                    pss[:], qh[:, it * P : (it + 1) * P], keh[:],
                    start=True, stop=True,
                )
                s2 = soft.tile([P, LQ], F32, tag="s2", name="s2", bufs=3)
                nc.vector.tensor_tensor(s2[:], pss[:], band[:], OP.add)
                nc.vector.tensor_tensor(s2[:], s2[:], bkebs[h][:], OP.add)
                nc.gpsimd.affine_select(
                    out=s2[:], in_=s2[:], compare_op=OP.is_ge, fill=-1e30,
                    base=it * P, pattern=[[-1, LQ]], channel_multiplier=1,
                )
                z = sm.tile([P, 1], F32, tag="z0", bufs=3)
                p_t = soft.tile(
                    [P, LQ], F32R, tag=f"p{it}{h % 2}", name="p_t", bufs=1
                )
                nc.scalar.activation(
                    p_t[:], s2[:], AF.Exp, scale=SCALE, accum_out=z[:]
                )
                nc.vector.reciprocal(z[:], z[:])
                nc.vector.tensor_scalar(p_t[:], p_t[:], z[:], None, OP.mult)
                pn[h][it] = p_t
        for h in pair:
            pt_tiles = [
                pt_pool.tile([P, LQ], F32R, tag=f"pt{jt}", name=f"pt{jt}")
                for jt in range(NT)
            ]
            for it in range(NT):
                for jt in range(it + 1):
                    tp = ps_t.tile([P, P], F32R, tag="tps")
                    nc.tensor.transpose(
                        tp[:], pn[h][it][:, jt * P : (jt + 1) * P], identity[:]
                    )
                    nc.vector.tensor_copy(
                        pt_tiles[jt][:, it * P : (it + 1) * P], tp[:]
                    )
            for it in range(NT):
                pso = ps_small.tile([64, P], F32, tag="small")
                for jt in range(it + 1):
                    nc.tensor.matmul(
                        pso[:], kv_tok[jt][:, 64 * h : 64 * h + 64],
                        pt_tiles[jt][:, it * P : (it + 1) * P],
                        start=(jt == 0), stop=(jt == it),
                    )
                dst = att_q1[h // 2][
                    64 * (h % 2) : 64 * (h % 2) + 64, it * P : (it + 1) * P
                ]
                src_q = q_fm[h // 2][
                    64 * (h % 2) : 64 * (h % 2) + 64, it * P : (it + 1) * P
                ]
                nc.vector.tensor_tensor(dst, pso[:], src_q, OP.add)

    # ================= block1 -> y =================
    lnq1 = ln_fm(att_q1, vecs["ln1_g"], vecs["ln1_b"])
    h1 = dense_wide(lnq1, "b1_W1", vecs["b1_b1"], "h1")
    y_fm = dense_narrow(h1, "b1_W2", HID, vecs["b1_b2"], attA, "attA")

    # ================= block2 -> k1 (fm), v1 (tok + ones col) ============
    lnv, _ = ln_tok_to_fm(v_in, vecs["ln2_g"], vecs["ln2_b"])
    h2 = dense_wide(lnv, "b2_W1", vecs["b2_b1"], "h2")
    k1_fm = dense_narrow(h2, "b2_W2k", HID, vecs["b2_b2k"], attA, "attA")

    v1i = []

    def v1_post(jt, psv, bias_full):
        t = kvA.tile([P, H * 65], F32R, tag="kvA")
        tv = t[:].rearrange("p (h x) -> p h x", h=H)
        nc.vector.tensor_tensor(
            tv[:, :, 0:64], psv[:].rearrange("p (h d) -> p h d", h=H),
            bias_full[:].rearrange("p (h d) -> p h d", h=H), OP.add,
        )
        nc.vector.tensor_copy(
            tv[:, :, 64:65], ones_col8[:].rearrange("p (h x) -> p h x", x=1)
        )
        v1i.append(t)

    dense_tok(h2, "b2_W2v", b2_b2v_full, v1_post)

    # ================= attention 1 (transposed) =================
    att_q2 = [resid.tile([P, LQ], F32R, tag="resid", name=f"q2_{c}") for c in range(ND)]
    for h in range(H):
        yh = head_slice(y_fm, h)
        k1h = head_slice(k1_fm, h)
        p1t_tiles = []
        for jt in range(NT):
            pss = ps.tile([P, LQ], F32, tag="mm")
            nc.tensor.matmul(
                pss[:], k1h[:, jt * P : (jt + 1) * P], yh[:], start=True, stop=True
            )
            p1 = soft.tile([P, LQ], F32R, tag=f"p{jt}0", name="p1", bufs=1)
            nc.scalar.activation(
                p1[:], pss[:], AF.Exp, scale=SCALE, bias=vmadd_sb[:, jt : jt + 1]
            )
            p1t_tiles.append(p1)
        pso = ps.tile([65, LQ], F32, tag="mm")
        for jt in range(NT):
            nc.tensor.matmul(
                pso[:], v1i[jt][:, 65 * h : 65 * h + 65], p1t_tiles[jt][:],
                start=(jt == 0), stop=(jt == NT - 1),
            )
        rz = sm.tile([1, LQ], F32R, tag="rz1", bufs=2)
        nc.vector.reciprocal(rz[:], pso[64:65, :])
        psb = ps_small.tile([64, LQ], F32, tag="small")
        nc.tensor.matmul(psb[:], ones1[:, 0:64], rz[:], start=True, stop=True)
        o1 = soft.tile([P, LQ], F32, tag="o1")
        o1s = o1[64 * (h % 2) : 64 * (h % 2) + 64, :]
        nc.scalar.activation(o1s, pso[0:64, :], AF.Copy)
        nc.vector.tensor_tensor(o1s, o1s, psb[:], OP.mult)
        dst = att_q2[h // 2][64 * (h % 2) : 64 * (h % 2) + 64, :]
        src_q = att_q1[h // 2][64 * (h % 2) : 64 * (h % 2) + 64, :]
        nc.vector.tensor_tensor(dst, o1s, src_q, OP.add)

    # ================= block3 residual FFN =================
    lnq3 = ln_fm(att_q2, vecs["ln3_g"], vecs["ln3_b"])
    h3 = dense_wide(lnq3, "b3_W1", vecs["b3_b1"], "h3")

    def b3_evict(o, pso, m):
        nc.vector.scalar_tensor_tensor(
            o[:], pso[:], vecs["b3_b2"][:, m : m + 1], att_q2[m][:],
            OP.add, OP.add,
        )

    q3 = dense_narrow(h3, "b3_W2", HID, None, resid, "resid", evict=b3_evict)

    # ---- back to token-major, store
    for it in range(NT):
        ot = act.tile([P, D], F32, tag="out_tok")
        for c in range(ND):
            tp = ps_t.tile([P, P], F32R, tag="tps")
            nc.tensor.transpose(tp[:], q3[c][:, it * P : (it + 1) * P], identity[:])
            nc.vector.tensor_copy(ot[:, c * P : (c + 1) * P], tp[:].bitcast(F32))
        nc.sync.dma_start(out[it * P : (it + 1) * P, :], ot[:])

    ctx.close()


_NC = None


def _get_nc():
    global _NC
    if _NC is None:
        _NC = build_nc()
    return _NC


def _build_in_maps(queries, values, queries_mask, values_mask, params):
    p = params
    f32 = np.float32

    def fmvec(v, n_tiles):
        return np.ascontiguousarray(np.asarray(v, f32).reshape(n_tiles, P).T)

    w2 = np.asarray(p["b2_W2"], f32).reshape(HID, H, 2, DIM)
    b2b = np.asarray(p["b2_b2"], f32).reshape(H, 2, DIM)

    shared = {
        "b0_W": np.asarray(p["b0_W"], f32), "q0_W": np.asarray(p["q0_W"], f32),
        "ke0_W": np.asarray(p["ke0_W"], f32), "kv0_W": np.asarray(p["kv0_W"], f32),
        "kr0_W": np.asarray(p["kr0_W"], f32), "ab0_W": np.asarray(p["ab0_W"], f32),
        "ab1_W": np.asarray(p["ab1_W"], f32),
        "b1_W1": np.asarray(p["b1_W1"], f32), "b1_W2": np.asarray(p["b1_W2"], f32),
        "b2_W1": np.asarray(p["b2_W1"], f32),
        "b2_W2k": np.ascontiguousarray(w2[:, :, 0, :].reshape(HID, D)),
        "b2_W2v": np.ascontiguousarray(w2[:, :, 1, :].reshape(HID, D)),
        "b3_W1": np.asarray(p["b3_W1"], f32), "b3_W2": np.asarray(p["b3_W2"], f32),
        "b0_b": fmvec(p["b0_b"], NH), "q0_b": fmvec(p["q0_b"], ND),
        "ke0_b": fmvec(p["ke0_b"], ND), "kr0_b": fmvec(p["kr0_b"], ND),
        "b1_b1": fmvec(p["b1_b1"], NH), "b1_b2": fmvec(p["b1_b2"], ND),
        "b2_b1": fmvec(p["b2_b1"], NH),
        "b2_b2k": fmvec(b2b[:, 0, :].reshape(D), ND),
        "b3_b1": fmvec(p["b3_b1"], NH), "b3_b2": fmvec(p["b3_b2"], ND),
        "ln0_g": fmvec(p["ln0_g"], ND), "ln0_b": fmvec(p["ln0_b"], ND),
        "ln1_g": fmvec(p["ln1_g"], ND), "ln1_b": fmvec(p["ln1_b"], ND),
        "ln2_g": fmvec(p["ln2_g"], ND), "ln2_b": fmvec(p["ln2_b"], ND),
        "ln3_g": fmvec(p["ln3_g"], ND), "ln3_b": fmvec(p["ln3_b"], ND),
        "ab0_b": np.asarray(p["ab0_b"], f32).reshape(H, 1),
        "ab1_b": np.asarray(p["ab1_b"], f32).reshape(H, 1),
        "kv0_b_full": np.tile(np.asarray(p["kv0_b"], f32).reshape(1, D), (P, 1)),
        "b2_b2v_full": np.tile(b2b[:, 1, :].reshape(1, D), (P, 1)),
        "encR": np.ascontiguousarray(_sinusoid_ext_rev().T),
        "idn": np.eye(P, dtype=f32),
        "ones_r": np.ones((1, P), f32),
        "ones_c": np.ones((P, 1), f32),
        "ones8": np.ones((P, H), f32),
        "epsc": np.full((P, 1), 1e-3, f32),
    }

    qm = np.asarray(queries_mask, f32)
    vm = np.asarray(values_mask, f32)
    in_maps = []
    for c in range(B):
        m = dict(shared)
        m["q_in"] = np.ascontiguousarray(np.asarray(queries[c], f32))
        m["v_in"] = np.ascontiguousarray(np.asarray(values[c], f32))
        m["qmadd"] = ((qm[c] - 1.0) * 8e9).reshape(1, LQ).astype(f32)
        m["vmadd"] = np.ascontiguousarray(
            ((vm[c] - 1.0) * 1e9).reshape(NT, P).T
        ).astype(f32)
        in_maps.append(m)
    return in_maps


def kernel(queries, values, queries_mask, values_mask, params):
    in_maps = _build_in_maps(queries, values, queries_mask, values_mask, params)
    nc = _get_nc()
    res = run_bass_kernel_spmd(nc, in_maps, core_ids=list(range(B)))
    return np.stack([res.results[c]["out"] for c in range(B)], axis=0).astype(
        np.float32
    )


def kernel_profiled(queries, values, queries_mask, values_mask, params,
                    tmpdir=None):
    """Same as kernel() but with NTFF tracing; returns (output, results)."""
    import kernel as _self  # works both as module and as __main__ helper

    in_maps = _build_in_maps(queries, values, queries_mask, values_mask, params)
    nc = _get_nc()
    res = run_bass_kernel_spmd(
        nc, in_maps, core_ids=list(range(B)), trace=True, tmpdir=tmpdir
    )
    out = np.stack([res.results[c]["out"] for c in range(B)], axis=0).astype(
        np.float32
    )
    return out, res


# revision 21
# speedup vs baseline: 1.3009x; 1.0999x over previous
"""Trainium2 Bass kernel for nn_DecoderWithRelativePositionalAttentionLayer.

Sharding: pure data-parallel over batch (B=8 -> 8 NeuronCores, one batch
element per core, identical SPMD program, no collectives).

Key algorithmic move: the reference materializes rel = enc[ridx] as a
[Lq, Lq, D] tensor and pushes it through a [D, D] dense (38.7 GMAC); since
ridx = clip(i-j, -R, R) takes only 201 values and the causal mask kills
j > i, we project a [384, D] extended/reversed table once and realize the
per-(i, j) band with a DRAM "skew" bounce: rows written at pitch 385 and
read back at pitch 384 / offset 383 land row i's entry t at column
j = i - 383 + t.

Layouts: activations are feature-major [F, T] so weight matmuls need no
transposes (lhsT = native [in, out] weight k-tiles, rhs = activations).
Attention-0 scores run in [i, j] (softmax along the free dim, causal via
gpsimd affine_select, band added pre-exp); the probability tiles are
PE-transposed for the PV matmul. Attention-1 has no positional term and is
computed directly transposed [j, i], with the softmax normalizer taken from
an extra ones-column interleaved into the value matrix.

All matmul operands are float32r (TF32-like, full-rate on the PE array;
measured ~2.5e-4 per-matmul relative error on HW).
"""

import sys

sys.path.insert(0, "/opt/trn_rl_repo")

import contextlib
import numpy as np
import concourse.bass as bass
import concourse.mybir as mybir
import concourse.tile as tile
from concourse.bass_utils import run_bass_kernel_spmd
import bass_rust

F32 = mybir.dt.float32
F32R = mybir.dt.float32r
AF = mybir.ActivationFunctionType
OP = mybir.AluOpType
AX = mybir.AxisListType

B, LQ, LK, D, H, HID = 8, 384, 384, 512, 8, 2048
DIM = D // H
REL = 100
P = 128
NT = LQ // P   # 3 token tiles
ND = D // P    # 4 feature tiles
NH = HID // P  # 16 hidden tiles
SCALE = float(1.0 / np.sqrt(np.float32(DIM)))
G_WPITCH = LQ + 1           # 385: write pitch of the skew scratch
G_STRIDE = G_WPITCH * LQ    # per-head segment

# packed-constant column layout: name -> (col, ncols)
_VP = {}
_c = 0
for _nm, _n in [
    ("b0_b", NH), ("q0_b", ND), ("ke0_b", ND), ("kr0_b", ND),
    ("b1_b1", NH), ("b1_b2", ND), ("b2_b1", NH), ("b2_b2k", ND),
    ("b3_b1", NH), ("b3_b2", ND),
    ("ln0_g", ND), ("ln0_b", ND), ("ln1_g", ND), ("ln1_b", ND),
    ("ln2_g", ND), ("ln2_b", ND), ("ln3_g", ND), ("ln3_b", ND),
    ("ab0_b", 1), ("ab1_b", 1), ("epsc", 1),
    ("kv0_b_full", D), ("b2_b2v_full", D),
]:
    _VP[_nm] = (_c, _n)
    _c += _n
VPACK_COLS = _c
_RP = {}
_c = 0
for _nm, _n in [("idn", P), ("ones_r", 1), ("ones_c", 1), ("ones8", H)]:
    _RP[_nm] = (_c, _n)
    _c += _n
RPACK_COLS = _c



def _split_multiwait_instructions(nc):
    """This toolchain's walrus accepts at most ONE sync wait per
    instruction, but Tile's tail drain aggregates several. Move extras onto
    same-engine nops placed immediately before the offending instruction."""
    counter = [0]

    def fresh_nop(engine, wait):
        counter[0] += 1
        nop = bass_rust.InstNoOp(name=f"WSPLIT-{counter[0]}", ins=[], outs=[])
        nop.engine = engine
        nop.sync_info = bass_rust.SyncInfo(on_wait=[wait], on_update=[])
        return nop

    for fn in nc.m.functions:
        for bb in fn.blocks:
            out = []
            changed = False
            for inst in bb.instructions:
                si = inst.sync_info
                if si is not None and len(si.on_wait) > 1:
                    waits = list(si.on_wait)
                    for w in waits[:-1]:
                        out.append(fresh_nop(inst.engine, w))
                    inst.sync_info = bass_rust.SyncInfo(
                        on_wait=[waits[-1]], on_update=list(si.on_update)
                    )
                    changed = True
                out.append(inst)
            if changed:
                bb.instructions = out


def _sinusoid_ext_rev():
    """encER[t] = enc[min(383 - t, REL) + REL]  -> [384, 512]."""
    pos = np.arange(-REL, REL + 1, dtype=np.float32)[:, None]
    i = np.arange(D // 2, dtype=np.float32)[None, :]
    ang = pos / np.power(np.float32(10000.0), 2.0 * i / np.float32(D))
    enc = np.concatenate([np.sin(ang), np.cos(ang)], axis=-1).astype(np.float32)
    o = (LQ - 1) - np.arange(LQ)
    return enc[np.minimum(o, REL) + REL]


def build_nc():
    nc = bass.Bass()

    def pin(name, shape, dt=F32R):
        return nc.declare_dram_parameter(name, list(shape), dt, isOutput=False)

    q_in = pin("q_in", [LQ, D])
    v_in = pin("v_in", [LK, D])
    qmadd = pin("qmadd", [1, LQ], F32)   # (qm-1)*8e9, pre-scale additive
    vmadd = pin("vmadd", [P, NT], F32)   # (vm-1)*1e9, post-scale additive

    w = {}
    for nm, shp in [
        ("b0_W", [D, HID]), ("q0_W", [HID, D]), ("ke0_W", [HID, D]),
        ("kv0_W", [HID, D]), ("kr0_W", [D, D]), ("ab0_W", [D, H]),
        ("ab1_W", [D, H]), ("b1_W1", [D, HID]), ("b1_W2", [HID, D]),
        ("b2_W1", [D, HID]), ("b2_W2k", [HID, D]), ("b2_W2v", [HID, D]),
        ("b3_W1", [D, HID]), ("b3_W2", [HID, D]),
    ]:
        w[nm] = pin(nm, shp)
    # all small f32 vectors packed into one [128, VPACK] tensor (host
    # places each at a known column), all f32r constants into another
    w["vpack"] = pin("vpack", [P, VPACK_COLS], F32)
    w["rpack"] = pin("rpack", [P, RPACK_COLS], F32R)

    encR = pin("encR", [D, LQ])
    ones_r = pin("ones_r", [1, P])

    out = nc.declare_dram_parameter("out", [LQ, D], F32, isOutput=True)
    g_scr = [
        [
            nc.dram_tensor(f"g_scratch{h}_{it}", [50048], mybir.dt.bfloat16)
            for it in range(NT)
        ]
        for h in range(H)
    ]

    with tile.TileContext(nc) as tc, nc.allow_low_precision(
        reason="float32r dataflow is intentional (TF32-like matmul operands)"
    ):
        _emit(nc, tc, q_in, v_in, qmadd, vmadd, w, encR, ones_r, out, g_scr)
    _split_multiwait_instructions(nc)
    return nc


def _emit(nc, tc, q_in, v_in, qmadd, vmadd, w, encR, ones_r, out, g_scr):
    ctx = contextlib.ExitStack()

    def pool(name, bufs, **kw):
        return ctx.enter_context(tc.tile_pool(name=name, bufs=bufs, **kw))

    const = pool("const", 1)
    wbig = pool("wbig", 8)       # [128, 1024] half k-tiles
    wsm = pool("wsm", 6)         # [128, <=512] k-tiles, streaming (k-outer)
    act = pool("act", 2)         # token-major staging
    fm_ln = pool("fm_ln", 4)     # LN output streams (ln0->ln1->ln2->ln3)
    fm_raw = pool("fm_raw", 4)   # raw queries, feature-major
    attA = pool("attA", 8)       # qp, kep -> y, k1
    kvA = pool("kvA", 3)         # kv_tok -> v1i
    resid = pool("resid", 8)     # ep -> q1 -> q2 -> q3
    hid = pool("hid", 16)        # hidden tiles + LN scratch
    soft = pool("soft", 2)
    pt_pool = pool("pt", 1)
    sm = pool("sm", 1)
    ps = pool("ps", 4, space="PSUM")
    ps_t = pool("ps_t", 2, space="PSUM")
    ps_small = pool("ps_small", 2, space="PSUM")

    # ---- constants (two packed DMAs + encR + masks)
    vpack_sb = const.tile([P, VPACK_COLS], F32)
    nc.sync.dma_start(vpack_sb[:], w["vpack"][:])
    rpack_sb = const.tile([P, RPACK_COLS], F32R)
    nc.sync.dma_start(rpack_sb[:], w["rpack"][:])

    def vp(name):
        c, n = _VP[name]
        return vpack_sb[:, c : c + n]

    vecs = {nm: vp(nm) for nm in _VP}
    identity = rpack_sb[:, _RP["idn"][0] : _RP["idn"][0] + P]
    onescol = rpack_sb[:, _RP["ones_c"][0] : _RP["ones_c"][0] + 1]
    ones_col8 = rpack_sb[:, _RP["ones8"][0] : _RP["ones8"][0] + H]
    eps_t = vp("epsc")
    kv0_b_full = vp("kv0_b_full")
    b2_b2v_full = vp("b2_b2v_full")
    encR_sb = const.tile([P, ND, LQ], F32R)
    nc.sync.dma_start(encR_sb[:], encR[:].rearrange("(k p) t -> p k t", p=P))
    qmadd_sb = const.tile([1, LQ], F32)
    nc.sync.dma_start(qmadd_sb[:], qmadd[:])
    vmadd_sb = const.tile([P, NT], F32)
    nc.sync.dma_start(vmadd_sb[:], vmadd[:])
    ones1 = const.tile([1, P], F32R)
    nc.sync.dma_start(ones1[:], ones_r[:])

    def w_ktile(name, k, ncols, p, tag):
        t = p.tile([P, ncols], F32R, tag=tag, name=f"{name}k{k}")
        nc.sync.dma_start(t[:], w[name][k * P : (k + 1) * P, :])
        return t

    # ---- dense helpers -------------------------------------------------
    def dense_wide(x_tiles, wname, bias_vec, out_tag):
        """[D -> HID] with relu. m-outer over 16 output tiles. Weights load
        as [128, 1024] half-m k-tiles so the second half (and the next
        block's first half) can prefetch while the first computes."""
        halves = [
            [None] * ND,
            [None] * ND,
        ]
        for half in range(2):
            for k in range(ND):
                t = wbig.tile([P, HID // 2], F32R, tag="w1024", name=f"{wname}h{half}k{k}")
                nc.sync.dma_start(
                    t[:],
                    w[wname][k * P : (k + 1) * P, half * (HID // 2) : (half + 1) * (HID // 2)],
                )
                halves[half][k] = t
        outs = []
        for m in range(NH):
            half, mloc = divmod(m, NH // 2)
            pso = ps.tile([P, LQ], F32, tag="mm")
            for k in range(ND):
                nc.tensor.matmul(
                    pso[:], halves[half][k][:, mloc * P : (mloc + 1) * P], x_tiles[k][:],
                    start=(k == 0), stop=(k == ND - 1),
                )
            o = hid.tile([P, LQ], F32R, tag="hidden", name=f"hw{m}")
            nc.scalar.activation(o[:], pso[:], AF.Relu, bias=bias_vec[:, m : m + 1])
            outs.append(o)
        return outs

    def dense_narrow(x_tiles, wname, n_in, bias_vec, out_pool, out_tag,
                     relu=False, evict=None):
        """[n_in -> 512] feature-major. k-outer so weight k-tiles stream
        with bufs=4; the 4 output psums accumulate concurrently."""
        nk = n_in // P
        psos = [ps.tile([P, LQ], F32, tag="mm", name=f"dnps{m}") for m in range(ND)]
        for k in range(nk):
            wk = w_ktile(wname, k, D, wsm, "w512")
            for m in range(ND):
                nc.tensor.matmul(
                    psos[m][:], wk[:, m * P : (m + 1) * P], x_tiles[k][:],
                    start=(k == 0), stop=(k == nk - 1),
                )
        outs = []
        for m in range(ND):
            o = out_pool.tile([P, LQ], F32R, tag=out_tag, name=f"dn{m}")
            if evict is not None:
                evict(o, psos[m], m)
            elif relu:
                nc.scalar.activation(
                    o[:], psos[m][:], AF.Relu, bias=bias_vec[:, m : m + 1]
                )
            else:
                nc.vector.tensor_scalar(
                    o[:], psos[m][:], bias_vec[:, m : m + 1], None, OP.add
                )
            outs.append(o)
        return outs

    def dense_tok(x_tiles, wname, bias_full, post):
        """[HID -> 512] token-major out: for each token tile jt a [128, 512]
        psum accumulates x[k][:, jt] @ W[k]; bias (a host-replicated full
        tile) is folded in by the consumer. post(jt, psum, bias) consumes."""
        psos = [ps.tile([P, D], F32, tag="mm", name=f"dtps{j}") for j in range(NT)]
        for k in range(NH):
            wk = w_ktile(wname, k, D, wsm, "w512")
            for jt in range(NT):
                nc.tensor.matmul(
                    psos[jt][:], x_tiles[k][:, jt * P : (jt + 1) * P], wk[:],
                    start=(k == 0), stop=(k == NH - 1),
                )
        for jt in range(NT):
            post(jt, psos[jt], bias_full)

    # ---- layernorm helpers ---------------------------------------------
    def ln_tok_to_fm(src_dram, g_vec, b_vec, want_raw=False):
        fm_tiles = [fm_ln.tile([P, LQ], F32R, tag="lnstream", name=f"lnfm{c}") for c in range(ND)]
        raw_tiles = (
            [fm_raw.tile([P, LQ], F32R, tag="qraw", name=f"qraw{c}") for c in range(ND)]
            if want_raw else None
        )
        for it in range(NT):
            xt = act.tile([P, D], F32R, tag="xt_in")
            nc.sync.dma_start(xt[:], src_dram[it * P : (it + 1) * P, :])
            stats = sm.tile([P, nc.vector.BN_STATS_DIM], F32, tag="bnst", bufs=2)
            nc.vector.bn_stats(stats[:], xt[:].bitcast(F32))
            mv = sm.tile([P, nc.vector.BN_AGGR_DIM], F32, tag="bnmv", bufs=2)
            nc.vector.bn_aggr(mv[:], stats[:])
            sd = sm.tile([P, 1], F32, tag="bnsd", bufs=2)
            nc.scalar.activation(sd[:], mv[:, 1:2], AF.Sqrt, bias=eps_t[:])
            nc.vector.reciprocal(sd[:], sd[:])
            xn = act.tile([P, D], F32R, tag="xt_n")
            nc.vector.tensor_scalar(
                xn[:], xt[:], mv[:, 0:1], sd[:], OP.subtract, OP.mult
            )
            for c in range(ND):
                tp = ps_t.tile([P, P], F32R, tag="tps")
                nc.tensor.transpose(tp[:], xn[:, c * P : (c + 1) * P], identity[:])
                nc.vector.tensor_scalar(
                    fm_tiles[c][:, it * P : (it + 1) * P], tp[:],
                    g_vec[:, c : c + 1], b_vec[:, c : c + 1], OP.mult, OP.add,
                )
                if raw_tiles is not None:
                    tpr = ps_t.tile([P, P], F32R, tag="tps")
                    nc.tensor.transpose(tpr[:], xt[:, c * P : (c + 1) * P], identity[:])
                    nc.vector.tensor_copy(raw_tiles[c][:, it * P : (it + 1) * P], tpr[:])
        return fm_tiles, raw_tiles

    def ln_fm(x_tiles, g_vec, b_vec):
        """LayerNorm over the partition (feature) direction of feature-major
        tiles, via ones-matmul sums and a PE broadcast."""
        s_ps = ps_small.tile([1, LQ], F32, tag="small")
        s2_ps = ps_small.tile([1, LQ], F32, tag="small")
        for c in range(ND):
            nc.tensor.matmul(
                s_ps[:], onescol[:], x_tiles[c][:],
                start=(c == 0), stop=(c == ND - 1),
            )
        sqs = []
        for c in range(ND):
            sq = hid.tile([P, LQ], F32R, tag="hidden", name=f"sq{c}")
            nc.scalar.activation(sq[:], x_tiles[c][:], AF.Square)
            sqs.append(sq)
        for c in range(ND):
            nc.tensor.matmul(
                s2_ps[:], onescol[:], sqs[c][:],
                start=(c == 0), stop=(c == ND - 1),
            )
        mu = sm.tile([1, LQ], F32R, tag="lnmu")
        nc.vector.tensor_scalar(mu[:], s_ps[:], 1.0 / D, None, OP.mult)
        var = sm.tile([1, LQ], F32, tag="lnvar")
        nc.vector.tensor_scalar(var[:], s2_ps[:], 1.0 / D, None, OP.mult)
        m2 = sm.tile([1, LQ], F32, tag="lnm2")
        nc.vector.tensor_tensor(m2[:], mu[:].bitcast(F32), mu[:].bitcast(F32), OP.mult)
        nc.vector.tensor_tensor(var[:], var[:], m2[:], OP.subtract)
        sd = sm.tile([1, LQ], F32R, tag="lnsd")
        nc.scalar.activation(sd[:], var[:], AF.Sqrt, bias=eps_t[0:1, 0:1])
        nc.vector.reciprocal(sd[:], sd[:])
        mub = ps_small.tile([P, LQ], F32, tag="small")
        nc.tensor.matmul(mub[:], ones1[:], mu[:], start=True, stop=True)
        sdb = ps_small.tile([P, LQ], F32, tag="small")
        nc.tensor.matmul(sdb[:], ones1[:], sd[:], start=True, stop=True)
        outs = []
        for c in range(ND):
            t1 = hid.tile([P, LQ], F32, tag="hidden", name=f"lt{c}")
            nc.vector.tensor_tensor(t1[:], x_tiles[c][:].bitcast(F32), mub[:], OP.subtract)
            nc.vector.tensor_tensor(t1[:], t1[:], sdb[:], OP.mult)
            o = fm_ln.tile([P, LQ], F32R, tag="lnstream", name=f"lno{c}")
            nc.vector.tensor_scalar(
                o[:], t1[:], g_vec[:, c : c + 1], b_vec[:, c : c + 1],
                OP.mult, OP.add,
            )
            outs.append(o)
        return outs

    def head_slice(tiles, h):
        return tiles[h // 2][64 * (h % 2) : 64 * (h % 2) + 64, :]

    # ================= block0 =================
    lnq, q_fm = ln_tok_to_fm(q_in, vecs["ln0_g"], vecs["ln0_b"], want_raw=True)
    x_tiles = dense_wide(lnq, "b0_W", vecs["b0_b"], "x")

    qp = dense_narrow(x_tiles, "q0_W", HID, vecs["q0_b"], attA, "attA")
    kep = dense_narrow(x_tiles, "ke0_W", HID, vecs["ke0_b"], attA, "attA")

    kv_tok = []

    def kv_post(jt, psv, bias_full):
        t = kvA.tile([P, H * 65], F32R, tag="kvA")
        nc.vector.tensor_tensor(t[:, 0:D], psv[:], bias_full[:], OP.add)
        kv_tok.append(t)

    dense_tok(x_tiles, "kv0_W", kv0_b_full, kv_post)

    # ---- rel tables
    ep_fm = []
    krk = [w_ktile("kr0_W", k, D, wsm, "w512") for k in range(ND)]
    for m in range(ND):
        pse = ps.tile([P, LQ], F32, tag="mm")
        for k in range(ND):
            nc.tensor.matmul(
                pse[:], krk[k][:, m * P : (m + 1) * P], encR_sb[:, k, :],
                start=(k == 0), stop=(k == ND - 1),
            )
        o = resid.tile([P, LQ], F32R, tag="resid", name=f"ep{m}")
        nc.vector.tensor_scalar(o[:], pse[:], vecs["kr0_b"][:, m : m + 1], None, OP.add)
        ep_fm.append(o)

    ab1k = [w_ktile("ab1_W", k, H, wsm, "w8") for k in range(ND)]
    gam_ps = ps_small.tile([H, LQ], F32, tag="small")
    for k in range(ND):
        nc.tensor.matmul(
            gam_ps[:], ab1k[k][:], ep_fm[k][:], start=(k == 0), stop=(k == ND - 1)
        )
    gam = sm.tile([H, LQ], F32R, tag="gam")
    nc.vector.tensor_scalar(gam[:], gam_ps[:], vecs["ab1_b"][0:H, :], None, OP.add)

    ab0k = [w_ktile("ab0_W", k, H, wsm, "w8") for k in range(ND)]
    bke_ps = ps_small.tile([H, LQ], F32, tag="small")
    for k in range(ND):
        nc.tensor.matmul(
            bke_ps[:], ab0k[k][:], kep[k][:], start=(k == 0), stop=(k == ND - 1)
        )
    bke = sm.tile([H, LQ], F32R, tag="bke")
    nc.vector.tensor_scalar(bke[:], bke_ps[:], vecs["ab0_b"][0:H, :], None, OP.add)

    gam_all = sm.tile([1, H * LQ], F32R, tag="gamall")
    bkem_all = sm.tile([1, H * LQ], F32R, tag="bkemall")
    for h in range(H):
        nc.sync.dma_start(gam_all[:, h * LQ : (h + 1) * LQ], gam[h : h + 1, :])
        nc.sync.dma_start(bkem_all[:, h * LQ : (h + 1) * LQ], bke[h : h + 1, :])
        nc.vector.tensor_tensor(
            bkem_all[:, h * LQ : (h + 1) * LQ],
            bkem_all[:, h * LQ : (h + 1) * LQ], qmadd_sb[:], OP.add,
        )
    gam_rows = [gam_all[:, h * LQ : (h + 1) * LQ] for h in range(H)]
    bkem_rows = [bkem_all[:, h * LQ : (h + 1) * LQ] for h in range(H)]

    # ================= attention 0 =================
    # Phase A: all rel-position band tiles -> DRAM (skew write). Reads in
    # phase B then never stall on the HBM round trip.
    for h in range(H):
        qh = head_slice(qp, h)
        eph = head_slice(ep_fm, h)
        gb_ps = ps.tile([P, LQ], F32, tag="mm", name="gb_ps")
        nc.tensor.matmul(gb_ps[:], ones1[:], gam_rows[h], start=True, stop=True)
        gamb = soft.tile([P, LQ], F32, tag="gamb", name="gamb")
        nc.scalar.activation(gamb[:], gb_ps[:], AF.Copy)
        for it in range(NT):
            psb = ps.tile([P, LQ], F32, tag="mm")
            nc.tensor.matmul(
                psb[:], qh[:, it * P : (it + 1) * P], eph[:],
                start=True, stop=True,
            )
            band_w = soft.tile(
                [P, G_WPITCH], mybir.dt.bfloat16, tag="bandw", name="band_w"
            )
            nc.vector.tensor_tensor(band_w[:, 0:LQ], psb[:], gamb[:], OP.add)
            nc.vector.memset(band_w[:, LQ : LQ + 1], 0.0)
            gw = bass.AP(
                tensor=g_scr[h][it], offset=0, ap=[[G_WPITCH, P], [1, G_WPITCH]]
            )
            nc.sync.dma_start(gw, band_w[:])

    # ================= block2 value path (independent of queries) ========
    lnv, _ = ln_tok_to_fm(v_in, vecs["ln2_g"], vecs["ln2_b"])
    h2 = dense_wide(lnv, "b2_W1", vecs["b2_b1"], "h2")

    # Phase B: scores + softmax + PV, head pairs interleaved so the K=64
    # matmuls land on disjoint PE row groups and run concurrently.
    att_q1 = [resid.tile([P, LQ], F32R, tag="resid", name=f"q1_{c}") for c in range(ND)]
    for hp in range(H // 2):
        pair = (2 * hp, 2 * hp + 1)
        bkebs = {}
        for h in pair:
            bk_ps = ps.tile([P, LQ], F32, tag="mm", name="bk_ps")
            nc.tensor.matmul(bk_ps[:], ones1[:], bkem_rows[h], start=True, stop=True)
            bkeb = soft.tile([P, LQ], F32, tag=f"bkeb{h % 2}", name="bkeb")
            nc.scalar.activation(bkeb[:], bk_ps[:], AF.Copy)
            bkebs[h] = bkeb
        pn = {h: [None] * NT for h in pair}
        for it in range(NT):
            for h in pair:
                qh = head_slice(qp, h)
                keh = head_slice(kep, h)
                band = soft.tile(
                    [P, LQ], mybir.dt.bfloat16, tag=f"band{h % 2}", name="band",
                    bufs=2,
                )
                gr_ap = bass.AP(
                    tensor=g_scr[h][it], offset=(LQ - 1) - it * P,
                    ap=[[LQ, P], [1, LQ]],
                )
                nc.sync.dma_start(band[:], gr_ap)
                pss = ps.tile([P, LQ], F32, tag="mm")
                nc.tensor.matmul(
                    pss[:], qh[:, it * P : (it + 1) * P], keh[:],
                    start=True, stop=True,
                )
                s2 = soft.tile([P, LQ], F32, tag="s2", name="s2", bufs=3)
                nc.vector.tensor_tensor(s2[:], pss[:], band[:], OP.add)
                nc.vector.tensor_tensor(s2[:], s2[:], bkebs[h][:], OP.add)
                nc.gpsimd.affine_select(
                    out=s2[:], in_=s2[:], compare_op=OP.is_ge, fill=-1e30,
                    base=it * P, pattern=[[-1, LQ]], channel_multiplier=1,
                )
                z = sm.tile([P, 1], F32, tag="z0", bufs=3)
                p_t = soft.tile(
                    [P, LQ], F32R, tag=f"p{it}{h % 2}", name="p_t", bufs=1
                )
                nc.scalar.activation(
                    p_t[:], s2[:], AF.Exp, scale=SCALE, accum_out=z[:]
                )
                nc.vector.reciprocal(z[:], z[:])
                nc.vector.tensor_scalar(p_t[:], p_t[:], z[:], None, OP.mult)
                pn[h][it] = p_t
        for h in pair:
            pt_tiles = [
                pt_pool.tile([P, LQ], F32R, tag=f"pt{jt}", name=f"pt{jt}")
                for jt in range(NT)
            ]
            for it in range(NT):
                for jt in range(it + 1):
                    tp = ps_t.tile([P, P], F32R, tag="tps")
                    nc.tensor.transpose(
                        tp[:], pn[h][it][:, jt * P : (jt + 1) * P], identity[:]
                    )
                    nc.vector.tensor_copy(
                        pt_tiles[jt][:, it * P : (it + 1) * P], tp[:]
                    )
            for it in range(NT):
                pso = ps_small.tile([64, P], F32, tag="small")
                for jt in range(it + 1):
                    nc.tensor.matmul(
                        pso[:], kv_tok[jt][:, 64 * h : 64 * h + 64],
                        pt_tiles[jt][:, it * P : (it + 1) * P],
                        start=(jt == 0), stop=(jt == it),
                    )
                dst = att_q1[h // 2][
                    64 * (h % 2) : 64 * (h % 2) + 64, it * P : (it + 1) * P
                ]
                src_q = q_fm[h // 2][
                    64 * (h % 2) : 64 * (h % 2) + 64, it * P : (it + 1) * P
                ]
                nc.vector.tensor_tensor(dst, pso[:], src_q, OP.add)

    # ---- k1 / v1 (consumed by attention 1)
    k1_fm = dense_narrow(h2, "b2_W2k", HID, vecs["b2_b2k"], attA, "attA")
    v1i = []

    def v1_post(jt, psv, bias_full):
        t = kvA.tile([P, H * 65], F32R, tag="kvA")
        tv = t[:].rearrange("p (h x) -> p h x", h=H)
        nc.vector.tensor_tensor(
            tv[:, :, 0:64], psv[:].rearrange("p (h d) -> p h d", h=H),
            bias_full[:].rearrange("p (h d) -> p h d", h=H), OP.add,
        )
        nc.vector.tensor_copy(
            tv[:, :, 64:65], ones_col8[:].rearrange("p (h x) -> p h x", x=1)
        )
        v1i.append(t)

    dense_tok(h2, "b2_W2v", b2_b2v_full, v1_post)

    # ================= block1 -> y =================
    lnq1 = ln_fm(att_q1, vecs["ln1_g"], vecs["ln1_b"])
    h1 = dense_wide(lnq1, "b1_W1", vecs["b1_b1"], "h1")
    y_fm = dense_narrow(h1, "b1_W2", HID, vecs["b1_b2"], attA, "attA")

    # ================= attention 1 (transposed) =================
    att_q2 = [resid.tile([P, LQ], F32R, tag="resid", name=f"q2_{c}") for c in range(ND)]
    for h in range(H):
        yh = head_slice(y_fm, h)
        k1h = head_slice(k1_fm, h)
        p1t_tiles = []
        for jt in range(NT):
            pss = ps.tile([P, LQ], F32, tag="mm")
            nc.tensor.matmul(
                pss[:], k1h[:, jt * P : (jt + 1) * P], yh[:], start=True, stop=True
            )
            p1 = soft.tile([P, LQ], F32R, tag=f"p{jt}0", name="p1", bufs=1)
            nc.scalar.activation(
                p1[:], pss[:], AF.Exp, scale=SCALE, bias=vmadd_sb[:, jt : jt + 1]
            )
            p1t_tiles.append(p1)
        pso = ps.tile([65, LQ], F32, tag="mm")
        for jt in range(NT):
            nc.tensor.matmul(
                pso[:], v1i[jt][:, 65 * h : 65 * h + 65], p1t_tiles[jt][:],
                start=(jt == 0), stop=(jt == NT - 1),
            )
        rz = sm.tile([1, LQ], F32R, tag="rz1", bufs=2)
        nc.vector.reciprocal(rz[:], pso[64:65, :])
        psb = ps_small.tile([64, LQ], F32, tag="small")
        nc.tensor.matmul(psb[:], ones1[:, 0:64], rz[:], start=True, stop=True)
        o1 = soft.tile([P, LQ], F32, tag="o1")
        o1s = o1[64 * (h % 2) : 64 * (h % 2) + 64, :]
        nc.scalar.activation(o1s, pso[0:64, :], AF.Copy)
        nc.vector.tensor_tensor(o1s, o1s, psb[:], OP.mult)
        dst = att_q2[h // 2][64 * (h % 2) : 64 * (h % 2) + 64, :]
        src_q = att_q1[h // 2][64 * (h % 2) : 64 * (h % 2) + 64, :]
        nc.vector.tensor_tensor(dst, o1s, src_q, OP.add)

    # ================= block3 residual FFN =================
    lnq3 = ln_fm(att_q2, vecs["ln3_g"], vecs["ln3_b"])
    h3 = dense_wide(lnq3, "b3_W1", vecs["b3_b1"], "h3")

    def b3_evict(o, pso, m):
        nc.vector.scalar_tensor_tensor(
            o[:], pso[:], vecs["b3_b2"][:, m : m + 1], att_q2[m][:],
            OP.add, OP.add,
        )

    q3 = dense_narrow(h3, "b3_W2", HID, None, resid, "resid", evict=b3_evict)

    # ---- back to token-major, store
    for it in range(NT):
        ot = act.tile([P, D], F32, tag="out_tok")
        for c in range(ND):
            tp = ps_t.tile([P, P], F32R, tag="tps")
            nc.tensor.transpose(tp[:], q3[c][:, it * P : (it + 1) * P], identity[:])
            nc.vector.tensor_copy(ot[:, c * P : (c + 1) * P], tp[:].bitcast(F32))
        nc.sync.dma_start(out[it * P : (it + 1) * P, :], ot[:])

    ctx.close()


_NC = None


def _get_nc():
    global _NC
    if _NC is None:
        _NC = build_nc()
    return _NC


def _build_in_maps(queries, values, queries_mask, values_mask, params):
    p = params
    f32 = np.float32

    def fmvec(v, n_tiles):
        return np.ascontiguousarray(np.asarray(v, f32).reshape(n_tiles, P).T)

    w2 = np.asarray(p["b2_W2"], f32).reshape(HID, H, 2, DIM)
    b2b = np.asarray(p["b2_b2"], f32).reshape(H, 2, DIM)

    shared = {
        "b0_W": np.asarray(p["b0_W"], f32), "q0_W": np.asarray(p["q0_W"], f32),
        "ke0_W": np.asarray(p["ke0_W"], f32), "kv0_W": np.asarray(p["kv0_W"], f32),
        "kr0_W": np.asarray(p["kr0_W"], f32), "ab0_W": np.asarray(p["ab0_W"], f32),
        "ab1_W": np.asarray(p["ab1_W"], f32),
        "b1_W1": np.asarray(p["b1_W1"], f32), "b1_W2": np.asarray(p["b1_W2"], f32),
        "b2_W1": np.asarray(p["b2_W1"], f32),
        "b2_W2k": np.ascontiguousarray(w2[:, :, 0, :].reshape(HID, D)),
        "b2_W2v": np.ascontiguousarray(w2[:, :, 1, :].reshape(HID, D)),
        "b3_W1": np.asarray(p["b3_W1"], f32), "b3_W2": np.asarray(p["b3_W2"], f32),
        "encR": np.ascontiguousarray(_sinusoid_ext_rev().T),
        "ones_r": np.ones((1, P), f32),
    }
    vpack = np.zeros((P, VPACK_COLS), f32)
    vals = {
        "b0_b": fmvec(p["b0_b"], NH), "q0_b": fmvec(p["q0_b"], ND),
        "ke0_b": fmvec(p["ke0_b"], ND), "kr0_b": fmvec(p["kr0_b"], ND),
        "b1_b1": fmvec(p["b1_b1"], NH), "b1_b2": fmvec(p["b1_b2"], ND),
        "b2_b1": fmvec(p["b2_b1"], NH),
        "b2_b2k": fmvec(b2b[:, 0, :].reshape(D), ND),
        "b3_b1": fmvec(p["b3_b1"], NH), "b3_b2": fmvec(p["b3_b2"], ND),
        "ln0_g": fmvec(p["ln0_g"], ND), "ln0_b": fmvec(p["ln0_b"], ND),
        "ln1_g": fmvec(p["ln1_g"], ND), "ln1_b": fmvec(p["ln1_b"], ND),
        "ln2_g": fmvec(p["ln2_g"], ND), "ln2_b": fmvec(p["ln2_b"], ND),
        "ln3_g": fmvec(p["ln3_g"], ND), "ln3_b": fmvec(p["ln3_b"], ND),
        "ab0_b": np.pad(
            np.asarray(p["ab0_b"], f32).reshape(H, 1), ((0, P - H), (0, 0))
        ),
        "ab1_b": np.pad(
            np.asarray(p["ab1_b"], f32).reshape(H, 1), ((0, P - H), (0, 0))
        ),
        "epsc": np.full((P, 1), 1e-3, f32),
        "kv0_b_full": np.tile(np.asarray(p["kv0_b"], f32).reshape(1, D), (P, 1)),
        "b2_b2v_full": np.tile(b2b[:, 1, :].reshape(1, D), (P, 1)),
    }
    for nm, (c, n) in _VP.items():
        vpack[:, c : c + n] = vals[nm]
    shared["vpack"] = vpack
    rpack = np.zeros((P, RPACK_COLS), f32)
    rpack[:, _RP["idn"][0] : _RP["idn"][0] + P] = np.eye(P, dtype=f32)
    rpack[:, _RP["ones_r"][0]] = 1.0
    rpack[:, _RP["ones_c"][0]] = 1.0
    rpack[:, _RP["ones8"][0] : _RP["ones8"][0] + H] = 1.0
    shared["rpack"] = rpack

    qm = np.asarray(queries_mask, f32)
    vm = np.asarray(values_mask, f32)
    in_maps = []
    for c in range(B):
        m = dict(shared)
        m["q_in"] = np.ascontiguousarray(np.asarray(queries[c], f32))
        m["v_in"] = np.ascontiguousarray(np.asarray(values[c], f32))
        m["qmadd"] = ((qm[c] - 1.0) * 8e9).reshape(1, LQ).astype(f32)
        m["vmadd"] = np.ascontiguousarray(
            ((vm[c] - 1.0) * 1e9).reshape(NT, P).T
        ).astype(f32)
        in_maps.append(m)
    return in_maps


def kernel(queries, values, queries_mask, values_mask, params):
    in_maps = _build_in_maps(queries, values, queries_mask, values_mask, params)
    nc = _get_nc()
    res = run_bass_kernel_spmd(nc, in_maps, core_ids=list(range(B)))
    return np.stack([res.results[c]["out"] for c in range(B)], axis=0).astype(
        np.float32
    )


def kernel_profiled(queries, values, queries_mask, values_mask, params,
                    tmpdir=None):
    """Same as kernel() but with NTFF tracing; returns (output, results)."""
    import kernel as _self  # works both as module and as __main__ helper

    in_maps = _build_in_maps(queries, values, queries_mask, values_mask, params)
    nc = _get_nc()
    res = run_bass_kernel_spmd(
        nc, in_maps, core_ids=list(range(B)), trace=True, tmpdir=tmpdir
    )
    out = np.stack([res.results[c]["out"] for c in range(B)], axis=0).astype(
        np.float32
    )
    return out, res


# revision 23
# speedup vs baseline: 1.3930x; 1.0708x over previous
"""Trainium2 Bass kernel for nn_DecoderWithRelativePositionalAttentionLayer.

Sharding: pure data-parallel over batch (B=8 -> 8 NeuronCores, one batch
element per core, identical SPMD program, no collectives).

Key algorithmic move: the reference materializes rel = enc[ridx] as a
[Lq, Lq, D] tensor and pushes it through a [D, D] dense (38.7 GMAC); since
ridx = clip(i-j, -R, R) takes only 201 values and the causal mask kills
j > i, we project a [384, D] extended/reversed table once and realize the
per-(i, j) band with a DRAM "skew" bounce: rows written at pitch 385 and
read back at pitch 384 / offset 383 land row i's entry t at column
j = i - 383 + t.

Layouts: activations are feature-major [F, T] so weight matmuls need no
transposes (lhsT = native [in, out] weight k-tiles, rhs = activations).
Attention-0 scores run in [i, j] (softmax along the free dim, causal via
gpsimd affine_select, band added pre-exp); the probability tiles are
PE-transposed for the PV matmul. Attention-1 has no positional term and is
computed directly transposed [j, i], with the softmax normalizer taken from
an extra ones-column interleaved into the value matrix.

All matmul operands are float32r (TF32-like, full-rate on the PE array;
measured ~2.5e-4 per-matmul relative error on HW).
"""

import sys

sys.path.insert(0, "/opt/trn_rl_repo")

import contextlib
import numpy as np
import concourse.bass as bass
import concourse.mybir as mybir
import concourse.tile as tile
from concourse.bass_utils import run_bass_kernel_spmd
import bass_rust

F32 = mybir.dt.float32
F32R = mybir.dt.float32r
BF16 = mybir.dt.bfloat16
AF = mybir.ActivationFunctionType
OP = mybir.AluOpType
AX = mybir.AxisListType

B, LQ, LK, D, H, HID = 8, 384, 384, 512, 8, 2048
DIM = D // H
REL = 100
P = 128
NT = LQ // P   # 3 token tiles
ND = D // P    # 4 feature tiles
NH = HID // P  # 16 hidden tiles
SCALE = float(1.0 / np.sqrt(np.float32(DIM)))
G_WPITCH = LQ + 1           # 385: write pitch of the skew scratch
G_STRIDE = G_WPITCH * LQ    # per-head segment

# packed-constant column layout: name -> (col, ncols)
_VP = {}
_c = 0
for _nm, _n in [
    ("b0_b", NH), ("q0_b", ND), ("ke0_b", ND), ("kr0_b", ND),
    ("b1_b1", NH), ("b1_b2", ND), ("b2_b1", NH), ("b2_b2k", ND),
    ("b3_b1", NH), ("b3_b2", ND),
    ("ln0_g", ND), ("ln0_b", ND), ("ln1_g", ND), ("ln1_b", ND),
    ("ln2_g", ND), ("ln2_b", ND), ("ln3_g", ND), ("ln3_b", ND),
    ("ab0_b", 1), ("ab1_b", 1), ("epsc", 1),
    ("kv0_b_full", D), ("b2_b2v_full", D),
]:
    _VP[_nm] = (_c, _n)
    _c += _n
VPACK_COLS = _c
_RP = {}
_c = 0
for _nm, _n in [("idn", P), ("ones_r", 1), ("ones_c", 1), ("ones8", H)]:
    _RP[_nm] = (_c, _n)
    _c += _n
RPACK_COLS = _c



def _split_multiwait_instructions(nc):
    """This toolchain's walrus accepts at most ONE sync wait per
    instruction, but Tile's tail drain aggregates several. Move extras onto
    same-engine nops placed immediately before the offending instruction."""
    counter = [0]

    def fresh_nop(engine, wait):
        counter[0] += 1
        nop = bass_rust.InstNoOp(name=f"WSPLIT-{counter[0]}", ins=[], outs=[])
        nop.engine = engine
        nop.sync_info = bass_rust.SyncInfo(on_wait=[wait], on_update=[])
        return nop

    for fn in nc.m.functions:
        for bb in fn.blocks:
            out = []
            changed = False
            for inst in bb.instructions:
                si = inst.sync_info
                if si is not None and len(si.on_wait) > 1:
                    waits = list(si.on_wait)
                    for w in waits[:-1]:
                        out.append(fresh_nop(inst.engine, w))
                    inst.sync_info = bass_rust.SyncInfo(
                        on_wait=[waits[-1]], on_update=list(si.on_update)
                    )
                    changed = True
                out.append(inst)
            if changed:
                bb.instructions = out


def _sinusoid_ext_rev():
    """encER[t] = enc[min(383 - t, REL) + REL]  -> [384, 512]."""
    pos = np.arange(-REL, REL + 1, dtype=np.float32)[:, None]
    i = np.arange(D // 2, dtype=np.float32)[None, :]
    ang = pos / np.power(np.float32(10000.0), 2.0 * i / np.float32(D))
    enc = np.concatenate([np.sin(ang), np.cos(ang)], axis=-1).astype(np.float32)
    o = (LQ - 1) - np.arange(LQ)
    return enc[np.minimum(o, REL) + REL]


def build_nc():
    nc = bass.Bass()

    def pin(name, shape, dt=F32R):
        return nc.declare_dram_parameter(name, list(shape), dt, isOutput=False)

    q_in = pin("q_in", [LQ, D])
    v_in = pin("v_in", [LK, D])
    qmadd = pin("qmadd", [1, LQ], F32)   # (qm-1)*8e9, pre-scale additive
    vmadd = pin("vmadd", [P, NT], F32)   # (vm-1)*1e9, post-scale additive

    w = {}
    for nm, shp in [
        ("b0_W", [D, HID]), ("q0_W", [HID, D]), ("ke0_W", [HID, D]),
        ("kv0_W", [HID, D]), ("kr0_W", [D, D]), ("ab0_W", [D, H]),
        ("ab1_W", [D, H]), ("b1_W1", [D, HID]), ("b1_W2", [HID, D]),
        ("b2_W1", [D, HID]), ("b2_W2k", [HID, D]), ("b2_W2v", [HID, D]),
        ("b3_W1", [D, HID]), ("b3_W2", [HID, D]),
    ]:
        w[nm] = pin(nm, shp, BF16)
    # all small f32 vectors packed into one [128, VPACK] tensor (host
    # places each at a known column), all f32r constants into another
    w["vpack"] = pin("vpack", [P, VPACK_COLS], F32)
    w["rpack"] = pin("rpack", [P, RPACK_COLS], F32R)

    encR = pin("encR", [D, LQ], BF16)
    ones_r = pin("ones_r", [1, P])
    idnb = pin("idnb", [P, P], BF16)

    out = nc.declare_dram_parameter("out", [LQ, D], F32, isOutput=True)
    g_scr = [
        [
            nc.dram_tensor(f"g_scratch{h}_{it}", [50048], mybir.dt.bfloat16)
            for it in range(NT)
        ]
        for h in range(H)
    ]

    with tile.TileContext(nc) as tc, nc.allow_low_precision(
        reason="float32r dataflow is intentional (TF32-like matmul operands)"
    ):
        _emit(nc, tc, q_in, v_in, qmadd, vmadd, w, encR, ones_r, idnb, out, g_scr)
    _split_multiwait_instructions(nc)
    return nc


def _emit(nc, tc, q_in, v_in, qmadd, vmadd, w, encR, ones_r, idnb, out, g_scr):
    ctx = contextlib.ExitStack()

    def pool(name, bufs, **kw):
        return ctx.enter_context(tc.tile_pool(name=name, bufs=bufs, **kw))

    const = pool("const", 1)
    wbig = pool("wbig", 8)       # [128, 1024] half k-tiles
    wsm = pool("wsm", 6)         # [128, <=512] k-tiles, streaming (k-outer)
    act = pool("act", 2)         # token-major staging
    fm_ln = pool("fm_ln", 4)     # LN output streams (ln0->ln1->ln2->ln3)
    fm_raw = pool("fm_raw", 4)   # raw queries, feature-major
    attA = pool("attA", 8)       # qp, kep -> y, k1
    kvA = pool("kvA", 3)         # kv_tok -> v1i
    resid = pool("resid", 8)     # ep -> q1 -> q2 -> q3
    hid = pool("hid", 16)        # hidden tiles + LN scratch
    soft = pool("soft", 2)
    pt_pool = pool("pt", 1)
    sm = pool("sm", 1)
    ps = pool("ps", 4, space="PSUM")
    ps_t = pool("ps_t", 2, space="PSUM")
    ps_small = pool("ps_small", 2, space="PSUM")

    # ---- constants (two packed DMAs + encR + masks)
    vpack_sb = const.tile([P, VPACK_COLS], F32)
    nc.sync.dma_start(vpack_sb[:], w["vpack"][:])
    rpack_sb = const.tile([P, RPACK_COLS], F32R)
    nc.sync.dma_start(rpack_sb[:], w["rpack"][:])

    def vp(name):
        c, n = _VP[name]
        return vpack_sb[:, c : c + n]

    vecs = {nm: vp(nm) for nm in _VP}
    identity = rpack_sb[:, _RP["idn"][0] : _RP["idn"][0] + P]
    onescol = rpack_sb[:, _RP["ones_c"][0] : _RP["ones_c"][0] + 1]
    ones_col8 = rpack_sb[:, _RP["ones8"][0] : _RP["ones8"][0] + H]
    eps_t = vp("epsc")
    kv0_b_full = vp("kv0_b_full")
    b2_b2v_full = vp("b2_b2v_full")
    identb = const.tile([P, P], BF16)
    nc.sync.dma_start(identb[:], idnb[:])
    encR_sb = const.tile([P, ND, LQ], BF16)
    nc.sync.dma_start(encR_sb[:], encR[:].rearrange("(k p) t -> p k t", p=P))
    qmadd_sb = const.tile([1, LQ], F32)
    nc.sync.dma_start(qmadd_sb[:], qmadd[:])
    vmadd_sb = const.tile([P, NT], F32)
    nc.sync.dma_start(vmadd_sb[:], vmadd[:])
    ones1 = const.tile([1, P], F32R)
    nc.sync.dma_start(ones1[:], ones_r[:])

    def w_ktile(name, k, ncols, p, tag):
        t = p.tile([P, ncols], BF16, tag=tag, name=f"{name}k{k}")
        nc.sync.dma_start(t[:], w[name][k * P : (k + 1) * P, :])
        return t

    # ---- dense helpers -------------------------------------------------
    def dense_wide(x_tiles, wname, bias_vec, out_tag):
        """[D -> HID] with relu. m-outer over 16 output tiles. Weights load
        as [128, 1024] half-m k-tiles so the second half (and the next
        block's first half) can prefetch while the first computes."""
        halves = [
            [None] * ND,
            [None] * ND,
        ]
        for half in range(2):
            for k in range(ND):
                t = wbig.tile([P, HID // 2], BF16, tag="w1024", name=f"{wname}h{half}k{k}")
                nc.sync.dma_start(
                    t[:],
                    w[wname][k * P : (k + 1) * P, half * (HID // 2) : (half + 1) * (HID // 2)],
                )
                halves[half][k] = t
        outs = []
        for m in range(NH):
            half, mloc = divmod(m, NH // 2)
            pso = ps.tile([P, LQ], F32, tag="mm")
            for k in range(ND):
                nc.tensor.matmul(
                    pso[:], halves[half][k][:, mloc * P : (mloc + 1) * P], x_tiles[k][:],
                    start=(k == 0), stop=(k == ND - 1),
                )
            o = hid.tile([P, LQ], BF16, tag="hidden", name=f"hw{m}")
            nc.scalar.activation(o[:], pso[:], AF.Relu, bias=bias_vec[:, m : m + 1])
            outs.append(o)
        return outs

    def dense_narrow(x_tiles, wname, n_in, bias_vec, out_pool, out_tag,
                     relu=False, evict=None, out_dt=BF16):
        """[n_in -> 512] feature-major. k-outer so weight k-tiles stream
        with bufs=4; the 4 output psums accumulate concurrently."""
        nk = n_in // P
        psos = [ps.tile([P, LQ], F32, tag="mm", name=f"dnps{m}") for m in range(ND)]
        for k in range(nk):
            wk = w_ktile(wname, k, D, wsm, "w512")
            for m in range(ND):
                nc.tensor.matmul(
                    psos[m][:], wk[:, m * P : (m + 1) * P], x_tiles[k][:],
                    start=(k == 0), stop=(k == nk - 1),
                )
        outs = []
        for m in range(ND):
            o = out_pool.tile([P, LQ], out_dt, tag=out_tag, name=f"dn{m}")
            if evict is not None:
                evict(o, psos[m], m)
            elif relu:
                nc.scalar.activation(
                    o[:], psos[m][:], AF.Relu, bias=bias_vec[:, m : m + 1]
                )
            else:
                nc.vector.tensor_scalar(
                    o[:], psos[m][:], bias_vec[:, m : m + 1], None, OP.add
                )
            outs.append(o)
        return outs

    def dense_tok(x_tiles, wname, bias_full, post):
        """[HID -> 512] token-major out: for each token tile jt a [128, 512]
        psum accumulates x[k][:, jt] @ W[k]; bias (a host-replicated full
        tile) is folded in by the consumer. post(jt, psum, bias) consumes."""
        psos = [ps.tile([P, D], F32, tag="mm", name=f"dtps{j}") for j in range(NT)]
        for k in range(NH):
            wk = w_ktile(wname, k, D, wsm, "w512")
            for jt in range(NT):
                nc.tensor.matmul(
                    psos[jt][:], x_tiles[k][:, jt * P : (jt + 1) * P], wk[:],
                    start=(k == 0), stop=(k == NH - 1),
                )
        for jt in range(NT):
            post(jt, psos[jt], bias_full)

    # ---- layernorm helpers ---------------------------------------------
    def ln_tok_to_fm(src_dram, g_vec, b_vec, want_raw=False):
        fm_tiles = [fm_ln.tile([P, LQ], BF16, tag="lnstream", name=f"lnfm{c}") for c in range(ND)]
        raw_tiles = (
            [fm_raw.tile([P, LQ], F32R, tag="qraw", name=f"qraw{c}") for c in range(ND)]
            if want_raw else None
        )
        for it in range(NT):
            xt = act.tile([P, D], F32R, tag="xt_in")
            nc.sync.dma_start(xt[:], src_dram[it * P : (it + 1) * P, :])
            stats = sm.tile([P, nc.vector.BN_STATS_DIM], F32, tag="bnst", bufs=2)
            nc.vector.bn_stats(stats[:], xt[:].bitcast(F32))
            mv = sm.tile([P, nc.vector.BN_AGGR_DIM], F32, tag="bnmv", bufs=2)
            nc.vector.bn_aggr(mv[:], stats[:])
            sd = sm.tile([P, 1], F32, tag="bnsd", bufs=2)
            nc.scalar.activation(sd[:], mv[:, 1:2], AF.Sqrt, bias=eps_t[:])
            nc.vector.reciprocal(sd[:], sd[:])
            xn = act.tile([P, D], F32R, tag="xt_n")
            nc.vector.tensor_scalar(
                xn[:], xt[:], mv[:, 0:1], sd[:], OP.subtract, OP.mult
            )
            for c in range(ND):
                tp = ps_t.tile([P, P], F32R, tag="tps")
                nc.tensor.transpose(tp[:], xn[:, c * P : (c + 1) * P], identity[:])
                nc.vector.tensor_scalar(
                    fm_tiles[c][:, it * P : (it + 1) * P], tp[:],
                    g_vec[:, c : c + 1], b_vec[:, c : c + 1], OP.mult, OP.add,
                )
                if raw_tiles is not None:
                    tpr = ps_t.tile([P, P], F32R, tag="tps")
                    nc.tensor.transpose(tpr[:], xt[:, c * P : (c + 1) * P], identity[:])
                    nc.vector.tensor_copy(raw_tiles[c][:, it * P : (it + 1) * P], tpr[:])
        return fm_tiles, raw_tiles

    def ln_fm(x_tiles, g_vec, b_vec):
        """LayerNorm over the partition (feature) direction of feature-major
        tiles, via ones-matmul sums and a PE broadcast."""
        s_ps = ps_small.tile([1, LQ], F32, tag="small")
        s2_ps = ps_small.tile([1, LQ], F32, tag="small")
        for c in range(ND):
            nc.tensor.matmul(
                s_ps[:], onescol[:], x_tiles[c][:],
                start=(c == 0), stop=(c == ND - 1),
            )
        sqs = []
        for c in range(ND):
            sq = hid.tile([P, LQ], F32R, tag="hidden", name=f"sq{c}")
            nc.scalar.activation(sq[:], x_tiles[c][:], AF.Square)
            sqs.append(sq)
        for c in range(ND):
            nc.tensor.matmul(
                s2_ps[:], onescol[:], sqs[c][:],
                start=(c == 0), stop=(c == ND - 1),
            )
        mu = sm.tile([1, LQ], F32R, tag="lnmu")
        nc.vector.tensor_scalar(mu[:], s_ps[:], 1.0 / D, None, OP.mult)
        var = sm.tile([1, LQ], F32, tag="lnvar")
        nc.vector.tensor_scalar(var[:], s2_ps[:], 1.0 / D, None, OP.mult)
        m2 = sm.tile([1, LQ], F32, tag="lnm2")
        nc.vector.tensor_tensor(m2[:], mu[:].bitcast(F32), mu[:].bitcast(F32), OP.mult)
        nc.vector.tensor_tensor(var[:], var[:], m2[:], OP.subtract)
        sd = sm.tile([1, LQ], F32R, tag="lnsd")
        nc.scalar.activation(sd[:], var[:], AF.Sqrt, bias=eps_t[0:1, 0:1])
        nc.vector.reciprocal(sd[:], sd[:])
        mub = ps_small.tile([P, LQ], F32, tag="small")
        nc.tensor.matmul(mub[:], ones1[:], mu[:], start=True, stop=True)
        sdb = ps_small.tile([P, LQ], F32, tag="small")
        nc.tensor.matmul(sdb[:], ones1[:], sd[:], start=True, stop=True)
        outs = []
        for c in range(ND):
            t1 = hid.tile([P, LQ], F32, tag="hidden", name=f"lt{c}")
            nc.vector.tensor_tensor(t1[:], x_tiles[c][:].bitcast(F32), mub[:], OP.subtract)
            nc.vector.tensor_tensor(t1[:], t1[:], sdb[:], OP.mult)
            o = fm_ln.tile([P, LQ], BF16, tag="lnstream", name=f"lno{c}")
            nc.vector.tensor_scalar(
                o[:], t1[:], g_vec[:, c : c + 1], b_vec[:, c : c + 1],
                OP.mult, OP.add,
            )
            outs.append(o)
        return outs

    def head_slice(tiles, h):
        return tiles[h // 2][64 * (h % 2) : 64 * (h % 2) + 64, :]

    # ================= block0 =================
    lnq, q_fm = ln_tok_to_fm(q_in, vecs["ln0_g"], vecs["ln0_b"], want_raw=True)
    x_tiles = dense_wide(lnq, "b0_W", vecs["b0_b"], "x")

    qp = dense_narrow(x_tiles, "q0_W", HID, vecs["q0_b"], attA, "attA")
    kep = dense_narrow(x_tiles, "ke0_W", HID, vecs["ke0_b"], attA, "attA")

    kv_tok = []

    def kv_post(jt, psv, bias_full):
        t = kvA.tile([P, H * 65], BF16, tag="kvA")
        nc.vector.tensor_tensor(t[:, 0:D], psv[:], bias_full[:], OP.add)
        kv_tok.append(t)

    dense_tok(x_tiles, "kv0_W", kv0_b_full, kv_post)

    # ---- rel tables
    ep_fm = []
    krk = [w_ktile("kr0_W", k, D, wsm, "w512") for k in range(ND)]
    for m in range(ND):
        pse = ps.tile([P, LQ], F32, tag="mm")
        for k in range(ND):
            nc.tensor.matmul(
                pse[:], krk[k][:, m * P : (m + 1) * P], encR_sb[:, k, :],
                start=(k == 0), stop=(k == ND - 1),
            )
        o = resid.tile([P, LQ], BF16, tag="resid", name=f"ep{m}")
        nc.vector.tensor_scalar(o[:], pse[:], vecs["kr0_b"][:, m : m + 1], None, OP.add)
        ep_fm.append(o)

    ab1k = [w_ktile("ab1_W", k, H, wsm, "w8") for k in range(ND)]
    gam_ps = ps_small.tile([H, LQ], F32, tag="small")
    for k in range(ND):
        nc.tensor.matmul(
            gam_ps[:], ab1k[k][:], ep_fm[k][:], start=(k == 0), stop=(k == ND - 1)
        )
    gam = sm.tile([H, LQ], F32R, tag="gam")
    nc.vector.tensor_scalar(gam[:], gam_ps[:], vecs["ab1_b"][0:H, :], None, OP.add)

    ab0k = [w_ktile("ab0_W", k, H, wsm, "w8") for k in range(ND)]
    bke_ps = ps_small.tile([H, LQ], F32, tag="small")
    for k in range(ND):
        nc.tensor.matmul(
            bke_ps[:], ab0k[k][:], kep[k][:], start=(k == 0), stop=(k == ND - 1)
        )
    bke = sm.tile([H, LQ], F32R, tag="bke")
    nc.vector.tensor_scalar(bke[:], bke_ps[:], vecs["ab0_b"][0:H, :], None, OP.add)

    gam_all = sm.tile([1, H * LQ], F32R, tag="gamall")
    bkem_all = sm.tile([1, H * LQ], F32R, tag="bkemall")
    for h in range(H):
        nc.sync.dma_start(gam_all[:, h * LQ : (h + 1) * LQ], gam[h : h + 1, :])
        nc.sync.dma_start(bkem_all[:, h * LQ : (h + 1) * LQ], bke[h : h + 1, :])
        nc.vector.tensor_tensor(
            bkem_all[:, h * LQ : (h + 1) * LQ],
            bkem_all[:, h * LQ : (h + 1) * LQ], qmadd_sb[:], OP.add,
        )
    gam_rows = [gam_all[:, h * LQ : (h + 1) * LQ] for h in range(H)]
    bkem_rows = [bkem_all[:, h * LQ : (h + 1) * LQ] for h in range(H)]

    # ================= attention 0 =================
    # Phase A: all rel-position band tiles -> DRAM (skew write). Reads in
    # phase B then never stall on the HBM round trip.
    for h in range(H):
        qh = head_slice(qp, h)
        eph = head_slice(ep_fm, h)
        gb_ps = ps.tile([P, LQ], F32, tag="mm", name="gb_ps")
        nc.tensor.matmul(gb_ps[:], ones1[:], gam_rows[h], start=True, stop=True)
        gamb = soft.tile([P, LQ], F32, tag="gamb", name="gamb")
        nc.scalar.activation(gamb[:], gb_ps[:], AF.Copy)
        for it in range(NT):
            psb = ps.tile([P, LQ], F32, tag="mm")
            nc.tensor.matmul(
                psb[:], qh[:, it * P : (it + 1) * P], eph[:],
                start=True, stop=True,
            )
            band_w = soft.tile(
                [P, G_WPITCH], mybir.dt.bfloat16, tag="bandw", name="band_w"
            )
            nc.vector.tensor_tensor(band_w[:, 0:LQ], psb[:], gamb[:], OP.add)
            nc.vector.memset(band_w[:, LQ : LQ + 1], 0.0)
            gw = bass.AP(
                tensor=g_scr[h][it], offset=0, ap=[[G_WPITCH, P], [1, G_WPITCH]]
            )
            nc.sync.dma_start(gw, band_w[:])

    # ================= block2 value path (independent of queries) ========
    lnv, _ = ln_tok_to_fm(v_in, vecs["ln2_g"], vecs["ln2_b"])
    h2 = dense_wide(lnv, "b2_W1", vecs["b2_b1"], "h2")

    # Phase B: scores + softmax + PV, head pairs interleaved so the K=64
    # matmuls land on disjoint PE row groups and run concurrently.
    att_q1 = [resid.tile([P, LQ], F32R, tag="resid", name=f"q1_{c}") for c in range(ND)]
    for hp in range(H // 2):
        pair = (2 * hp, 2 * hp + 1)
        bkebs = {}
        for h in pair:
            bk_ps = ps.tile([P, LQ], F32, tag="mm", name="bk_ps")
            nc.tensor.matmul(bk_ps[:], ones1[:], bkem_rows[h], start=True, stop=True)
            bkeb = soft.tile([P, LQ], F32, tag=f"bkeb{h % 2}", name="bkeb")
            nc.scalar.activation(bkeb[:], bk_ps[:], AF.Copy)
            bkebs[h] = bkeb
        pn = {h: [None] * NT for h in pair}
        for it in range(NT):
            for h in pair:
                qh = head_slice(qp, h)
                keh = head_slice(kep, h)
                band = soft.tile(
                    [P, LQ], mybir.dt.bfloat16, tag=f"band{h % 2}", name="band",
                    bufs=2,
                )
                gr_ap = bass.AP(
                    tensor=g_scr[h][it], offset=(LQ - 1) - it * P,
                    ap=[[LQ, P], [1, LQ]],
                )
                nc.sync.dma_start(band[:], gr_ap)
                pss = ps.tile([P, LQ], F32, tag="mm")
                nc.tensor.matmul(
                    pss[:], qh[:, it * P : (it + 1) * P], keh[:],
                    start=True, stop=True,
                )
                s2 = soft.tile([P, LQ], F32, tag="s2", name="s2", bufs=3)
                nc.vector.tensor_tensor(s2[:], pss[:], band[:], OP.add)
                nc.vector.tensor_tensor(s2[:], s2[:], bkebs[h][:], OP.add)
                nc.gpsimd.affine_select(
                    out=s2[:], in_=s2[:], compare_op=OP.is_ge, fill=-1e30,
                    base=it * P, pattern=[[-1, LQ]], channel_multiplier=1,
                )
                z = sm.tile([P, 1], F32, tag="z0", bufs=3)
                p_t = soft.tile(
                    [P, LQ], BF16, tag=f"p{it}{h % 2}", name="p_t", bufs=1
                )
                nc.scalar.activation(
                    p_t[:], s2[:], AF.Exp, scale=SCALE, accum_out=z[:]
                )
                nc.vector.reciprocal(z[:], z[:])
                nc.vector.tensor_scalar(p_t[:], p_t[:], z[:], None, OP.mult)
                pn[h][it] = p_t
        for h in pair:
            pt_tiles = [
                pt_pool.tile([P, LQ], BF16, tag=f"pt{jt}", name=f"pt{jt}")
                for jt in range(NT)
            ]
            for it in range(NT):
                for jt in range(it + 1):
                    tp = ps_small.tile([P, P], BF16, tag="small")
                    nc.tensor.transpose(
                        tp[:], pn[h][it][:, jt * P : (jt + 1) * P], identb[:]
                    )
                    nc.vector.tensor_copy(
                        pt_tiles[jt][:, it * P : (it + 1) * P], tp[:]
                    )
            for it in range(NT):
                pso = ps_small.tile([64, P], F32, tag="small")
                for jt in range(it + 1):
                    nc.tensor.matmul(
                        pso[:], kv_tok[jt][:, 64 * h : 64 * h + 64],
                        pt_tiles[jt][:, it * P : (it + 1) * P],
                        start=(jt == 0), stop=(jt == it),
                    )
                dst = att_q1[h // 2][
                    64 * (h % 2) : 64 * (h % 2) + 64, it * P : (it + 1) * P
                ]
                src_q = q_fm[h // 2][
                    64 * (h % 2) : 64 * (h % 2) + 64, it * P : (it + 1) * P
                ]
                nc.vector.tensor_tensor(dst, pso[:], src_q, OP.add)

    # ---- k1 / v1 (consumed by attention 1)
    k1_fm = dense_narrow(h2, "b2_W2k", HID, vecs["b2_b2k"], attA, "attA")
    v1i = []

    def v1_post(jt, psv, bias_full):
        t = kvA.tile([P, H * 65], BF16, tag="kvA")
        tv = t[:].rearrange("p (h x) -> p h x", h=H)
        nc.vector.tensor_tensor(
            tv[:, :, 0:64], psv[:].rearrange("p (h d) -> p h d", h=H),
            bias_full[:].rearrange("p (h d) -> p h d", h=H), OP.add,
        )
        nc.vector.tensor_copy(
            tv[:, :, 64:65], ones_col8[:].rearrange("p (h x) -> p h x", x=1)
        )
        v1i.append(t)

    dense_tok(h2, "b2_W2v", b2_b2v_full, v1_post)

    # ================= block1 -> y =================
    lnq1 = ln_fm(att_q1, vecs["ln1_g"], vecs["ln1_b"])
    h1 = dense_wide(lnq1, "b1_W1", vecs["b1_b1"], "h1")
    y_fm = dense_narrow(h1, "b1_W2", HID, vecs["b1_b2"], attA, "attA")

    # ================= attention 1 (transposed) =================
    att_q2 = [resid.tile([P, LQ], F32R, tag="resid", name=f"q2_{c}") for c in range(ND)]
    for h in range(H):
        yh = head_slice(y_fm, h)
        k1h = head_slice(k1_fm, h)
        p1t_tiles = []
        for jt in range(NT):
            pss = ps.tile([P, LQ], F32, tag="mm")
            nc.tensor.matmul(
                pss[:], k1h[:, jt * P : (jt + 1) * P], yh[:], start=True, stop=True
            )
            p1 = soft.tile([P, LQ], BF16, tag=f"p{jt}0", name="p1", bufs=1)
            nc.scalar.activation(
                p1[:], pss[:], AF.Exp, scale=SCALE, bias=vmadd_sb[:, jt : jt + 1]
            )
            p1t_tiles.append(p1)
        pso = ps.tile([65, LQ], F32, tag="mm")
        for jt in range(NT):
            nc.tensor.matmul(
                pso[:], v1i[jt][:, 65 * h : 65 * h + 65], p1t_tiles[jt][:],
                start=(jt == 0), stop=(jt == NT - 1),
            )
        rz = sm.tile([1, LQ], F32R, tag="rz1", bufs=2)
        nc.vector.reciprocal(rz[:], pso[64:65, :])
        psb = ps_small.tile([64, LQ], F32, tag="small")
        nc.tensor.matmul(psb[:], ones1[:, 0:64], rz[:], start=True, stop=True)
        o1 = soft.tile([P, LQ], F32, tag="o1")
        o1s = o1[64 * (h % 2) : 64 * (h % 2) + 64, :]
        nc.scalar.activation(o1s, pso[0:64, :], AF.Copy)
        nc.vector.tensor_tensor(o1s, o1s, psb[:], OP.mult)
        dst = att_q2[h // 2][64 * (h % 2) : 64 * (h % 2) + 64, :]
        src_q = att_q1[h // 2][64 * (h % 2) : 64 * (h % 2) + 64, :]
        nc.vector.tensor_tensor(dst, o1s, src_q, OP.add)

    # ================= block3 residual FFN =================
    lnq3 = ln_fm(att_q2, vecs["ln3_g"], vecs["ln3_b"])
    h3 = dense_wide(lnq3, "b3_W1", vecs["b3_b1"], "h3")

    def b3_evict(o, pso, m):
        nc.vector.scalar_tensor_tensor(
            o[:], pso[:], vecs["b3_b2"][:, m : m + 1], att_q2[m][:],
            OP.add, OP.add,
        )

    q3 = dense_narrow(h3, "b3_W2", HID, None, resid, "resid", evict=b3_evict, out_dt=F32R)

    # ---- back to token-major, store
    for it in range(NT):
        ot = act.tile([P, D], F32, tag="out_tok")
        for c in range(ND):
            tp = ps_t.tile([P, P], F32R, tag="tps")
            nc.tensor.transpose(tp[:], q3[c][:, it * P : (it + 1) * P], identity[:])
            nc.vector.tensor_copy(ot[:, c * P : (c + 1) * P], tp[:].bitcast(F32))
        nc.sync.dma_start(out[it * P : (it + 1) * P, :], ot[:])

    ctx.close()


_NC = None


def _get_nc():
    global _NC
    if _NC is None:
        _NC = build_nc()
    return _NC


def _build_in_maps(queries, values, queries_mask, values_mask, params):
    import ml_dtypes

    p = params
    f32 = np.float32
    bf16 = ml_dtypes.bfloat16

    def fmvec(v, n_tiles):
        return np.ascontiguousarray(np.asarray(v, f32).reshape(n_tiles, P).T)

    w2 = np.asarray(p["b2_W2"], f32).reshape(HID, H, 2, DIM)
    b2b = np.asarray(p["b2_b2"], f32).reshape(H, 2, DIM)

    shared = {
        "b0_W": np.asarray(p["b0_W"], bf16), "q0_W": np.asarray(p["q0_W"], bf16),
        "ke0_W": np.asarray(p["ke0_W"], bf16), "kv0_W": np.asarray(p["kv0_W"], bf16),
        "kr0_W": np.asarray(p["kr0_W"], bf16), "ab0_W": np.asarray(p["ab0_W"], bf16),
        "ab1_W": np.asarray(p["ab1_W"], bf16),
        "b1_W1": np.asarray(p["b1_W1"], bf16), "b1_W2": np.asarray(p["b1_W2"], bf16),
        "b2_W1": np.asarray(p["b2_W1"], bf16),
        "b2_W2k": np.ascontiguousarray(w2[:, :, 0, :].reshape(HID, D)).astype(bf16),
        "b2_W2v": np.ascontiguousarray(w2[:, :, 1, :].reshape(HID, D)).astype(bf16),
        "b3_W1": np.asarray(p["b3_W1"], bf16), "b3_W2": np.asarray(p["b3_W2"], bf16),
        "encR": np.ascontiguousarray(_sinusoid_ext_rev().T).astype(bf16),
        "ones_r": np.ones((1, P), f32),
        "idnb": np.eye(P, dtype=f32).astype(bf16),
    }
    vpack = np.zeros((P, VPACK_COLS), f32)
    vals = {
        "b0_b": fmvec(p["b0_b"], NH), "q0_b": fmvec(p["q0_b"], ND),
        "ke0_b": fmvec(p["ke0_b"], ND), "kr0_b": fmvec(p["kr0_b"], ND),
        "b1_b1": fmvec(p["b1_b1"], NH), "b1_b2": fmvec(p["b1_b2"], ND),
        "b2_b1": fmvec(p["b2_b1"], NH),
        "b2_b2k": fmvec(b2b[:, 0, :].reshape(D), ND),
        "b3_b1": fmvec(p["b3_b1"], NH), "b3_b2": fmvec(p["b3_b2"], ND),
        "ln0_g": fmvec(p["ln0_g"], ND), "ln0_b": fmvec(p["ln0_b"], ND),
        "ln1_g": fmvec(p["ln1_g"], ND), "ln1_b": fmvec(p["ln1_b"], ND),
        "ln2_g": fmvec(p["ln2_g"], ND), "ln2_b": fmvec(p["ln2_b"], ND),
        "ln3_g": fmvec(p["ln3_g"], ND), "ln3_b": fmvec(p["ln3_b"], ND),
        "ab0_b": np.pad(
            np.asarray(p["ab0_b"], f32).reshape(H, 1), ((0, P - H), (0, 0))
        ),
        "ab1_b": np.pad(
            np.asarray(p["ab1_b"], f32).reshape(H, 1), ((0, P - H), (0, 0))
        ),
        "epsc": np.full((P, 1), 1e-3, f32),
        "kv0_b_full": np.tile(np.asarray(p["kv0_b"], f32).reshape(1, D), (P, 1)),
        "b2_b2v_full": np.tile(b2b[:, 1, :].reshape(1, D), (P, 1)),
    }
    for nm, (c, n) in _VP.items():
        vpack[:, c : c + n] = vals[nm]
    shared["vpack"] = vpack
    rpack = np.zeros((P, RPACK_COLS), f32)
    rpack[:, _RP["idn"][0] : _RP["idn"][0] + P] = np.eye(P, dtype=f32)
    rpack[:, _RP["ones_r"][0]] = 1.0
    rpack[:, _RP["ones_c"][0]] = 1.0
    rpack[:, _RP["ones8"][0] : _RP["ones8"][0] + H] = 1.0
    shared["rpack"] = rpack

    qm = np.asarray(queries_mask, f32)
    vm = np.asarray(values_mask, f32)
    in_maps = []
    for c in range(B):
        m = dict(shared)
        m["q_in"] = np.ascontiguousarray(np.asarray(queries[c], f32))
        m["v_in"] = np.ascontiguousarray(np.asarray(values[c], f32))
        m["qmadd"] = ((qm[c] - 1.0) * 8e9).reshape(1, LQ).astype(f32)
        m["vmadd"] = np.ascontiguousarray(
            ((vm[c] - 1.0) * 1e9).reshape(NT, P).T
        ).astype(f32)
        in_maps.append(m)
    return in_maps


def kernel(queries, values, queries_mask, values_mask, params):
    in_maps = _build_in_maps(queries, values, queries_mask, values_mask, params)
    nc = _get_nc()
    res = run_bass_kernel_spmd(nc, in_maps, core_ids=list(range(B)))
    return np.stack([res.results[c]["out"] for c in range(B)], axis=0).astype(
        np.float32
    )


def kernel_profiled(queries, values, queries_mask, values_mask, params,
                    tmpdir=None):
    """Same as kernel() but with NTFF tracing; returns (output, results)."""
    import kernel as _self  # works both as module and as __main__ helper

    in_maps = _build_in_maps(queries, values, queries_mask, values_mask, params)
    nc = _get_nc()
    res = run_bass_kernel_spmd(
        nc, in_maps, core_ids=list(range(B)), trace=True, tmpdir=tmpdir
    )
    out = np.stack([res.results[c]["out"] for c in range(B)], axis=0).astype(
        np.float32
    )
    return out, res
